# revision 26
# baseline (speedup 1.0000x reference)
"""Local (banded) attention kernel for Trainium2, 8 NeuronCores SPMD.

Problem: nn_LocalAttention  (B=4, S=2048, D=512, H=8 heads, DK=64, band W=16)
  out = (softmax(band_mask(QK^T/sqrt(DK))) V) Wo + bo   with Q/K/V = x W* + b*

Sharding: 8 cores = 4 batches x 2 sequence halves. Each core computes its
1024-query slice end-to-end. K/V get a 16-row halo (zero-padded at sequence
ends) so no inter-core attention communication is needed.

The measured metric is the end-to-end wall time of one full execution
(upload + NEFF exec + download) over the axon tunnel, which serializes all
RPC + data bytes in both directions at ~50MB/s peak (~19.6ms/MB floor even
for all-zero payloads, plus a content-entropy-dependent extra of up to
~8ms/MB; transfers do NOT overlap each other or exec, and each *blocked*
RPC costs a ~80ms round trip that async dispatch hides). On-device compute
is a few ms. The kernel therefore minimizes moved bytes and round trips:
  - Custom async PJRT runner (_make_runner): same _bass_exec_p path as
    bass2jax.run_bass_via_pjrt, but (a) the donated output buffers are
    created ON DEVICE by a tiny separate jit (saves the 4.2MB host zeros
    upload, ~100ms), and (b) zeros jit + sharded device_put + NEFF call are
    all issued async with only the final np.asarray blocking (hides ~3
    RPC round trips, ~150ms). One device_put of one concatenated global
    array = one streamed transfer (8 separate per-device puts pay ~45ms
    fixed cost EACH and serialize).
  - Q/K/V uploaded int8 with per-column scales folded into the weights on
    the host (X_INT8; per-tensor scales fail the 2e-2 gate, per-column
    passes). X_INT8=False falls back to fp16 (rel ~7e-4).
  - Weights uploaded once as 1/8 shards, 12-bit packed (W12: per row 512
    low bytes + 256 shared hi-nibble bytes for cols j/j+256, plus per-
    column scales; <= f16 abs error), AllGather'd and unpacked on device
    with int16 DVE bit ops; scales are applied per-partition at the
    projection psum-copy (Wq/Wk, per-d_out), the xvT upcast (Wv, per-d_in)
    and the ctxT copy (Wo, per-d_in). 1.69MB vs 2.10MB f16.
  - Band mask is an inline NEFF constant; sequence-edge validity is a tiny
    per-core [NQT,128] "vones" vector that becomes the fused-denominator
    column of V (replaces the 264KB/core mask upload).
  - Output is int8 with a per-row f32 scale packed into the last 4 bytes of
    each row (OUT_INT8; halves the download vs fp16).
  - All inputs ride in ONE int8 blob per core; host packing (quantize,
    transpose, blob assembly) is threaded (ThreadPoolExecutor) since it is
    wall-time before the timed region.
  - jax persistent compilation cache turns recompiles into disk hits.
Measured: 2.655s original -> 0.560s staged baseline -> ~0.406s
(best-of-3; run-to-run noise +-15ms), rel err 1.761e-2 (gate 2e-2).

Per-core device pipeline (fp16 operands, f32 psum):
  - int8 x tiles upcast to fp16 on DVE (values <=127 are exact in fp16).
  - QT = Wq^T @ XqT -> [64,1024] per head; KT likewise [64,1056].
  - V window-major [kpos, 8, 65]; col 64 = vones (validity) -> fused softmax
    denominator that automatically excludes padded keys.
  - Per q-tile (96 queries, 128-key window) and head:
      scoresT = KT_win^T.QT_tile (psum f32); attnT = exp(scoresT) (ACT, f16)
      attnT *= band (gpsimd, inline 0/1 const, broadcast over heads)
      ctx_aug = attnT^T.V_aug (PE); ctx = ctx_aug[:,:64]/den (DVE reciprocal)
      ctxT via PE-transpose -> [512,1024]
  - out = ctxT^T.Wo (+bo) -> [1024,512], per-row absmax/127 int8 quantize,
    scale bitcast into out[:, 512:516] -> DRAM.
"""

import os
import sys

for _p in ("/opt/trn_rl_repo", "/root/.axon_site/_ro/trn_rl_repo"):
    if os.path.isdir(_p) and _p not in sys.path:
        sys.path.insert(0, _p)
        break

import numpy as np
import ml_dtypes

# Persist compiled PJRT executables across calls: run_bass_kernel_spmd builds a
# fresh jit closure per call, so without this every call re-lowers/recompiles
# the identical program (~0.2s) before transferring anything.
try:
    import tempfile

    import jax

    _cache_dir = os.path.join(
        tempfile.gettempdir(), f"jax_comp_cache_{os.getuid()}")
    jax.config.update("jax_compilation_cache_dir", _cache_dir)
    jax.config.update("jax_persistent_cache_min_entry_size_bytes", -1)
    jax.config.update("jax_persistent_cache_min_compile_time_secs", 0.0)
except Exception:
    pass

import concourse.bass as bass
import concourse.tile as tile
from concourse import bacc, bass2jax, mybir

BF16 = ml_dtypes.bfloat16
F16 = np.float16

B, S, D, H, W = 4, 2048, 512, 8, 16
DK = D // H          # 64
NCORES = 8
SH = S // 2          # 1024 rows per core
PADK = SH + 2 * W    # 1056 padded key rows
QT = 96              # q-tile size
NQT = (SH + QT - 1) // QT   # 11 tiles (last = 64)
WIN = QT + 2 * W     # 128-key window per q-tile
SCALE = 1.0 / np.sqrt(DK)
WROWS = 4 * D        # 2048 stacked weight rows
WSH = WROWS // NCORES  # 256 rows per core shard

X_INT8 = True        # upload Q/K/V as int8 (per-column scales folded into W)
OUT_INT8 = True      # download output as int8 + per-row f32 scales
W12 = True           # pack weights as 12-bit planes (vs f16) in the blob

# single-blob input layout (int8-x mode): one ExternalInput array per core.
# Weights ride as 12-bit packed planes: per row, 512 low bytes + 256 shared
# hi-nibble bytes (cols j and j+256 share one byte), plus 2048 f32 scales
# (per-d_out for Wq/Wk, per-d_in for Wv/Wo) duplicated in every core's blob.
SXQ = D * SH          # 524288   xqT int8 [512, 1024]
SXK = D * PADK        # 540672   xkT int8 [512, 1056]
WPACK = D + D // 2    # 768 packed bytes per weight row
OFF_XQ = 0
OFF_XK = OFF_XQ + SXQ
OFF_XV = OFF_XK + SXK
OFF_W = OFF_XV + SXK            # packed weight shard [256, 768] int8
if W12:
    OFF_SC = OFF_W + WSH * WPACK    # weight scales f32 [4*D]
    OFF_V = OFF_SC + 4 * D * 4      # vones int8 [NQT, 128]
else:
    OFF_SC = None
    OFF_V = OFF_W + WSH * D * 2     # f16 weight shard [256, 512]
BLOB = OFF_V + NQT * 128        # 1811840 bytes (W12) / 1869184 (f16)

TRACE = False        # set True (from test.py) to collect an NTFF profile
LAST = {}            # stash for exec_time_ns / profile info

from concurrent.futures import ThreadPoolExecutor

_host_pool = ThreadPoolExecutor(max_workers=8)   # numpy packing parallelism

_programs = {}       # (x_int8, out_int8, has_b) -> compiled nc


def _emit(nc, tc, pools, dram, x_int8, out_int8, has_b):
    dt = mybir.dt
    f16, f32, i8 = dt.float16, dt.float32, dt.int8
    consts, work, psA, psB, psC = pools
    out_d = dram["out"]

    def blob_ap(off, pattern):
        b0 = dram["blob"][0:1]
        return bass.AP(tensor=b0.tensor, offset=off, ap=pattern)

    # ---- weights: bounce -> AllGather -> SBUF ----------------------------
    if x_int8 and W12:
        wch_src = blob_ap(OFF_W, [[WPACK, WSH], [1, WPACK]])
    elif x_int8:
        wch_src = blob_ap(OFF_W, [[D * 2, WSH], [1, D * 2]]).bitcast(f16)
    else:
        wch_src = dram["wchunk"][:, :]
    nc.sync.dma_start(out=dram["wch_b"][:, :], in_=wch_src)
    nc.gpsimd.collective_compute(
        "AllGather",
        mybir.AluOpType.bypass,
        replica_groups=[list(range(NCORES))],
        ins=[dram["wch_b"].ap().opt()],
        outs=[dram["wfull"].ap().opt()],
    )
    sc_sb = None
    w_sb = {}
    if x_int8 and W12:
        # per-chunk weight scales [128, 16]; col 4i+k = scales[512i + 128k + p]
        sc_sb = consts.tile([128, 16], f32, tag="wsc")
        nc.sync.dma_start(
            out=sc_sb[:],
            in_=blob_ap(OFF_SC, [[64, 128], [1, 64]]).bitcast(f32),
        )
        i16 = dt.int16
        for i, name in enumerate(("wq", "wk", "wv", "wo")):
            w_sb[name] = []
            for k in range(4):
                r0 = D * i + 128 * k
                lo8 = work.tile([128, D], i8, tag="wlo8")
                hi8 = work.tile([128, D // 2], i8, tag="whi8")
                nc.sync.dma_start(out=lo8[:], in_=dram["wfull"][r0:r0 + 128, 0:D])
                nc.sync.dma_start(out=hi8[:], in_=dram["wfull"][r0:r0 + 128, D:WPACK])
                lo16 = work.tile([128, D], i16, tag="wlo16")
                hi16 = work.tile([128, D // 2], i16, tag="whi16")
                ev16 = work.tile([128, D // 2], i16, tag="wev16")
                od16 = work.tile([128, D // 2], i16, tag="wod16")
                nc.vector.tensor_copy(out=lo16[:], in_=lo8[:])
                nc.vector.tensor_scalar(
                    out=lo16[:], in0=lo16[:], scalar1=255, scalar2=None,
                    op0=mybir.AluOpType.bitwise_and)
                nc.vector.tensor_copy(out=hi16[:], in_=hi8[:])
                nc.vector.tensor_scalar(
                    out=hi16[:], in0=hi16[:], scalar1=255, scalar2=None,
                    op0=mybir.AluOpType.bitwise_and)
                nc.vector.tensor_scalar(
                    out=ev16[:], in0=hi16[:], scalar1=15, scalar2=8,
                    op0=mybir.AluOpType.bitwise_and,
                    op1=mybir.AluOpType.logical_shift_left)
                nc.vector.tensor_scalar(
                    out=od16[:], in0=hi16[:], scalar1=4, scalar2=8,
                    op0=mybir.AluOpType.logical_shift_right,
                    op1=mybir.AluOpType.logical_shift_left)
                v16 = work.tile([128, D], i16, tag="wv16")
                h = D // 2
                nc.vector.tensor_add(out=v16[:, 0:h], in0=lo16[:, 0:h], in1=ev16[:])
                nc.vector.tensor_add(out=v16[:, h:D], in0=lo16[:, h:D], in1=od16[:])
                nc.vector.tensor_scalar_add(out=v16[:], in0=v16[:], scalar1=-2048)
                t = consts.tile([128, D], f16, tag=f"{name}{k}")
                nc.vector.tensor_copy(out=t[:], in_=v16[:])
                w_sb[name].append(t)
    else:
        for i, name in enumerate(("wq", "wk", "wv", "wo")):
            w_sb[name] = []
            for k in range(4):
                t = consts.tile([128, D], f16, tag=f"{name}{k}")
                r0 = D * i + 128 * k
                nc.sync.dma_start(out=t[:], in_=dram["wfull"][r0:r0 + 128, :])
                w_sb[name].append(t)

    # ---- load x (fp16 direct, or int8-from-blob + DVE upcast) ------------
    def load_xt(key, off, ncols, sc0=None):
        tiles = []
        for k in range(4):
            if x_int8:
                t8 = consts.tile([128, ncols], i8, tag=f"{key}{k}i8")
                nc.sync.dma_start(
                    out=t8[:],
                    in_=blob_ap(off + 128 * k * ncols, [[ncols, 128], [1, ncols]]),
                )
                t = consts.tile([128, ncols], f16, tag=f"{key}{k}")
                nc.vector.tensor_copy(out=t[:], in_=t8[:])
                if sc0 is not None:
                    # fold Wv's per-d_in 12-bit scale into the upcast
                    nc.vector.tensor_scalar_mul(
                        out=t[:], in0=t[:],
                        scalar1=sc_sb[:, sc0 + k:sc0 + k + 1])
            else:
                t = consts.tile([128, ncols], f16, tag=f"{key}{k}")
                nc.sync.dma_start(out=t[:], in_=dram[key][128 * k:128 * (k + 1), :])
            tiles.append(t)
        return tiles

    xqt_sb = load_xt("xqt", OFF_XQ, SH)
    xkt_sb = load_xt("xkt", OFF_XK, PADK)
    xvt_sb = load_xt("xvt", OFF_XV, PADK, sc0=8 if (x_int8 and W12) else None)

    vones_sb = consts.tile([128, NQT], f32, tag="vones")
    if x_int8:
        # vones int8 [NQT, 128] in the blob; partition-first AP transposes
        v8 = consts.tile([128, NQT], i8, tag="vones8")
        nc.sync.dma_start(out=v8[:], in_=blob_ap(OFF_V, [[1, 128], [128, NQT]]))
        nc.vector.tensor_copy(out=vones_sb[:], in_=v8[:])
    else:
        nc.sync.dma_start(
            out=vones_sb[:], in_=dram["vones"].ap().rearrange("t p -> p t"))

    band_sb = consts.tile([128, QT], f16, tag="band")
    nc.sync.dma_start(out=band_sb[:], in_=dram["band"][:])
    ident_sb = consts.tile([QT, QT], f16, tag="ident")
    nc.sync.dma_start(out=ident_sb[:], in_=dram["ident"][:])

    bq_sb = bk_sb = bv_sb = bo_sb = None
    if has_b:
        bq_sb = consts.tile([128, 4], f32, tag="bq")
        nc.sync.dma_start(out=bq_sb[:], in_=dram["bqc"].ap().rearrange("c p -> p c"))
        bk_sb = consts.tile([128, 4], f32, tag="bk")
        nc.sync.dma_start(out=bk_sb[:], in_=dram["bkc"].ap().rearrange("c p -> p c"))
        bv_sb = consts.tile([128, D], f32, tag="bv")
        nc.sync.dma_start(out=bv_sb[:], in_=dram["bvb"][:])
        bo_sb = consts.tile([128, D], f32, tag="bo")
        nc.sync.dma_start(out=bo_sb[:], in_=dram["bob"][:])

    # ---- Q/K projections -> per-head QT [64, SH], KT [64, PADK] (f16) ----
    # Per-head tiles keep every matmul operand at partition offset 0: the HW
    # crashes on (partition-offset operand + intra-bank psum write offset).
    qt_sb, kt_sb = [], []
    for h in range(H):
        qt_sb.append(consts.tile([64, SH], f16, tag=f"qt{h}", name=f"qt{h}"))
        kt_sb.append(consts.tile([64, PADK], f16, tag=f"kt{h}", name=f"kt{h}"))

    def project_T(xt_sb, w, out_tiles, bias_sb, ncols, sc0):
        # head 2m / 2m+1 live in rows 0:64 / 64:128 of dout-chunk m
        for m in range(4):
            c0 = 0
            while c0 < ncols:
                cw = min(512, ncols - c0)
                ps = psA.tile([128, 512], f32, tag="big")
                for k in range(4):
                    nc.tensor.matmul(
                        ps[:, :cw],
                        lhsT=w[k][:, 128 * m:128 * (m + 1)],
                        rhs=xt_sb[k][:, c0:c0 + cw],
                        start=(k == 0),
                        stop=(k == 3),
                    )
                for half in range(2):
                    r0, r1 = 64 * half, 64 * half + 64
                    dst = out_tiles[2 * m + half][:, c0:c0 + cw]
                    src = ps[r0:r1, :cw]
                    if sc0 is not None:
                        # 12-bit weights: scale rows by per-d_out scale
                        # (optionally fused with bias add)
                        if has_b:
                            nc.vector.tensor_scalar(
                                out=dst, in0=src,
                                scalar1=sc_sb[r0:r1, sc0 + m:sc0 + m + 1],
                                scalar2=bias_sb[r0:r1, m:m + 1],
                                op0=mybir.AluOpType.mult,
                                op1=mybir.AluOpType.add,
                            )
                        else:
                            nc.vector.tensor_scalar_mul(
                                out=dst, in0=src,
                                scalar1=sc_sb[r0:r1, sc0 + m:sc0 + m + 1],
                            )
                    elif has_b:
                        nc.vector.tensor_scalar_add(
                            out=dst, in0=src,
                            scalar1=bias_sb[r0:r1, m:m + 1],
                        )
                    else:
                        nc.vector.tensor_copy(out=dst, in_=src)
                c0 += cw

    wsc0 = 0 if (x_int8 and W12) else None
    project_T(xqt_sb, w_sb["wq"], qt_sb, bq_sb, SH, wsc0)
    project_T(xkt_sb, w_sb["wk"], kt_sb, bk_sb, PADK,
              4 if (x_int8 and W12) else None)

    # ---- V projection, window-major; col 64 = vones (validity) -----------
    v_sb = []
    for t in range(NQT):
        w0 = QT * t
        wr = min(WIN, PADK - w0)
        vt = consts.tile([128, H, DK + 1], f16, tag=f"v{t}")
        v_sb.append(vt)
        ps = psA.tile([128, 512], f32, tag="big")
        for k in range(4):
            nc.tensor.matmul(
                ps[:wr, :],
                lhsT=xvt_sb[k][:, w0:w0 + wr],
                rhs=w_sb["wv"][k][:],
                start=(k == 0),
                stop=(k == 3),
            )
        src = ps[:wr, :].rearrange("p (h x) -> p h x", h=H)
        if has_b:
            bvv = bv_sb[:wr, :].rearrange("p (h x) -> p h x", h=H)
            nc.vector.tensor_add(out=vt[:wr, :, 0:DK], in0=src, in1=bvv)
            # zero out padded-key rows so bias doesn't leak into the band sum
            nc.vector.tensor_scalar_mul(
                out=vt[:wr, :, 0:DK],
                in0=vt[:wr, :, 0:DK],
                scalar1=vones_sb[:wr, t:t + 1],
            )
        else:
            nc.vector.tensor_copy(out=vt[:wr, :, 0:DK], in_=src)
        vb = vones_sb[:wr, t:t + 1]
        vb_bc = bass.AP(
            tensor=vb.tensor, offset=vb.offset,
            ap=[vb.ap[0], [0, H], vb.ap[1]],
        )
        nc.vector.tensor_copy(out=vt[:wr, :, DK:DK + 1], in_=vb_bc)

    # ---- attention -------------------------------------------------------
    ctxT_sb = []
    for c in range(4):
        ctxT_sb.append(consts.tile([128, SH], f16, tag=f"ctxT{c}", name=f"ctxT{c}"))

    head_groups = ((0, 5), (5, 8))
    for t in range(NQT):
        q0 = QT * t
        qw = min(QT, SH - q0)
        w0 = QT * t
        wr = min(WIN, PADK - w0)

        attn_sb = work.tile([128, H, QT], f16, tag="attn")
        for h0, h1 in head_groups:
            nh = h1 - h0
            ps_sc = psB.tile([128, 5, QT], f32, tag="sc")
            for j, h in enumerate(range(h0, h1)):
                nc.tensor.matmul(
                    ps_sc[:wr, j, :qw],
                    lhsT=kt_sb[h][:, w0:w0 + wr],
                    rhs=qt_sb[h][:, q0:q0 + qw],
                    start=True,
                    stop=True,
                )
            nc.scalar.activation(
                out=attn_sb[:wr, h0:h1, :qw],
                in_=ps_sc[:wr, :nh, :qw],
                func=mybir.ActivationFunctionType.Exp,
            )

        # multiplicative band mask, broadcast over heads (gpsimd)
        mbase = band_sb[:wr, :qw]
        mask_bc = bass.AP(
            tensor=mbase.tensor, offset=mbase.offset,
            ap=[mbase.ap[0], [0, H], mbase.ap[1]],
        )
        nc.gpsimd.tensor_mul(
            out=attn_sb[:wr, :, :qw], in0=attn_sb[:wr, :, :qw], in1=mask_bc
        )

        recip_sb = work.tile([QT, H], f32, tag="recip")
        ctx_sb = work.tile([QT, H, DK], f16, tag="ctx")
        for g in range(2):
            ps_ctx = psC.tile([QT, 4, DK + 1], f32, tag="ctx")
            for j, h in enumerate(range(4 * g, 4 * g + 4)):
                nc.tensor.matmul(
                    ps_ctx[:qw, j, :],
                    lhsT=attn_sb[:wr, h, :qw],
                    rhs=v_sb[t][:wr, h, :],
                    start=True,
                    stop=True,
                )
            nc.vector.reciprocal(
                out=recip_sb[:qw, 4 * g:4 * g + 4],
                in_=ps_ctx[:qw, :, DK:DK + 1],
            )
            rbase = recip_sb[:qw, 4 * g:4 * g + 4]
            recip_bc = bass.AP(
                tensor=rbase.tensor, offset=rbase.offset,
                ap=[rbase.ap[0], rbase.ap[1], [0, DK]],
            )
            nc.vector.tensor_mul(
                out=ctx_sb[:qw, 4 * g:4 * g + 4, :],
                in0=ps_ctx[:qw, :, 0:DK],
                in1=recip_bc,
            )

        # transpose ctx [qw, 512] -> ctxT [512, qw]  (4 chunks of 128)
        for c in range(4):
            ps_t = psA.tile([128, QT], f16, tag="big")
            nc.tensor.transpose(
                out=ps_t[:, :qw],
                in_=ctx_sb[:qw, 2 * c:2 * c + 2, :],
                identity=ident_sb[:qw, :qw],
            )
            if x_int8 and W12:
                # fold Wo's per-d_in 12-bit scale into the ctxT copy
                nc.vector.tensor_scalar_mul(
                    out=ctxT_sb[c][:, q0:q0 + qw], in0=ps_t[:, :qw],
                    scalar1=sc_sb[:, 12 + c:12 + c + 1])
            else:
                nc.vector.tensor_copy(
                    out=ctxT_sb[c][:, q0:q0 + qw], in_=ps_t[:, :qw])

    # ---- O-projection ----------------------------------------------------
    for mt in range(8):
        r0 = 128 * mt
        ps = psA.tile([128, 512], f32, tag="big")
        for k in range(4):
            nc.tensor.matmul(
                ps[:],
                lhsT=ctxT_sb[k][:, r0:r0 + 128],
                rhs=w_sb["wo"][k][:],
                start=(k == 0),
                stop=(k == 3),
            )
        src = ps[:]
        if has_b:
            of_sb = work.tile([128, D], f32, tag="osbf")
            nc.vector.tensor_add(out=of_sb[:], in0=ps[:], in1=bo_sb[:])
            src = of_sb[:]
        if out_int8:
            # per-row int8 quantization; scale = absmax/127 rides in the last
            # 4 bytes of each int8 output row (bitcast f32)
            amax_sb = work.tile([128, 1], f32, tag="amax")
            osc_sb = work.tile([128, 1], f32, tag="osc")
            rsc_sb = work.tile([128, 1], f32, tag="rsc")
            o_sb = work.tile([128, D], dt.int8, tag="osb8")
            nc.vector.tensor_reduce(
                out=amax_sb[:], in_=src,
                axis=mybir.AxisListType.X, op=mybir.AluOpType.max,
                apply_absolute_value=True,
            )
            nc.vector.tensor_scalar_max(out=amax_sb[:], in0=amax_sb[:], scalar1=1e-30)
            nc.vector.tensor_scalar_mul(out=osc_sb[:], in0=amax_sb[:], scalar1=1.0 / 127.0)
            nc.vector.reciprocal(out=rsc_sb[:], in_=osc_sb[:])
            nc.vector.tensor_scalar_mul(out=o_sb[:], in0=src, scalar1=rsc_sb[:, 0:1])
            nc.sync.dma_start(out=out_d[r0:r0 + 128, 0:D], in_=o_sb[:])
            nc.sync.dma_start(
                out=out_d[r0:r0 + 128, D:D + 4].bitcast(f32), in_=osc_sb[:]
            )
        else:
            o_sb = work.tile([128, D], f16, tag="osb")
            nc.vector.tensor_copy(out=o_sb[:], in_=src)
            nc.sync.dma_start(out=out_d[r0:r0 + 128, :], in_=o_sb[:])


def _build_band() -> np.ndarray:
    i = np.arange(128)[:, None]   # window row (key)
    j = np.arange(QT)[None, :]    # q column
    band = (i - j >= 0) & (i - j <= 2 * W)
    return band.astype(F16)


def _build_program(x_int8: bool, out_int8: bool, has_b: bool):
    dt = mybir.dt
    f16, f32 = dt.float16, dt.float32
    xdt = dt.int8 if x_int8 else f16
    odt = dt.int8 if out_int8 else f16

    nc = bacc.Bacc("TRN2", target_bir_lowering=False, debug=False, num_devices=NCORES)

    dram = {}
    if x_int8:
        dram["blob"] = nc.dram_tensor("blob", [BLOB], dt.int8, kind="ExternalInput")
    else:
        dram["xqt"] = nc.dram_tensor("xqt", [D, SH], xdt, kind="ExternalInput")
        dram["xkt"] = nc.dram_tensor("xkt", [D, PADK], xdt, kind="ExternalInput")
        dram["xvt"] = nc.dram_tensor("xvt", [D, PADK], xdt, kind="ExternalInput")
        dram["wchunk"] = nc.dram_tensor("wchunk", [WSH, D], f16, kind="ExternalInput")
        dram["vones"] = nc.dram_tensor("vones", [NQT, 128], f32, kind="ExternalInput")
    dram.update({
        "out": nc.dram_tensor(
            "out", [SH, D + 4] if out_int8 else [SH, D], odt, kind="ExternalOutput"),
        "band": nc.inline_tensor(_build_band(), name="band"),
        "ident": nc.inline_tensor(np.eye(QT, dtype=F16), name="ident"),
    })
    if x_int8 and W12:
        dram["wch_b"] = nc.dram_tensor("wch_b", [WSH, WPACK], dt.int8)
        dram["wfull"] = nc.dram_tensor("wfull", [WROWS, WPACK], dt.int8)
    else:
        dram["wch_b"] = nc.dram_tensor("wch_b", [WSH, D], f16)
        dram["wfull"] = nc.dram_tensor("wfull", [WROWS, D], f16)
    if has_b:
        dram["bqc"] = nc.dram_tensor("bqc", [4, 128], f32, kind="ExternalInput")
        dram["bkc"] = nc.dram_tensor("bkc", [4, 128], f32, kind="ExternalInput")
        dram["bvb"] = nc.dram_tensor("bvb", [128, D], f32, kind="ExternalInput")
        dram["bob"] = nc.dram_tensor("bob", [128, D], f32, kind="ExternalInput")

    with tile.TileContext(nc) as tc:
        with (
            tc.tile_pool(name="consts", bufs=1) as consts,
            tc.tile_pool(name="work", bufs=3) as work,
            tc.tile_pool(name="psA", bufs=2, space="PSUM") as psA,
            tc.tile_pool(name="psB", bufs=2, space="PSUM") as psB,
            tc.tile_pool(name="psC", bufs=4, space="PSUM") as psC,
        ):
            _emit(nc, tc, (consts, work, psA, psB, psC), dram, x_int8, out_int8, has_b)

    nc.compile()
    return nc


def _get_program(x_int8, out_int8, has_b):
    key = (x_int8, out_int8, has_b)
    if key not in _programs:
        _programs[key] = _build_program(x_int8, out_int8, has_b)
    return _programs[key]


# ---------------------------------------------------------------------------
# Custom PJRT runner. Same _bass_exec_p path as bass2jax.run_bass_via_pjrt,
# with two wall-time fixes (the measured metric is transfer-bound over the
# axon tunnel, which serializes all RPC + data bytes in both directions):
#   - the donated output buffers are created ON DEVICE by a separate tiny jit
#     (jnp.zeros + out_shardings) instead of uploading host np.zeros — saves
#     the 4.2MB zero upload (~100ms). They can't be created inside the same
#     jit: neuronx_cc_hook requires every bass_exec operand to be a plain HLO
#     parameter.
#   - everything is issued async (zeros jit, sharded device_put, NEFF call)
#     and only the final np.asarray blocks, so per-RPC round-trip latencies
#     (~60-80ms each when blocked individually) overlap with the data stream.
# ---------------------------------------------------------------------------

_runners = {}


def _make_runner(nc):
    import jax
    import jax.numpy as jnp
    from jax.sharding import Mesh, NamedSharding, PartitionSpec
    import warnings
    with warnings.catch_warnings():
        warnings.simplefilter("ignore")
        from jax.experimental.shard_map import shard_map

    bass2jax.install_neuronx_cc_hook()
    partition_name = nc.partition_id_tensor.name if nc.partition_id_tensor else None
    in_names, out_names, out_avals = [], [], []
    for alloc in nc.m.functions[0].allocations:
        if not isinstance(alloc, mybir.MemoryLocationSet):
            continue
        name = alloc.memorylocations[0].name
        if alloc.kind == "ExternalInput":
            if name != partition_name:
                in_names.append(name)
        elif alloc.kind == "ExternalOutput":
            out_names.append(name)
            out_avals.append(
                jax.core.ShapedArray(
                    tuple(alloc.tensor_shape), mybir.dt.np(alloc.dtype)))
    n_params = len(in_names)
    n_outs = len(out_avals)
    in_names_all = in_names + out_names
    if partition_name is not None:
        in_names_all.append(partition_name)
    donate = tuple(range(n_params, n_params + n_outs))

    def _body(*args):
        operands = list(args)
        if partition_name is not None:
            operands.append(bass2jax.partition_id_tensor())
        outs = bass2jax._bass_exec_p.bind(
            *operands,
            out_avals=tuple(out_avals),
            in_names=tuple(in_names_all),
            out_names=tuple(out_names),
            lowering_input_output_aliases=(),
            sim_require_finite=True,
            sim_require_nnan=True,
            nc=nc,
        )
        return tuple(outs)

    devices = jax.devices()[:NCORES]
    mesh = Mesh(np.asarray(devices), ("core",))
    spec = NamedSharding(mesh, PartitionSpec("core"))
    in_specs = (PartitionSpec("core"),) * (n_params + n_outs)
    out_specs = (PartitionSpec("core"),) * n_outs
    sharded = jax.jit(
        shard_map(_body, mesh=mesh, in_specs=in_specs, out_specs=out_specs,
                  check_rep=False),
        donate_argnums=donate, keep_unused=True)
    gshapes = [(NCORES * a.shape[0], *a.shape[1:]) for a in out_avals]
    zeros_fn = jax.jit(
        lambda: tuple(jnp.zeros(s, a.dtype) for s, a in zip(gshapes, out_avals)),
        out_shardings=(spec,) * n_outs)

    def run(concat_in):
        """concat_in: list of global [NCORES*rows, ...] arrays in in_names
        order. Returns list of global output arrays (np, gathered)."""
        z = zeros_fn()                                   # async, device-side
        g_in = [jax.device_put(a, spec) for a in concat_in]   # async upload
        outs = sharded(*g_in, *z)                        # async NEFF exec
        return [np.asarray(o) for o in outs]             # blocks

    run.in_names = in_names
    return run


def _get_runner(nc):
    if id(nc) not in _runners:
        _runners[id(nc)] = _make_runner(nc)
    return _runners[id(nc)]


def _build_vones(half: int) -> np.ndarray:
    # vones[t, i] = 1.0 iff padded K/V row (96t + i) holds a real key
    v = np.zeros((NQT, 128), np.float32)
    r = QT * np.arange(NQT)[:, None] + np.arange(128)[None, :]
    lo, hi = (W, PADK) if half == 0 else (0, PADK - W)
    v[:] = ((r >= lo) & (r < hi)).astype(np.float32)
    return v


_vones_cache = {}


def kernel(query, key, value, Wq, bq, Wk, bk, Wv, bv, Wo, bo):
    query = np.asarray(query, np.float32)
    key = np.asarray(key, np.float32)
    value = np.asarray(value, np.float32)
    Wq = np.asarray(Wq, np.float32)
    Wk = np.asarray(Wk, np.float32)
    Wv = np.asarray(Wv, np.float32)
    Wo = np.asarray(Wo, np.float32)
    bq = np.asarray(bq, np.float32)
    bk = np.asarray(bk, np.float32)
    bv = np.asarray(bv, np.float32)
    bo = np.asarray(bo, np.float32)

    has_b = bool(np.any(bq) or np.any(bk) or np.any(bv) or np.any(bo))
    x_int8 = X_INT8
    out_int8 = OUT_INT8
    nc = _get_program(x_int8, out_int8, has_b)

    if x_int8:
        # per-column int8 scales, folded into the weight rows on the host

        def colmax(x):
            return np.maximum(np.abs(x).max(axis=(0, 1)) / 127.0, 1e-30)

        def quant(x, s):
            # s = absmax/127 bounds |x/s| <= 127 (+1 ulp, absorbed by rint),
            # so no clip pass is needed
            t = x * (1.0 / s).astype(np.float32)
            np.rint(t, out=t)
            return t.astype(np.int8)

        sq, sk, sv = _host_pool.map(colmax, (query, key, value))
        qx, kx, vx = _host_pool.map(
            lambda a: quant(*a), ((query, sq), (key, sk), (value, sv)))
        wq_f = Wq * (sq[:, None] * SCALE)
        wk_f = Wk * sk[:, None]
        wv_f = Wv * sv[:, None]
        xdt = np.int8
    else:
        wq_f = Wq * SCALE
        wk_f = Wk
        wv_f = Wv
        qx, kx, vx = query.astype(F16), key.astype(F16), value.astype(F16)
        xdt = F16

    if x_int8 and W12:
        # 12-bit weight quantization: Wq/Wk per-column (d_out, scale applied
        # on the projection psum rows), Wv/Wo per-row (d_in, scale folded
        # into the xvT upcast / ctxT copy respectively)
        def q12(w, axis):
            s = np.maximum(np.abs(w).max(axis=axis), 1e-30) / 2047.0
            q = np.rint(w / (s[None, :] if axis == 0 else s[:, None]))
            return q.astype(np.int32), s.astype(np.float32)
        qq, sq_w = q12(wq_f, 0)
        qk, sk_w = q12(wk_f, 0)
        qv, sv_w = q12(wv_f, 1)
        qo, so_w = q12(Wo, 1)
        u = (np.concatenate([qq, qk, qv, qo], axis=0) + 2048).astype(np.uint16)
        lo = (u & 255).astype(np.uint8)
        hi4 = (u >> 8).astype(np.uint8)
        hi = hi4[:, :D // 2] | (hi4[:, D // 2:] << 4)
        wpacked = np.concatenate([lo, hi], axis=1).view(np.int8)  # [2048, 768]
        # transposed [p, chunk] layout so the device DMA reads contiguous
        # 64-byte runs per partition: wscales_t[p*16 + c] = s[128c + p]
        wscales = np.ascontiguousarray(
            np.concatenate([sq_w, sk_w, sv_w, so_w]).reshape(16, 128).T
        ).reshape(-1).view(np.int8)
        wstack = None
    else:
        wstack = np.ascontiguousarray(
            np.concatenate([wq_f, wk_f, wv_f, Wo], axis=0).astype(F16))

    if not _vones_cache:
        _vones_cache[0] = _build_vones(0)
        _vones_cache[1] = _build_vones(1)

    globals_by_name = {}
    if x_int8:
        gblob = np.empty((NCORES, BLOB), np.int8)
        globals_by_name["blob"] = gblob
    else:
        globals_by_name["xqt"] = np.empty((NCORES * D, SH), F16)
        globals_by_name["xkt"] = np.empty((NCORES * D, PADK), F16)
        globals_by_name["xvt"] = np.empty((NCORES * D, PADK), F16)
        globals_by_name["wchunk"] = np.empty((NCORES * WSH, D), F16)
        globals_by_name["vones"] = np.empty((NCORES * NQT, 128), np.float32)
    def pack_core(core):
        b, half = core // 2, core % 2
        s0 = half * SH
        xq = qx[b, s0:s0 + SH]
        lo, hi = s0 - W, s0 + SH + W
        clo, chi = max(lo, 0), min(hi, S)
        xk = np.zeros((PADK, D), xdt)
        xv = np.zeros((PADK, D), xdt)
        xk[clo - lo:chi - lo] = kx[b, clo:chi]
        xv[clo - lo:chi - lo] = vx[b, clo:chi]

        if x_int8:
            blob = gblob[core]
            blob[OFF_XQ:OFF_XK] = xq.T.reshape(-1)
            blob[OFF_XK:OFF_XV] = xk.T.reshape(-1)
            blob[OFF_XV:OFF_W] = xv.T.reshape(-1)
            if W12:
                blob[OFF_W:OFF_SC] = wpacked[WSH * core:WSH * (core + 1)].reshape(-1)
                blob[OFF_SC:OFF_V] = wscales
            else:
                blob[OFF_W:OFF_V] = wstack[WSH * core:WSH * (core + 1)].view(np.int8).reshape(-1)
            blob[OFF_V:BLOB] = _vones_cache[half].astype(np.int8).reshape(-1)
        else:
            wchunk = wstack[WSH * core:WSH * (core + 1)]
            globals_by_name["xqt"][core * D:(core + 1) * D] = xq.T
            globals_by_name["xkt"][core * D:(core + 1) * D] = xk.T
            globals_by_name["xvt"][core * D:(core + 1) * D] = xv.T
            globals_by_name["wchunk"][core * WSH:(core + 1) * WSH] = wchunk
            globals_by_name["vones"][core * NQT:(core + 1) * NQT] = _vones_cache[half]

    list(_host_pool.map(pack_core, range(NCORES)))
    if has_b:
        def rep(name, arr):
            g = np.empty((NCORES * arr.shape[0], *arr.shape[1:]), arr.dtype)
            g[:] = np.tile(arr, (NCORES,) + (1,) * (arr.ndim - 1))
            globals_by_name[name] = g
        rep("bqc", np.ascontiguousarray((bq * SCALE).reshape(4, 128)))
        rep("bkc", np.ascontiguousarray(bk.reshape(4, 128)))
        rep("bvb", np.broadcast_to(bv, (128, D)).astype(np.float32))
        rep("bob", np.broadcast_to(bo, (128, D)).astype(np.float32))
    if x_int8:
        globals_by_name["blob"] = gblob.reshape(-1)

    import time as _time
    run = _get_runner(nc)
    concat_in = [globals_by_name[nm] for nm in run.in_names]
    outs = run(concat_in)
    if TRACE:
        best = None
        for _ in range(3):
            t0 = _time.perf_counter()
            outs = run(concat_in)
            dtns = (_time.perf_counter() - t0) * 1e9
            best = dtns if best is None else min(best, dtns)
        LAST["wall_ns"] = best
    LAST["exec_time_ns"] = None

    ow = D + 4 if out_int8 else D
    oglob = outs[0].reshape(NCORES, SH, ow)
    out = np.empty((B, S, D), np.float32)

    def unpack_core(core):
        b, half = core // 2, core % 2
        o = oglob[core]
        dst = out[b, half * SH:(half + 1) * SH]
        if out_int8:
            scale = np.ascontiguousarray(o[:, D:D + 4]).view(np.float32)
            np.multiply(o[:, 0:D], scale, out=dst, dtype=np.float32)
        else:
            dst[:] = o

    list(_host_pool.map(unpack_core, range(NCORES)))
    return out


if __name__ == "__main__":
    rng = np.random.default_rng(0)
    sc = 1.0 / np.sqrt(D)
    inputs = {
        "query": rng.standard_normal((B, S, D)).astype(np.float32),
        "key": rng.standard_normal((B, S, D)).astype(np.float32),
        "value": rng.standard_normal((B, S, D)).astype(np.float32),
        "Wq": (rng.standard_normal((D, D)) * sc).astype(np.float32),
        "bq": np.zeros(D, np.float32),
        "Wk": (rng.standard_normal((D, D)) * sc).astype(np.float32),
        "bk": np.zeros(D, np.float32),
        "Wv": (rng.standard_normal((D, D)) * sc).astype(np.float32),
        "bv": np.zeros(D, np.float32),
        "Wo": (rng.standard_normal((D, D)) * sc).astype(np.float32),
        "bo": np.zeros(D, np.float32),
    }
    out = kernel(**inputs)
    print("out", out.shape, out.dtype, out[0, 0, :4])



# revision 29
# speedup vs baseline: 1.0009x; 1.0009x over previous
"""Local (banded) attention kernel for Trainium2, 8 NeuronCores SPMD.

Problem: nn_LocalAttention  (B=4, S=2048, D=512, H=8 heads, DK=64, band W=16)
  out = (softmax(band_mask(QK^T/sqrt(DK))) V) Wo + bo   with Q/K/V = x W* + b*

Sharding: 8 cores = 4 batches x 2 sequence halves. Each core computes its
1024-query slice end-to-end. K/V get a 16-row halo (zero-padded at sequence
ends) so no inter-core attention communication is needed.

The measured metric is the end-to-end wall time of one full execution
(upload + NEFF exec + download) over the axon tunnel, which serializes all
RPC + data bytes in both directions at ~50MB/s peak (~19.6ms/MB floor even
for all-zero payloads, plus a content-entropy-dependent extra of up to
~8ms/MB; transfers do NOT overlap each other or exec, and each *blocked*
RPC costs a ~80ms round trip that async dispatch hides). On-device compute
is a few ms. The kernel therefore minimizes moved bytes and round trips:
  - Custom async PJRT runner (_make_runner): same _bass_exec_p path as
    bass2jax.run_bass_via_pjrt, but (a) the donated output buffers are
    created ON DEVICE by a tiny separate jit (saves the 4.2MB host zeros
    upload, ~100ms), and (b) zeros jit + sharded device_put + NEFF call are
    all issued async with only the final np.asarray blocking (hides ~3
    RPC round trips, ~150ms). One device_put of one concatenated global
    array = one streamed transfer (8 separate per-device puts pay ~45ms
    fixed cost EACH and serialize).
  - Q/K/V uploaded int8 with per-column scales folded into the weights on
    the host (X_INT8; per-tensor scales fail the 2e-2 gate, per-column
    passes). X_INT8=False falls back to fp16 (rel ~7e-4).
  - Weights uploaded once as 1/8 shards, 12-bit packed (W12: per row 512
    low bytes + 256 shared hi-nibble bytes for cols j/j+256, plus per-
    column scales; <= f16 abs error), AllGather'd and unpacked on device
    with int16 DVE bit ops; scales are applied per-partition at the
    projection psum-copy (Wq/Wk, per-d_out), the xvT upcast (Wv, per-d_in)
    and the ctxT copy (Wo, per-d_in). 1.69MB vs 2.10MB f16.
  - Band mask is an inline NEFF constant; sequence-edge validity is a tiny
    per-core [NQT,128] "vones" vector that becomes the fused-denominator
    column of V (replaces the 264KB/core mask upload).
  - Output is int8 with a per-row f32 scale packed into the last 4 bytes of
    each row (OUT_INT8; halves the download vs fp16).
  - All inputs ride in ONE int8 blob per core; host packing (quantize,
    transpose, blob assembly) is threaded (ThreadPoolExecutor) since it is
    wall-time before the timed region.
  - jax persistent compilation cache turns recompiles into disk hits.
Measured: 2.655s original -> 0.560s staged baseline -> ~0.406s
(best-of-3; run-to-run noise +-15ms), rel err 1.761e-2 (gate 2e-2).

Per-core device pipeline (fp16 operands, f32 psum):
  - int8 x tiles upcast to fp16 on DVE (values <=127 are exact in fp16).
  - QT = Wq^T @ XqT -> [64,1024] per head; KT likewise [64,1056].
  - V window-major [kpos, 8, 65]; col 64 = vones (validity) -> fused softmax
    denominator that automatically excludes padded keys.
  - Per q-tile (96 queries, 128-key window) and head:
      scoresT = KT_win^T.QT_tile (psum f32); attnT = exp(scoresT) (ACT, f16)
      attnT *= band (gpsimd, inline 0/1 const, broadcast over heads)
      ctx_aug = attnT^T.V_aug (PE); ctx = ctx_aug[:,:64]/den (DVE reciprocal)
      ctxT via PE-transpose -> [512,1024]
  - out = ctxT^T.Wo (+bo) -> [1024,512], per-row absmax/127 int8 quantize,
    scale bitcast into out[:, 512:516] -> DRAM.
"""

import os
import sys

for _p in ("/opt/trn_rl_repo", "/root/.axon_site/_ro/trn_rl_repo"):
    if os.path.isdir(_p) and _p not in sys.path:
        sys.path.insert(0, _p)
        break

import numpy as np
import ml_dtypes

# Persist compiled PJRT executables across calls: run_bass_kernel_spmd builds a
# fresh jit closure per call, so without this every call re-lowers/recompiles
# the identical program (~0.2s) before transferring anything.
try:
    import tempfile

    import jax

    _cache_dir = os.path.join(
        tempfile.gettempdir(), f"jax_comp_cache_{os.getuid()}")
    jax.config.update("jax_compilation_cache_dir", _cache_dir)
    jax.config.update("jax_persistent_cache_min_entry_size_bytes", -1)
    jax.config.update("jax_persistent_cache_min_compile_time_secs", 0.0)
except Exception:
    pass

import concourse.bass as bass
import concourse.tile as tile
from concourse import bacc, bass2jax, mybir

BF16 = ml_dtypes.bfloat16
F16 = np.float16

B, S, D, H, W = 4, 2048, 512, 8, 16
DK = D // H          # 64
NCORES = 8
SH = S // 2          # 1024 rows per core
PADK = SH + 2 * W    # 1056 padded key rows
QT = 96              # q-tile size
NQT = (SH + QT - 1) // QT   # 11 tiles (last = 64)
WIN = QT + 2 * W     # 128-key window per q-tile
SCALE = 1.0 / np.sqrt(DK)
WROWS = 4 * D        # 2048 stacked weight rows
WSH = WROWS // NCORES  # 256 rows per core shard

X_INT8 = True        # upload Q/K/V as int8 (per-column scales folded into W)
OUT_INT8 = True      # download output as int8 + per-row f32 scales
W12 = True           # pack weights as 12-bit planes (vs f16) in the blob

# single-blob input layout (int8-x mode): one ExternalInput array per core.
# Weights ride as 12-bit packed planes: per row, 512 low bytes + 256 shared
# hi-nibble bytes (cols j and j+256 share one byte), plus 2048 f32 scales
# (per-d_out for Wq/Wk, per-d_in for Wv/Wo) duplicated in every core's blob.
SXQ = D * SH          # 524288   xqT int8 [512, 1024]
SXK = D * PADK        # 540672   xkT int8 [512, 1056]
WPACK = D + D // 2    # 768 packed bytes per weight row
OFF_XQ = 0
OFF_XK = OFF_XQ + SXQ
OFF_XV = OFF_XK + SXK
OFF_W = OFF_XV + SXK            # packed weight shard [256, 768] int8
if W12:
    OFF_SC = OFF_W + WSH * WPACK    # weight scales f32 [4*D]
    OFF_V = OFF_SC + 4 * D * 4      # vones int8 [NQT, 128]
else:
    OFF_SC = None
    OFF_V = OFF_W + WSH * D * 2     # f16 weight shard [256, 512]
BLOB = OFF_V + NQT * 128        # 1811840 bytes (W12) / 1869184 (f16)

TRACE = False        # set True (from test.py) to collect an NTFF profile
LAST = {}            # stash for exec_time_ns / profile info

from concurrent.futures import ThreadPoolExecutor

_host_pool = ThreadPoolExecutor(max_workers=8)   # numpy packing parallelism

_programs = {}       # (x_int8, out_int8, has_b) -> compiled nc


def _emit(nc, tc, pools, dram, x_int8, out_int8, has_b):
    dt = mybir.dt
    f16, f32, i8 = dt.float16, dt.float32, dt.int8
    consts, work, psA, psB, psC = pools
    out_d = dram["out"]

    def blob_ap(off, pattern):
        b0 = dram["blob"][0:1]
        return bass.AP(tensor=b0.tensor, offset=off, ap=pattern)

    # ---- weights: bounce -> AllGather -> SBUF ----------------------------
    if x_int8 and W12:
        wch_src = blob_ap(OFF_W, [[WPACK, WSH], [1, WPACK]])
    elif x_int8:
        wch_src = blob_ap(OFF_W, [[D * 2, WSH], [1, D * 2]]).bitcast(f16)
    else:
        wch_src = dram["wchunk"][:, :]
    nc.sync.dma_start(out=dram["wch_b"][:, :], in_=wch_src)
    nc.gpsimd.collective_compute(
        "AllGather",
        mybir.AluOpType.bypass,
        replica_groups=[list(range(NCORES))],
        ins=[dram["wch_b"].ap().opt()],
        outs=[dram["wfull"].ap().opt()],
    )
    sc_sb = None
    w_sb = {}
    if x_int8 and W12:
        # per-chunk weight scales [128, 16]; col 4i+k = scales[512i + 128k + p]
        sc_sb = consts.tile([128, 16], f32, tag="wsc")
        nc.sync.dma_start(
            out=sc_sb[:],
            in_=blob_ap(OFF_SC, [[64, 128], [1, 64]]).bitcast(f32),
        )
        i16 = dt.int16
        for i, name in enumerate(("wq", "wk", "wv", "wo")):
            w_sb[name] = []
            for k in range(4):
                r0 = D * i + 128 * k
                lo8 = work.tile([128, D], i8, tag="wlo8")
                hi8 = work.tile([128, D // 2], i8, tag="whi8")
                nc.sync.dma_start(out=lo8[:], in_=dram["wfull"][r0:r0 + 128, 0:D])
                nc.sync.dma_start(out=hi8[:], in_=dram["wfull"][r0:r0 + 128, D:WPACK])
                lo16 = work.tile([128, D], i16, tag="wlo16")
                hi16 = work.tile([128, D // 2], i16, tag="whi16")
                ev16 = work.tile([128, D // 2], i16, tag="wev16")
                od16 = work.tile([128, D // 2], i16, tag="wod16")
                nc.vector.tensor_copy(out=lo16[:], in_=lo8[:])
                nc.vector.tensor_scalar(
                    out=lo16[:], in0=lo16[:], scalar1=255, scalar2=None,
                    op0=mybir.AluOpType.bitwise_and)
                nc.vector.tensor_copy(out=hi16[:], in_=hi8[:])
                nc.vector.tensor_scalar(
                    out=hi16[:], in0=hi16[:], scalar1=255, scalar2=None,
                    op0=mybir.AluOpType.bitwise_and)
                nc.vector.tensor_scalar(
                    out=ev16[:], in0=hi16[:], scalar1=15, scalar2=8,
                    op0=mybir.AluOpType.bitwise_and,
                    op1=mybir.AluOpType.logical_shift_left)
                nc.vector.tensor_scalar(
                    out=od16[:], in0=hi16[:], scalar1=4, scalar2=8,
                    op0=mybir.AluOpType.logical_shift_right,
                    op1=mybir.AluOpType.logical_shift_left)
                v16 = work.tile([128, D], i16, tag="wv16")
                h = D // 2
                nc.vector.tensor_add(out=v16[:, 0:h], in0=lo16[:, 0:h], in1=ev16[:])
                nc.vector.tensor_add(out=v16[:, h:D], in0=lo16[:, h:D], in1=od16[:])
                nc.vector.tensor_scalar_add(out=v16[:], in0=v16[:], scalar1=-2048)
                t = consts.tile([128, D], f16, tag=f"{name}{k}")
                nc.vector.tensor_copy(out=t[:], in_=v16[:])
                w_sb[name].append(t)
    else:
        for i, name in enumerate(("wq", "wk", "wv", "wo")):
            w_sb[name] = []
            for k in range(4):
                t = consts.tile([128, D], f16, tag=f"{name}{k}")
                r0 = D * i + 128 * k
                nc.sync.dma_start(out=t[:], in_=dram["wfull"][r0:r0 + 128, :])
                w_sb[name].append(t)

    # ---- load x (fp16 direct, or int8-from-blob + DVE upcast) ------------
    def load_xt(key, off, ncols, sc0=None):
        tiles = []
        for k in range(4):
            if x_int8:
                t8 = consts.tile([128, ncols], i8, tag=f"{key}{k}i8")
                nc.sync.dma_start(
                    out=t8[:],
                    in_=blob_ap(off + 128 * k * ncols, [[ncols, 128], [1, ncols]]),
                )
                t = consts.tile([128, ncols], f16, tag=f"{key}{k}")
                nc.vector.tensor_copy(out=t[:], in_=t8[:])
                if sc0 is not None:
                    # fold Wv's per-d_in 12-bit scale into the upcast
                    nc.vector.tensor_scalar_mul(
                        out=t[:], in0=t[:],
                        scalar1=sc_sb[:, sc0 + k:sc0 + k + 1])
            else:
                t = consts.tile([128, ncols], f16, tag=f"{key}{k}")
                nc.sync.dma_start(out=t[:], in_=dram[key][128 * k:128 * (k + 1), :])
            tiles.append(t)
        return tiles

    xqt_sb = load_xt("xqt", OFF_XQ, SH)
    xkt_sb = load_xt("xkt", OFF_XK, PADK)
    xvt_sb = load_xt("xvt", OFF_XV, PADK, sc0=8 if (x_int8 and W12) else None)

    vones_sb = consts.tile([128, NQT], f32, tag="vones")
    if x_int8:
        # vones int8 [NQT, 128] in the blob; partition-first AP transposes
        v8 = consts.tile([128, NQT], i8, tag="vones8")
        nc.sync.dma_start(out=v8[:], in_=blob_ap(OFF_V, [[1, 128], [128, NQT]]))
        nc.vector.tensor_copy(out=vones_sb[:], in_=v8[:])
    else:
        nc.sync.dma_start(
            out=vones_sb[:], in_=dram["vones"].ap().rearrange("t p -> p t"))

    band_sb = consts.tile([128, QT], f16, tag="band")
    nc.sync.dma_start(out=band_sb[:], in_=dram["band"][:])
    ident_sb = consts.tile([QT, QT], f16, tag="ident")
    nc.sync.dma_start(out=ident_sb[:], in_=dram["ident"][:])

    bq_sb = bk_sb = bv_sb = bo_sb = None
    if has_b:
        bq_sb = consts.tile([128, 4], f32, tag="bq")
        nc.sync.dma_start(out=bq_sb[:], in_=dram["bqc"].ap().rearrange("c p -> p c"))
        bk_sb = consts.tile([128, 4], f32, tag="bk")
        nc.sync.dma_start(out=bk_sb[:], in_=dram["bkc"].ap().rearrange("c p -> p c"))
        bv_sb = consts.tile([128, D], f32, tag="bv")
        nc.sync.dma_start(out=bv_sb[:], in_=dram["bvb"][:])
        bo_sb = consts.tile([128, D], f32, tag="bo")
        nc.sync.dma_start(out=bo_sb[:], in_=dram["bob"][:])

    # ---- Q/K projections -> per-head QT [64, SH], KT [64, PADK] (f16) ----
    # Per-head tiles keep every matmul operand at partition offset 0: the HW
    # crashes on (partition-offset operand + intra-bank psum write offset).
    qt_sb, kt_sb = [], []
    for h in range(H):
        qt_sb.append(consts.tile([64, SH], f16, tag=f"qt{h}", name=f"qt{h}"))
        kt_sb.append(consts.tile([64, PADK], f16, tag=f"kt{h}", name=f"kt{h}"))

    def project_T(xt_sb, w, out_tiles, bias_sb, ncols, sc0):
        # head 2m / 2m+1 live in rows 0:64 / 64:128 of dout-chunk m
        for m in range(4):
            c0 = 0
            while c0 < ncols:
                cw = min(512, ncols - c0)
                ps = psA.tile([128, 512], f32, tag="big")
                for k in range(4):
                    nc.tensor.matmul(
                        ps[:, :cw],
                        lhsT=w[k][:, 128 * m:128 * (m + 1)],
                        rhs=xt_sb[k][:, c0:c0 + cw],
                        start=(k == 0),
                        stop=(k == 3),
                    )
                for half in range(2):
                    r0, r1 = 64 * half, 64 * half + 64
                    dst = out_tiles[2 * m + half][:, c0:c0 + cw]
                    src = ps[r0:r1, :cw]
                    if sc0 is not None:
                        # 12-bit weights: scale rows by per-d_out scale
                        # (optionally fused with bias add)
                        if has_b:
                            nc.vector.tensor_scalar(
                                out=dst, in0=src,
                                scalar1=sc_sb[r0:r1, sc0 + m:sc0 + m + 1],
                                scalar2=bias_sb[r0:r1, m:m + 1],
                                op0=mybir.AluOpType.mult,
                                op1=mybir.AluOpType.add,
                            )
                        else:
                            nc.vector.tensor_scalar_mul(
                                out=dst, in0=src,
                                scalar1=sc_sb[r0:r1, sc0 + m:sc0 + m + 1],
                            )
                    elif has_b:
                        nc.vector.tensor_scalar_add(
                            out=dst, in0=src,
                            scalar1=bias_sb[r0:r1, m:m + 1],
                        )
                    else:
                        nc.vector.tensor_copy(out=dst, in_=src)
                c0 += cw

    wsc0 = 0 if (x_int8 and W12) else None
    project_T(xqt_sb, w_sb["wq"], qt_sb, bq_sb, SH, wsc0)
    project_T(xkt_sb, w_sb["wk"], kt_sb, bk_sb, PADK,
              4 if (x_int8 and W12) else None)

    # ---- V projection, window-major; col 64 = vones (validity) -----------
    v_sb = []
    for t in range(NQT):
        w0 = QT * t
        wr = min(WIN, PADK - w0)
        vt = consts.tile([128, H, DK + 1], f16, tag=f"v{t}")
        v_sb.append(vt)
        ps = psA.tile([128, 512], f32, tag="big")
        for k in range(4):
            nc.tensor.matmul(
                ps[:wr, :],
                lhsT=xvt_sb[k][:, w0:w0 + wr],
                rhs=w_sb["wv"][k][:],
                start=(k == 0),
                stop=(k == 3),
            )
        src = ps[:wr, :].rearrange("p (h x) -> p h x", h=H)
        if has_b:
            bvv = bv_sb[:wr, :].rearrange("p (h x) -> p h x", h=H)
            nc.vector.tensor_add(out=vt[:wr, :, 0:DK], in0=src, in1=bvv)
            # zero out padded-key rows so bias doesn't leak into the band sum
            nc.vector.tensor_scalar_mul(
                out=vt[:wr, :, 0:DK],
                in0=vt[:wr, :, 0:DK],
                scalar1=vones_sb[:wr, t:t + 1],
            )
        else:
            nc.vector.tensor_copy(out=vt[:wr, :, 0:DK], in_=src)
        vb = vones_sb[:wr, t:t + 1]
        vb_bc = bass.AP(
            tensor=vb.tensor, offset=vb.offset,
            ap=[vb.ap[0], [0, H], vb.ap[1]],
        )
        nc.vector.tensor_copy(out=vt[:wr, :, DK:DK + 1], in_=vb_bc)

    # ---- attention -------------------------------------------------------
    ctxT_sb = []
    for c in range(4):
        ctxT_sb.append(consts.tile([128, SH], f16, tag=f"ctxT{c}", name=f"ctxT{c}"))

    head_groups = ((0, 5), (5, 8))
    for t in range(NQT):
        q0 = QT * t
        qw = min(QT, SH - q0)
        w0 = QT * t
        wr = min(WIN, PADK - w0)

        attn_sb = work.tile([128, H, QT], f16, tag="attn")
        for h0, h1 in head_groups:
            nh = h1 - h0
            ps_sc = psB.tile([128, 5, QT], f32, tag="sc")
            for j, h in enumerate(range(h0, h1)):
                nc.tensor.matmul(
                    ps_sc[:wr, j, :qw],
                    lhsT=kt_sb[h][:, w0:w0 + wr],
                    rhs=qt_sb[h][:, q0:q0 + qw],
                    start=True,
                    stop=True,
                )
            nc.scalar.activation(
                out=attn_sb[:wr, h0:h1, :qw],
                in_=ps_sc[:wr, :nh, :qw],
                func=mybir.ActivationFunctionType.Exp,
            )

        # multiplicative band mask, broadcast over heads (gpsimd)
        mbase = band_sb[:wr, :qw]
        mask_bc = bass.AP(
            tensor=mbase.tensor, offset=mbase.offset,
            ap=[mbase.ap[0], [0, H], mbase.ap[1]],
        )
        nc.gpsimd.tensor_mul(
            out=attn_sb[:wr, :, :qw], in0=attn_sb[:wr, :, :qw], in1=mask_bc
        )

        recip_sb = work.tile([QT, H], f32, tag="recip")
        ctx_sb = work.tile([QT, H, DK], f16, tag="ctx")
        for g in range(2):
            ps_ctx = psC.tile([QT, 4, DK + 1], f32, tag="ctx")
            for j, h in enumerate(range(4 * g, 4 * g + 4)):
                nc.tensor.matmul(
                    ps_ctx[:qw, j, :],
                    lhsT=attn_sb[:wr, h, :qw],
                    rhs=v_sb[t][:wr, h, :],
                    start=True,
                    stop=True,
                )
            nc.vector.reciprocal(
                out=recip_sb[:qw, 4 * g:4 * g + 4],
                in_=ps_ctx[:qw, :, DK:DK + 1],
            )
            rbase = recip_sb[:qw, 4 * g:4 * g + 4]
            recip_bc = bass.AP(
                tensor=rbase.tensor, offset=rbase.offset,
                ap=[rbase.ap[0], rbase.ap[1], [0, DK]],
            )
            nc.vector.tensor_mul(
                out=ctx_sb[:qw, 4 * g:4 * g + 4, :],
                in0=ps_ctx[:qw, :, 0:DK],
                in1=recip_bc,
            )

        # transpose ctx [qw, 512] -> ctxT [512, qw]  (4 chunks of 128)
        for c in range(4):
            ps_t = psA.tile([128, QT], f16, tag="big")
            nc.tensor.transpose(
                out=ps_t[:, :qw],
                in_=ctx_sb[:qw, 2 * c:2 * c + 2, :],
                identity=ident_sb[:qw, :qw],
            )
            if x_int8 and W12:
                # fold Wo's per-d_in 12-bit scale into the ctxT copy
                nc.vector.tensor_scalar_mul(
                    out=ctxT_sb[c][:, q0:q0 + qw], in0=ps_t[:, :qw],
                    scalar1=sc_sb[:, 12 + c:12 + c + 1])
            else:
                nc.vector.tensor_copy(
                    out=ctxT_sb[c][:, q0:q0 + qw], in_=ps_t[:, :qw])

    # ---- O-projection ----------------------------------------------------
    for mt in range(8):
        r0 = 128 * mt
        ps = psA.tile([128, 512], f32, tag="big")
        for k in range(4):
            nc.tensor.matmul(
                ps[:],
                lhsT=ctxT_sb[k][:, r0:r0 + 128],
                rhs=w_sb["wo"][k][:],
                start=(k == 0),
                stop=(k == 3),
            )
        src = ps[:]
        if has_b:
            of_sb = work.tile([128, D], f32, tag="osbf")
            nc.vector.tensor_add(out=of_sb[:], in0=ps[:], in1=bo_sb[:])
            src = of_sb[:]
        if out_int8:
            # per-row int8 quantization; scale = absmax/127 rides in the last
            # 4 bytes of each int8 output row (bitcast f32)
            amax_sb = work.tile([128, 1], f32, tag="amax")
            osc_sb = work.tile([128, 1], f32, tag="osc")
            rsc_sb = work.tile([128, 1], f32, tag="rsc")
            o_sb = work.tile([128, D], dt.int8, tag="osb8")
            nc.vector.tensor_reduce(
                out=amax_sb[:], in_=src,
                axis=mybir.AxisListType.X, op=mybir.AluOpType.max,
                apply_absolute_value=True,
            )
            nc.vector.tensor_scalar_max(out=amax_sb[:], in0=amax_sb[:], scalar1=1e-30)
            nc.vector.tensor_scalar_mul(out=osc_sb[:], in0=amax_sb[:], scalar1=1.0 / 127.0)
            nc.vector.reciprocal(out=rsc_sb[:], in_=osc_sb[:])
            nc.vector.tensor_scalar_mul(out=o_sb[:], in0=src, scalar1=rsc_sb[:, 0:1])
            nc.sync.dma_start(out=out_d[r0:r0 + 128, 0:D], in_=o_sb[:])
            nc.sync.dma_start(
                out=out_d[r0:r0 + 128, D:D + 4].bitcast(f32), in_=osc_sb[:]
            )
        else:
            o_sb = work.tile([128, D], f16, tag="osb")
            nc.vector.tensor_copy(out=o_sb[:], in_=src)
            nc.sync.dma_start(out=out_d[r0:r0 + 128, :], in_=o_sb[:])


def _build_band() -> np.ndarray:
    i = np.arange(128)[:, None]   # window row (key)
    j = np.arange(QT)[None, :]    # q column
    band = (i - j >= 0) & (i - j <= 2 * W)
    return band.astype(F16)


def _build_program(x_int8: bool, out_int8: bool, has_b: bool):
    dt = mybir.dt
    f16, f32 = dt.float16, dt.float32
    xdt = dt.int8 if x_int8 else f16
    odt = dt.int8 if out_int8 else f16

    nc = bacc.Bacc("TRN2", target_bir_lowering=False, debug=False, num_devices=NCORES)

    dram = {}
    if x_int8:
        dram["blob"] = nc.dram_tensor("blob", [BLOB], dt.int8, kind="ExternalInput")
    else:
        dram["xqt"] = nc.dram_tensor("xqt", [D, SH], xdt, kind="ExternalInput")
        dram["xkt"] = nc.dram_tensor("xkt", [D, PADK], xdt, kind="ExternalInput")
        dram["xvt"] = nc.dram_tensor("xvt", [D, PADK], xdt, kind="ExternalInput")
        dram["wchunk"] = nc.dram_tensor("wchunk", [WSH, D], f16, kind="ExternalInput")
        dram["vones"] = nc.dram_tensor("vones", [NQT, 128], f32, kind="ExternalInput")
    dram.update({
        "out": nc.dram_tensor(
            "out", [SH, D + 4] if out_int8 else [SH, D], odt, kind="ExternalOutput"),
        "band": nc.inline_tensor(_build_band(), name="band"),
        "ident": nc.inline_tensor(np.eye(QT, dtype=F16), name="ident"),
    })
    if x_int8 and W12:
        dram["wch_b"] = nc.dram_tensor("wch_b", [WSH, WPACK], dt.int8)
        dram["wfull"] = nc.dram_tensor("wfull", [WROWS, WPACK], dt.int8)
    else:
        dram["wch_b"] = nc.dram_tensor("wch_b", [WSH, D], f16)
        dram["wfull"] = nc.dram_tensor("wfull", [WROWS, D], f16)
    if has_b:
        dram["bqc"] = nc.dram_tensor("bqc", [4, 128], f32, kind="ExternalInput")
        dram["bkc"] = nc.dram_tensor("bkc", [4, 128], f32, kind="ExternalInput")
        dram["bvb"] = nc.dram_tensor("bvb", [128, D], f32, kind="ExternalInput")
        dram["bob"] = nc.dram_tensor("bob", [128, D], f32, kind="ExternalInput")

    with tile.TileContext(nc) as tc:
        with (
            tc.tile_pool(name="consts", bufs=1) as consts,
            tc.tile_pool(name="work", bufs=3) as work,
            tc.tile_pool(name="psA", bufs=2, space="PSUM") as psA,
            tc.tile_pool(name="psB", bufs=2, space="PSUM") as psB,
            tc.tile_pool(name="psC", bufs=4, space="PSUM") as psC,
        ):
            _emit(nc, tc, (consts, work, psA, psB, psC), dram, x_int8, out_int8, has_b)

    nc.compile()
    return nc


def _get_program(x_int8, out_int8, has_b):
    key = (x_int8, out_int8, has_b)
    if key not in _programs:
        _programs[key] = _build_program(x_int8, out_int8, has_b)
    return _programs[key]


# ---------------------------------------------------------------------------
# Custom PJRT runner. Same _bass_exec_p path as bass2jax.run_bass_via_pjrt,
# with two wall-time fixes (the measured metric is transfer-bound over the
# axon tunnel, which serializes all RPC + data bytes in both directions):
#   - the donated output buffers are created ON DEVICE by a separate tiny jit
#     (jnp.zeros + out_shardings) instead of uploading host np.zeros — saves
#     the 4.2MB zero upload (~100ms). They can't be created inside the same
#     jit: neuronx_cc_hook requires every bass_exec operand to be a plain HLO
#     parameter.
#   - everything is issued async (zeros jit, sharded device_put, NEFF call)
#     and only the final np.asarray blocks, so per-RPC round-trip latencies
#     (~60-80ms each when blocked individually) overlap with the data stream.
# ---------------------------------------------------------------------------

_runners = {}


def _make_runner(nc):
    import jax
    import jax.numpy as jnp
    from jax.sharding import Mesh, NamedSharding, PartitionSpec
    import warnings
    with warnings.catch_warnings():
        warnings.simplefilter("ignore")
        from jax.experimental.shard_map import shard_map

    bass2jax.install_neuronx_cc_hook()
    partition_name = nc.partition_id_tensor.name if nc.partition_id_tensor else None
    in_names, out_names, out_avals = [], [], []
    for alloc in nc.m.functions[0].allocations:
        if not isinstance(alloc, mybir.MemoryLocationSet):
            continue
        name = alloc.memorylocations[0].name
        if alloc.kind == "ExternalInput":
            if name != partition_name:
                in_names.append(name)
        elif alloc.kind == "ExternalOutput":
            out_names.append(name)
            out_avals.append(
                jax.core.ShapedArray(
                    tuple(alloc.tensor_shape), mybir.dt.np(alloc.dtype)))
    n_params = len(in_names)
    n_outs = len(out_avals)
    in_names_all = in_names + out_names
    if partition_name is not None:
        in_names_all.append(partition_name)

    def _body(*args):
        operands = list(args)
        if partition_name is not None:
            operands.append(bass2jax.partition_id_tensor())
        outs = bass2jax._bass_exec_p.bind(
            *operands,
            out_avals=tuple(out_avals),
            in_names=tuple(in_names_all),
            out_names=tuple(out_names),
            lowering_input_output_aliases=(),
            sim_require_finite=True,
            sim_require_nnan=True,
            nc=nc,
        )
        return tuple(outs)

    devices = jax.devices()[:NCORES]
    mesh = Mesh(np.asarray(devices), ("core",))
    spec = NamedSharding(mesh, PartitionSpec("core"))
    in_specs = (PartitionSpec("core"),) * (n_params + n_outs)
    out_specs = (PartitionSpec("core"),) * n_outs
    # No donation: our kernel writes every output element, so the NEFF's
    # output operands never need meaningful content. A single device-side
    # zeros tuple is created once and passed (never consumed) every call —
    # zero per-call cost on the terminal's serial RPC queue.
    sharded = jax.jit(
        shard_map(_body, mesh=mesh, in_specs=in_specs, out_specs=out_specs,
                  check_rep=False),
        keep_unused=True)
    gshapes = [(NCORES * a.shape[0], *a.shape[1:]) for a in out_avals]
    zeros_fn = jax.jit(
        lambda: tuple(jnp.zeros(s, a.dtype) for s, a in zip(gshapes, out_avals)),
        out_shardings=(spec,) * n_outs)
    zeros_persist = zeros_fn()
    jax.block_until_ready(zeros_persist)

    def run(concat_in):
        """concat_in: list of global [NCORES*rows, ...] arrays in in_names
        order. Returns list of global output arrays (np, gathered)."""
        g_in = [jax.device_put(a, spec) for a in concat_in]   # async upload
        outs = sharded(*g_in, *zeros_persist)            # async NEFF exec
        return [np.asarray(o) for o in outs]             # blocks

    run.in_names = in_names
    return run


def _get_runner(nc):
    if id(nc) not in _runners:
        _runners[id(nc)] = _make_runner(nc)
    return _runners[id(nc)]


def _build_vones(half: int) -> np.ndarray:
    # vones[t, i] = 1.0 iff padded K/V row (96t + i) holds a real key
    v = np.zeros((NQT, 128), np.float32)
    r = QT * np.arange(NQT)[:, None] + np.arange(128)[None, :]
    lo, hi = (W, PADK) if half == 0 else (0, PADK - W)
    v[:] = ((r >= lo) & (r < hi)).astype(np.float32)
    return v


_vones_cache = {}


def kernel(query, key, value, Wq, bq, Wk, bk, Wv, bv, Wo, bo):
    query = np.asarray(query, np.float32)
    key = np.asarray(key, np.float32)
    value = np.asarray(value, np.float32)
    Wq = np.asarray(Wq, np.float32)
    Wk = np.asarray(Wk, np.float32)
    Wv = np.asarray(Wv, np.float32)
    Wo = np.asarray(Wo, np.float32)
    bq = np.asarray(bq, np.float32)
    bk = np.asarray(bk, np.float32)
    bv = np.asarray(bv, np.float32)
    bo = np.asarray(bo, np.float32)

    has_b = bool(np.any(bq) or np.any(bk) or np.any(bv) or np.any(bo))
    x_int8 = X_INT8
    out_int8 = OUT_INT8
    nc = _get_program(x_int8, out_int8, has_b)

    if x_int8:
        # per-column int8 scales, folded into the weight rows on the host

        def colmax(x):
            return np.maximum(np.abs(x).max(axis=(0, 1)) / 127.0, 1e-30)

        def quant(x, s):
            # s = absmax/127 bounds |x/s| <= 127 (+1 ulp, absorbed by rint),
            # so no clip pass is needed
            t = x * (1.0 / s).astype(np.float32)
            np.rint(t, out=t)
            return t.astype(np.int8)

        sq, sk, sv = _host_pool.map(colmax, (query, key, value))
        qx, kx, vx = _host_pool.map(
            lambda a: quant(*a), ((query, sq), (key, sk), (value, sv)))
        wq_f = Wq * (sq[:, None] * SCALE)
        wk_f = Wk * sk[:, None]
        wv_f = Wv * sv[:, None]
        xdt = np.int8
    else:
        wq_f = Wq * SCALE
        wk_f = Wk
        wv_f = Wv
        qx, kx, vx = query.astype(F16), key.astype(F16), value.astype(F16)
        xdt = F16

    if x_int8 and W12:
        # 12-bit weight quantization: Wq/Wk per-column (d_out, scale applied
        # on the projection psum rows), Wv/Wo per-row (d_in, scale folded
        # into the xvT upcast / ctxT copy respectively)
        def q12(w, axis):
            s = np.maximum(np.abs(w).max(axis=axis), 1e-30) / 2047.0
            q = np.rint(w / (s[None, :] if axis == 0 else s[:, None]))
            return q.astype(np.int32), s.astype(np.float32)
        qq, sq_w = q12(wq_f, 0)
        qk, sk_w = q12(wk_f, 0)
        qv, sv_w = q12(wv_f, 1)
        qo, so_w = q12(Wo, 1)
        u = (np.concatenate([qq, qk, qv, qo], axis=0) + 2048).astype(np.uint16)
        lo = (u & 255).astype(np.uint8)
        hi4 = (u >> 8).astype(np.uint8)
        hi = hi4[:, :D // 2] | (hi4[:, D // 2:] << 4)
        wpacked = np.concatenate([lo, hi], axis=1).view(np.int8)  # [2048, 768]
        # transposed [p, chunk] layout so the device DMA reads contiguous
        # 64-byte runs per partition: wscales_t[p*16 + c] = s[128c + p]
        wscales = np.ascontiguousarray(
            np.concatenate([sq_w, sk_w, sv_w, so_w]).reshape(16, 128).T
        ).reshape(-1).view(np.int8)
        wstack = None
    else:
        wstack = np.ascontiguousarray(
            np.concatenate([wq_f, wk_f, wv_f, Wo], axis=0).astype(F16))

    if not _vones_cache:
        _vones_cache[0] = _build_vones(0)
        _vones_cache[1] = _build_vones(1)

    globals_by_name = {}
    if x_int8:
        gblob = np.empty((NCORES, BLOB), np.int8)
        globals_by_name["blob"] = gblob
    else:
        globals_by_name["xqt"] = np.empty((NCORES * D, SH), F16)
        globals_by_name["xkt"] = np.empty((NCORES * D, PADK), F16)
        globals_by_name["xvt"] = np.empty((NCORES * D, PADK), F16)
        globals_by_name["wchunk"] = np.empty((NCORES * WSH, D), F16)
        globals_by_name["vones"] = np.empty((NCORES * NQT, 128), np.float32)
    def pack_core(core):
        b, half = core // 2, core % 2
        s0 = half * SH
        xq = qx[b, s0:s0 + SH]
        lo, hi = s0 - W, s0 + SH + W
        clo, chi = max(lo, 0), min(hi, S)
        xk = np.zeros((PADK, D), xdt)
        xv = np.zeros((PADK, D), xdt)
        xk[clo - lo:chi - lo] = kx[b, clo:chi]
        xv[clo - lo:chi - lo] = vx[b, clo:chi]

        if x_int8:
            blob = gblob[core]
            blob[OFF_XQ:OFF_XK] = xq.T.reshape(-1)
            blob[OFF_XK:OFF_XV] = xk.T.reshape(-1)
            blob[OFF_XV:OFF_W] = xv.T.reshape(-1)
            if W12:
                blob[OFF_W:OFF_SC] = wpacked[WSH * core:WSH * (core + 1)].reshape(-1)
                blob[OFF_SC:OFF_V] = wscales
            else:
                blob[OFF_W:OFF_V] = wstack[WSH * core:WSH * (core + 1)].view(np.int8).reshape(-1)
            blob[OFF_V:BLOB] = _vones_cache[half].astype(np.int8).reshape(-1)
        else:
            wchunk = wstack[WSH * core:WSH * (core + 1)]
            globals_by_name["xqt"][core * D:(core + 1) * D] = xq.T
            globals_by_name["xkt"][core * D:(core + 1) * D] = xk.T
            globals_by_name["xvt"][core * D:(core + 1) * D] = xv.T
            globals_by_name["wchunk"][core * WSH:(core + 1) * WSH] = wchunk
            globals_by_name["vones"][core * NQT:(core + 1) * NQT] = _vones_cache[half]

    list(_host_pool.map(pack_core, range(NCORES)))
    if has_b:
        def rep(name, arr):
            g = np.empty((NCORES * arr.shape[0], *arr.shape[1:]), arr.dtype)
            g[:] = np.tile(arr, (NCORES,) + (1,) * (arr.ndim - 1))
            globals_by_name[name] = g
        rep("bqc", np.ascontiguousarray((bq * SCALE).reshape(4, 128)))
        rep("bkc", np.ascontiguousarray(bk.reshape(4, 128)))
        rep("bvb", np.broadcast_to(bv, (128, D)).astype(np.float32))
        rep("bob", np.broadcast_to(bo, (128, D)).astype(np.float32))
    if x_int8:
        globals_by_name["blob"] = gblob.reshape(-1)

    import time as _time
    run = _get_runner(nc)
    concat_in = [globals_by_name[nm] for nm in run.in_names]
    outs = run(concat_in)
    if TRACE:
        best = None
        for _ in range(5):
            t0 = _time.perf_counter()
            outs = run(concat_in)
            dtns = (_time.perf_counter() - t0) * 1e9
            best = dtns if best is None else min(best, dtns)
        LAST["wall_ns"] = best
    LAST["exec_time_ns"] = None

    ow = D + 4 if out_int8 else D
    oglob = outs[0].reshape(NCORES, SH, ow)
    out = np.empty((B, S, D), np.float32)

    def unpack_core(core):
        b, half = core // 2, core % 2
        o = oglob[core]
        dst = out[b, half * SH:(half + 1) * SH]
        if out_int8:
            scale = np.ascontiguousarray(o[:, D:D + 4]).view(np.float32)
            np.multiply(o[:, 0:D], scale, out=dst, dtype=np.float32)
        else:
            dst[:] = o

    list(_host_pool.map(unpack_core, range(NCORES)))
    return out


if __name__ == "__main__":
    rng = np.random.default_rng(0)
    sc = 1.0 / np.sqrt(D)
    inputs = {
        "query": rng.standard_normal((B, S, D)).astype(np.float32),
        "key": rng.standard_normal((B, S, D)).astype(np.float32),
        "value": rng.standard_normal((B, S, D)).astype(np.float32),
        "Wq": (rng.standard_normal((D, D)) * sc).astype(np.float32),
        "bq": np.zeros(D, np.float32),
        "Wk": (rng.standard_normal((D, D)) * sc).astype(np.float32),
        "bk": np.zeros(D, np.float32),
        "Wv": (rng.standard_normal((D, D)) * sc).astype(np.float32),
        "bv": np.zeros(D, np.float32),
        "Wo": (rng.standard_normal((D, D)) * sc).astype(np.float32),
        "bo": np.zeros(D, np.float32),
    }
    out = kernel(**inputs)
    print("out", out.shape, out.dtype, out[0, 0, :4])



# revision 30
# speedup vs baseline: 1.0641x; 1.0632x over previous
"""Local (banded) attention kernel for Trainium2, 8 NeuronCores SPMD.

Problem: nn_LocalAttention  (B=4, S=2048, D=512, H=8 heads, DK=64, band W=16)
  out = (softmax(band_mask(QK^T/sqrt(DK))) V) Wo + bo   with Q/K/V = x W* + b*

Sharding: 8 cores = 4 batches x 2 sequence halves. Each core computes its
1024-query slice end-to-end. K/V get a 16-row halo (zero-padded at sequence
ends) so no inter-core attention communication is needed.

The measured metric is the end-to-end wall time of one full execution
(upload + NEFF exec + download) over the axon tunnel, which serializes all
RPC + data bytes in both directions at ~50MB/s peak (~19.6ms/MB floor even
for all-zero payloads, plus a content-entropy-dependent extra of up to
~8ms/MB; transfers do NOT overlap each other or exec, and each *blocked*
RPC costs a ~80ms round trip that async dispatch hides). On-device compute
is a few ms. The kernel therefore minimizes moved bytes and round trips:
  - Custom async PJRT runner (_make_runner): same _bass_exec_p path as
    bass2jax.run_bass_via_pjrt, but (a) the NEFF's output-buffer operands
    are a PERSISTENT device-side zeros tuple created once at runner build
    with no donation (our kernel writes every output element, so their
    content never matters) — saves the 4.2MB host zeros upload (~100ms)
    and all per-call zeros work, and (b) device_put + NEFF call are issued
    async with only the final np.asarray blocking (hides ~3 RPC round
    trips, ~150ms). One device_put of one concatenated global array = one
    streamed transfer (8 separate per-device puts pay ~45ms fixed cost
    EACH and serialize; 2 split puts are ~90ms worse than 1).
  - Q/K/V uploaded int8 with per-column scales folded into the weights on
    the host (X_INT8; per-tensor scales fail the 2e-2 gate, per-column
    passes). X_INT8=False falls back to fp16 (rel ~7e-4).
  - Weights uploaded once as 1/8 shards, 12-bit packed (W12: per row 512
    low bytes + 256 shared hi-nibble bytes for cols j/j+256, plus per-
    column scales; <= f16 abs error), AllGather'd and unpacked on device
    with int16 DVE bit ops; scales are applied per-partition at the
    projection psum-copy (Wq/Wk, per-d_out), the xvT upcast (Wv, per-d_in)
    and the ctxT copy (Wo, per-d_in). 1.69MB vs 2.10MB f16.
  - Band mask is an inline NEFF constant; sequence-edge validity is a tiny
    per-core [NQT,128] "vones" vector that becomes the fused-denominator
    column of V (replaces the 264KB/core mask upload).
  - Output is int8 with a per-row f32 scale packed into the last 4 bytes of
    each row (OUT_INT8; halves the download vs fp16).
  - All inputs ride in ONE int8 blob per core; host packing (quantize,
    transpose, blob assembly) is threaded (ThreadPoolExecutor) since it is
    wall-time before the timed region.
  - jax persistent compilation cache turns recompiles into disk hits.
Measured: 2.655s original -> 0.560s staged baseline -> ~0.406s
(best-of-3; run-to-run noise +-15ms), rel err 1.761e-2 (gate 2e-2).

Per-core device pipeline (fp16 operands, f32 psum):
  - int8 x tiles upcast to fp16 on DVE (values <=127 are exact in fp16).
  - QT = Wq^T @ XqT -> [64,1024] per head; KT likewise [64,1056].
  - V window-major [kpos, 8, 65]; col 64 = vones (validity) -> fused softmax
    denominator that automatically excludes padded keys.
  - Per q-tile (96 queries, 128-key window) and head:
      scoresT = KT_win^T.QT_tile (psum f32); attnT = exp(scoresT) (ACT, f16)
      attnT *= band (gpsimd, inline 0/1 const, broadcast over heads)
      ctx_aug = attnT^T.V_aug (PE); ctx = ctx_aug[:,:64]/den (DVE reciprocal)
      ctxT via PE-transpose -> [512,1024]
  - out = ctxT^T.Wo (+bo) -> [1024,512], per-row absmax/127 int8 quantize,
    scale bitcast into out[:, 512:516] -> DRAM.
"""

import os
import sys

for _p in ("/opt/trn_rl_repo", "/root/.axon_site/_ro/trn_rl_repo"):
    if os.path.isdir(_p) and _p not in sys.path:
        sys.path.insert(0, _p)
        break

import numpy as np
import ml_dtypes

# Persist compiled PJRT executables across calls: run_bass_kernel_spmd builds a
# fresh jit closure per call, so without this every call re-lowers/recompiles
# the identical program (~0.2s) before transferring anything.
try:
    import tempfile

    import jax

    _cache_dir = os.path.join(
        tempfile.gettempdir(), f"jax_comp_cache_{os.getuid()}")
    jax.config.update("jax_compilation_cache_dir", _cache_dir)
    jax.config.update("jax_persistent_cache_min_entry_size_bytes", -1)
    jax.config.update("jax_persistent_cache_min_compile_time_secs", 0.0)
except Exception:
    pass

import concourse.bass as bass
import concourse.tile as tile
from concourse import bacc, bass2jax, mybir

BF16 = ml_dtypes.bfloat16
F16 = np.float16

B, S, D, H, W = 4, 2048, 512, 8, 16
DK = D // H          # 64
NCORES = 8
SH = S // 2          # 1024 rows per core
PADK = SH + 2 * W    # 1056 padded key rows
QT = 96              # q-tile size
NQT = (SH + QT - 1) // QT   # 11 tiles (last = 64)
WIN = QT + 2 * W     # 128-key window per q-tile
SCALE = 1.0 / np.sqrt(DK)
WROWS = 4 * D        # 2048 stacked weight rows
WSH = WROWS // NCORES  # 256 rows per core shard

X_INT8 = True        # upload Q/K/V as int8 (per-column scales folded into W)
OUT_INT8 = True      # download output as int8 + per-row f32 scales
W12 = True           # pack weights as 12-bit planes (vs f16) in the blob

# single-blob input layout (int8-x mode): one ExternalInput array per core.
# Weights ride as 12-bit packed planes: per row, 512 low bytes + 256 shared
# hi-nibble bytes (cols j and j+256 share one byte), plus 2048 f32 scales
# (per-d_out for Wq/Wk, per-d_in for Wv/Wo) duplicated in every core's blob.
SXQ = D * SH          # 524288   xqT int8 [512, 1024]
SXK = D * PADK        # 540672   xkT int8 [512, 1056]
WPACK = D + D // 2    # 768 packed bytes per weight row
OFF_XQ = 0
OFF_XK = OFF_XQ + SXQ
OFF_XV = OFF_XK + SXK
OFF_W = OFF_XV + SXK            # packed weight shard [256, 768] int8
if W12:
    OFF_SC = OFF_W + WSH * WPACK    # weight scales f32 [4*D]
    OFF_V = OFF_SC + 4 * D * 4      # vones int8 [NQT, 128]
else:
    OFF_SC = None
    OFF_V = OFF_W + WSH * D * 2     # f16 weight shard [256, 512]
BLOB = OFF_V + NQT * 128        # 1811840 bytes (W12) / 1869184 (f16)

TRACE = False        # set True (from test.py) to collect an NTFF profile
LAST = {}            # stash for exec_time_ns / profile info

from concurrent.futures import ThreadPoolExecutor

_host_pool = ThreadPoolExecutor(max_workers=8)   # numpy packing parallelism

_programs = {}       # (x_int8, out_int8, has_b) -> compiled nc


def _emit(nc, tc, pools, dram, x_int8, out_int8, has_b):
    dt = mybir.dt
    f16, f32, i8 = dt.float16, dt.float32, dt.int8
    consts, work, psA, psB, psC = pools
    out_d = dram["out"]

    def blob_ap(off, pattern):
        b0 = dram["blob"][0:1]
        return bass.AP(tensor=b0.tensor, offset=off, ap=pattern)

    # ---- weights: bounce -> AllGather -> SBUF ----------------------------
    if x_int8 and W12:
        wch_src = blob_ap(OFF_W, [[WPACK, WSH], [1, WPACK]])
    elif x_int8:
        wch_src = blob_ap(OFF_W, [[D * 2, WSH], [1, D * 2]]).bitcast(f16)
    else:
        wch_src = dram["wchunk"][:, :]
    nc.sync.dma_start(out=dram["wch_b"][:, :], in_=wch_src)
    nc.gpsimd.collective_compute(
        "AllGather",
        mybir.AluOpType.bypass,
        replica_groups=[list(range(NCORES))],
        ins=[dram["wch_b"].ap().opt()],
        outs=[dram["wfull"].ap().opt()],
    )
    sc_sb = None
    w_sb = {}
    if x_int8 and W12:
        # per-chunk weight scales [128, 16]; col 4i+k = scales[512i + 128k + p]
        sc_sb = consts.tile([128, 16], f32, tag="wsc")
        nc.sync.dma_start(
            out=sc_sb[:],
            in_=blob_ap(OFF_SC, [[64, 128], [1, 64]]).bitcast(f32),
        )
        i16 = dt.int16
        for i, name in enumerate(("wq", "wk", "wv", "wo")):
            w_sb[name] = []
            for k in range(4):
                r0 = D * i + 128 * k
                lo8 = work.tile([128, D], i8, tag="wlo8")
                hi8 = work.tile([128, D // 2], i8, tag="whi8")
                nc.sync.dma_start(out=lo8[:], in_=dram["wfull"][r0:r0 + 128, 0:D])
                nc.sync.dma_start(out=hi8[:], in_=dram["wfull"][r0:r0 + 128, D:WPACK])
                lo16 = work.tile([128, D], i16, tag="wlo16")
                hi16 = work.tile([128, D // 2], i16, tag="whi16")
                ev16 = work.tile([128, D // 2], i16, tag="wev16")
                od16 = work.tile([128, D // 2], i16, tag="wod16")
                nc.vector.tensor_copy(out=lo16[:], in_=lo8[:])
                nc.vector.tensor_scalar(
                    out=lo16[:], in0=lo16[:], scalar1=255, scalar2=None,
                    op0=mybir.AluOpType.bitwise_and)
                nc.vector.tensor_copy(out=hi16[:], in_=hi8[:])
                nc.vector.tensor_scalar(
                    out=hi16[:], in0=hi16[:], scalar1=255, scalar2=None,
                    op0=mybir.AluOpType.bitwise_and)
                nc.vector.tensor_scalar(
                    out=ev16[:], in0=hi16[:], scalar1=15, scalar2=8,
                    op0=mybir.AluOpType.bitwise_and,
                    op1=mybir.AluOpType.logical_shift_left)
                nc.vector.tensor_scalar(
                    out=od16[:], in0=hi16[:], scalar1=4, scalar2=8,
                    op0=mybir.AluOpType.logical_shift_right,
                    op1=mybir.AluOpType.logical_shift_left)
                v16 = work.tile([128, D], i16, tag="wv16")
                h = D // 2
                nc.vector.tensor_add(out=v16[:, 0:h], in0=lo16[:, 0:h], in1=ev16[:])
                nc.vector.tensor_add(out=v16[:, h:D], in0=lo16[:, h:D], in1=od16[:])
                nc.vector.tensor_scalar_add(out=v16[:], in0=v16[:], scalar1=-2048)
                t = consts.tile([128, D], f16, tag=f"{name}{k}")
                nc.vector.tensor_copy(out=t[:], in_=v16[:])
                w_sb[name].append(t)
    else:
        for i, name in enumerate(("wq", "wk", "wv", "wo")):
            w_sb[name] = []
            for k in range(4):
                t = consts.tile([128, D], f16, tag=f"{name}{k}")
                r0 = D * i + 128 * k
                nc.sync.dma_start(out=t[:], in_=dram["wfull"][r0:r0 + 128, :])
                w_sb[name].append(t)

    # ---- load x (fp16 direct, or int8-from-blob + DVE upcast) ------------
    def load_xt(key, off, ncols, sc0=None):
        tiles = []
        for k in range(4):
            if x_int8:
                t8 = consts.tile([128, ncols], i8, tag=f"{key}{k}i8")
                nc.sync.dma_start(
                    out=t8[:],
                    in_=blob_ap(off + 128 * k * ncols, [[ncols, 128], [1, ncols]]),
                )
                t = consts.tile([128, ncols], f16, tag=f"{key}{k}")
                nc.vector.tensor_copy(out=t[:], in_=t8[:])
                if sc0 is not None:
                    # fold Wv's per-d_in 12-bit scale into the upcast
                    nc.vector.tensor_scalar_mul(
                        out=t[:], in0=t[:],
                        scalar1=sc_sb[:, sc0 + k:sc0 + k + 1])
            else:
                t = consts.tile([128, ncols], f16, tag=f"{key}{k}")
                nc.sync.dma_start(out=t[:], in_=dram[key][128 * k:128 * (k + 1), :])
            tiles.append(t)
        return tiles

    xqt_sb = load_xt("xqt", OFF_XQ, SH)
    xkt_sb = load_xt("xkt", OFF_XK, PADK)
    xvt_sb = load_xt("xvt", OFF_XV, PADK, sc0=8 if (x_int8 and W12) else None)

    vones_sb = consts.tile([128, NQT], f32, tag="vones")
    if x_int8:
        # vones int8 [NQT, 128] in the blob; partition-first AP transposes
        v8 = consts.tile([128, NQT], i8, tag="vones8")
        nc.sync.dma_start(out=v8[:], in_=blob_ap(OFF_V, [[1, 128], [128, NQT]]))
        nc.vector.tensor_copy(out=vones_sb[:], in_=v8[:])
    else:
        nc.sync.dma_start(
            out=vones_sb[:], in_=dram["vones"].ap().rearrange("t p -> p t"))

    band_sb = consts.tile([128, QT], f16, tag="band")
    nc.sync.dma_start(out=band_sb[:], in_=dram["band"][:])
    ident_sb = consts.tile([QT, QT], f16, tag="ident")
    nc.sync.dma_start(out=ident_sb[:], in_=dram["ident"][:])

    bq_sb = bk_sb = bv_sb = bo_sb = None
    if has_b:
        bq_sb = consts.tile([128, 4], f32, tag="bq")
        nc.sync.dma_start(out=bq_sb[:], in_=dram["bqc"].ap().rearrange("c p -> p c"))
        bk_sb = consts.tile([128, 4], f32, tag="bk")
        nc.sync.dma_start(out=bk_sb[:], in_=dram["bkc"].ap().rearrange("c p -> p c"))
        bv_sb = consts.tile([128, D], f32, tag="bv")
        nc.sync.dma_start(out=bv_sb[:], in_=dram["bvb"][:])
        bo_sb = consts.tile([128, D], f32, tag="bo")
        nc.sync.dma_start(out=bo_sb[:], in_=dram["bob"][:])

    # ---- Q/K projections -> per-head QT [64, SH], KT [64, PADK] (f16) ----
    # Per-head tiles keep every matmul operand at partition offset 0: the HW
    # crashes on (partition-offset operand + intra-bank psum write offset).
    qt_sb, kt_sb = [], []
    for h in range(H):
        qt_sb.append(consts.tile([64, SH], f16, tag=f"qt{h}", name=f"qt{h}"))
        kt_sb.append(consts.tile([64, PADK], f16, tag=f"kt{h}", name=f"kt{h}"))

    def project_T(xt_sb, w, out_tiles, bias_sb, ncols, sc0):
        # head 2m / 2m+1 live in rows 0:64 / 64:128 of dout-chunk m
        for m in range(4):
            c0 = 0
            while c0 < ncols:
                cw = min(512, ncols - c0)
                ps = psA.tile([128, 512], f32, tag="big")
                for k in range(4):
                    nc.tensor.matmul(
                        ps[:, :cw],
                        lhsT=w[k][:, 128 * m:128 * (m + 1)],
                        rhs=xt_sb[k][:, c0:c0 + cw],
                        start=(k == 0),
                        stop=(k == 3),
                    )
                for half in range(2):
                    r0, r1 = 64 * half, 64 * half + 64
                    dst = out_tiles[2 * m + half][:, c0:c0 + cw]
                    src = ps[r0:r1, :cw]
                    if sc0 is not None:
                        # 12-bit weights: scale rows by per-d_out scale
                        # (optionally fused with bias add)
                        if has_b:
                            nc.vector.tensor_scalar(
                                out=dst, in0=src,
                                scalar1=sc_sb[r0:r1, sc0 + m:sc0 + m + 1],
                                scalar2=bias_sb[r0:r1, m:m + 1],
                                op0=mybir.AluOpType.mult,
                                op1=mybir.AluOpType.add,
                            )
                        else:
                            nc.vector.tensor_scalar_mul(
                                out=dst, in0=src,
                                scalar1=sc_sb[r0:r1, sc0 + m:sc0 + m + 1],
                            )
                    elif has_b:
                        nc.vector.tensor_scalar_add(
                            out=dst, in0=src,
                            scalar1=bias_sb[r0:r1, m:m + 1],
                        )
                    else:
                        nc.vector.tensor_copy(out=dst, in_=src)
                c0 += cw

    wsc0 = 0 if (x_int8 and W12) else None
    project_T(xqt_sb, w_sb["wq"], qt_sb, bq_sb, SH, wsc0)
    project_T(xkt_sb, w_sb["wk"], kt_sb, bk_sb, PADK,
              4 if (x_int8 and W12) else None)

    # ---- V projection, window-major; col 64 = vones (validity) -----------
    v_sb = []
    for t in range(NQT):
        w0 = QT * t
        wr = min(WIN, PADK - w0)
        vt = consts.tile([128, H, DK + 1], f16, tag=f"v{t}")
        v_sb.append(vt)
        ps = psA.tile([128, 512], f32, tag="big")
        for k in range(4):
            nc.tensor.matmul(
                ps[:wr, :],
                lhsT=xvt_sb[k][:, w0:w0 + wr],
                rhs=w_sb["wv"][k][:],
                start=(k == 0),
                stop=(k == 3),
            )
        src = ps[:wr, :].rearrange("p (h x) -> p h x", h=H)
        if has_b:
            bvv = bv_sb[:wr, :].rearrange("p (h x) -> p h x", h=H)
            nc.vector.tensor_add(out=vt[:wr, :, 0:DK], in0=src, in1=bvv)
            # zero out padded-key rows so bias doesn't leak into the band sum
            nc.vector.tensor_scalar_mul(
                out=vt[:wr, :, 0:DK],
                in0=vt[:wr, :, 0:DK],
                scalar1=vones_sb[:wr, t:t + 1],
            )
        else:
            nc.vector.tensor_copy(out=vt[:wr, :, 0:DK], in_=src)
        vb = vones_sb[:wr, t:t + 1]
        vb_bc = bass.AP(
            tensor=vb.tensor, offset=vb.offset,
            ap=[vb.ap[0], [0, H], vb.ap[1]],
        )
        nc.vector.tensor_copy(out=vt[:wr, :, DK:DK + 1], in_=vb_bc)

    # ---- attention -------------------------------------------------------
    ctxT_sb = []
    for c in range(4):
        ctxT_sb.append(consts.tile([128, SH], f16, tag=f"ctxT{c}", name=f"ctxT{c}"))

    head_groups = ((0, 5), (5, 8))
    for t in range(NQT):
        q0 = QT * t
        qw = min(QT, SH - q0)
        w0 = QT * t
        wr = min(WIN, PADK - w0)

        attn_sb = work.tile([128, H, QT], f16, tag="attn")
        for h0, h1 in head_groups:
            nh = h1 - h0
            ps_sc = psB.tile([128, 5, QT], f32, tag="sc")
            for j, h in enumerate(range(h0, h1)):
                nc.tensor.matmul(
                    ps_sc[:wr, j, :qw],
                    lhsT=kt_sb[h][:, w0:w0 + wr],
                    rhs=qt_sb[h][:, q0:q0 + qw],
                    start=True,
                    stop=True,
                )
            nc.scalar.activation(
                out=attn_sb[:wr, h0:h1, :qw],
                in_=ps_sc[:wr, :nh, :qw],
                func=mybir.ActivationFunctionType.Exp,
            )

        # multiplicative band mask, broadcast over heads (gpsimd)
        mbase = band_sb[:wr, :qw]
        mask_bc = bass.AP(
            tensor=mbase.tensor, offset=mbase.offset,
            ap=[mbase.ap[0], [0, H], mbase.ap[1]],
        )
        nc.gpsimd.tensor_mul(
            out=attn_sb[:wr, :, :qw], in0=attn_sb[:wr, :, :qw], in1=mask_bc
        )

        recip_sb = work.tile([QT, H], f32, tag="recip")
        ctx_sb = work.tile([QT, H, DK], f16, tag="ctx")
        for g in range(2):
            ps_ctx = psC.tile([QT, 4, DK + 1], f32, tag="ctx")
            for j, h in enumerate(range(4 * g, 4 * g + 4)):
                nc.tensor.matmul(
                    ps_ctx[:qw, j, :],
                    lhsT=attn_sb[:wr, h, :qw],
                    rhs=v_sb[t][:wr, h, :],
                    start=True,
                    stop=True,
                )
            nc.vector.reciprocal(
                out=recip_sb[:qw, 4 * g:4 * g + 4],
                in_=ps_ctx[:qw, :, DK:DK + 1],
            )
            rbase = recip_sb[:qw, 4 * g:4 * g + 4]
            recip_bc = bass.AP(
                tensor=rbase.tensor, offset=rbase.offset,
                ap=[rbase.ap[0], rbase.ap[1], [0, DK]],
            )
            nc.vector.tensor_mul(
                out=ctx_sb[:qw, 4 * g:4 * g + 4, :],
                in0=ps_ctx[:qw, :, 0:DK],
                in1=recip_bc,
            )

        # transpose ctx [qw, 512] -> ctxT [512, qw]  (4 chunks of 128)
        for c in range(4):
            ps_t = psA.tile([128, QT], f16, tag="big")
            nc.tensor.transpose(
                out=ps_t[:, :qw],
                in_=ctx_sb[:qw, 2 * c:2 * c + 2, :],
                identity=ident_sb[:qw, :qw],
            )
            if x_int8 and W12:
                # fold Wo's per-d_in 12-bit scale into the ctxT copy
                nc.vector.tensor_scalar_mul(
                    out=ctxT_sb[c][:, q0:q0 + qw], in0=ps_t[:, :qw],
                    scalar1=sc_sb[:, 12 + c:12 + c + 1])
            else:
                nc.vector.tensor_copy(
                    out=ctxT_sb[c][:, q0:q0 + qw], in_=ps_t[:, :qw])

    # ---- O-projection ----------------------------------------------------
    for mt in range(8):
        r0 = 128 * mt
        ps = psA.tile([128, 512], f32, tag="big")
        for k in range(4):
            nc.tensor.matmul(
                ps[:],
                lhsT=ctxT_sb[k][:, r0:r0 + 128],
                rhs=w_sb["wo"][k][:],
                start=(k == 0),
                stop=(k == 3),
            )
        src = ps[:]
        if has_b:
            of_sb = work.tile([128, D], f32, tag="osbf")
            nc.vector.tensor_add(out=of_sb[:], in0=ps[:], in1=bo_sb[:])
            src = of_sb[:]
        if out_int8:
            # per-row int8 quantization; scale = absmax/127 rides in the last
            # 4 bytes of each int8 output row (bitcast f32)
            amax_sb = work.tile([128, 1], f32, tag="amax")
            osc_sb = work.tile([128, 1], f32, tag="osc")
            rsc_sb = work.tile([128, 1], f32, tag="rsc")
            o_sb = work.tile([128, D], dt.int8, tag="osb8")
            nc.vector.tensor_reduce(
                out=amax_sb[:], in_=src,
                axis=mybir.AxisListType.X, op=mybir.AluOpType.max,
                apply_absolute_value=True,
            )
            nc.vector.tensor_scalar_max(out=amax_sb[:], in0=amax_sb[:], scalar1=1e-30)
            nc.vector.tensor_scalar_mul(out=osc_sb[:], in0=amax_sb[:], scalar1=1.0 / 127.0)
            nc.vector.reciprocal(out=rsc_sb[:], in_=osc_sb[:])
            nc.vector.tensor_scalar_mul(out=o_sb[:], in0=src, scalar1=rsc_sb[:, 0:1])
            nc.sync.dma_start(out=out_d[r0:r0 + 128, 0:D], in_=o_sb[:])
            nc.sync.dma_start(
                out=out_d[r0:r0 + 128, D:D + 4].bitcast(f32), in_=osc_sb[:]
            )
        else:
            o_sb = work.tile([128, D], f16, tag="osb")
            nc.vector.tensor_copy(out=o_sb[:], in_=src)
            nc.sync.dma_start(out=out_d[r0:r0 + 128, :], in_=o_sb[:])


def _build_band() -> np.ndarray:
    i = np.arange(128)[:, None]   # window row (key)
    j = np.arange(QT)[None, :]    # q column
    band = (i - j >= 0) & (i - j <= 2 * W)
    return band.astype(F16)


def _build_program(x_int8: bool, out_int8: bool, has_b: bool):
    dt = mybir.dt
    f16, f32 = dt.float16, dt.float32
    xdt = dt.int8 if x_int8 else f16
    odt = dt.int8 if out_int8 else f16

    nc = bacc.Bacc("TRN2", target_bir_lowering=False, debug=False, num_devices=NCORES)

    dram = {}
    if x_int8:
        dram["blob"] = nc.dram_tensor("blob", [BLOB], dt.int8, kind="ExternalInput")
    else:
        dram["xqt"] = nc.dram_tensor("xqt", [D, SH], xdt, kind="ExternalInput")
        dram["xkt"] = nc.dram_tensor("xkt", [D, PADK], xdt, kind="ExternalInput")
        dram["xvt"] = nc.dram_tensor("xvt", [D, PADK], xdt, kind="ExternalInput")
        dram["wchunk"] = nc.dram_tensor("wchunk", [WSH, D], f16, kind="ExternalInput")
        dram["vones"] = nc.dram_tensor("vones", [NQT, 128], f32, kind="ExternalInput")
    dram.update({
        "out": nc.dram_tensor(
            "out", [SH, D + 4] if out_int8 else [SH, D], odt, kind="ExternalOutput"),
        "band": nc.inline_tensor(_build_band(), name="band"),
        "ident": nc.inline_tensor(np.eye(QT, dtype=F16), name="ident"),
    })
    if x_int8 and W12:
        dram["wch_b"] = nc.dram_tensor("wch_b", [WSH, WPACK], dt.int8)
        dram["wfull"] = nc.dram_tensor("wfull", [WROWS, WPACK], dt.int8)
    else:
        dram["wch_b"] = nc.dram_tensor("wch_b", [WSH, D], f16)
        dram["wfull"] = nc.dram_tensor("wfull", [WROWS, D], f16)
    if has_b:
        dram["bqc"] = nc.dram_tensor("bqc", [4, 128], f32, kind="ExternalInput")
        dram["bkc"] = nc.dram_tensor("bkc", [4, 128], f32, kind="ExternalInput")
        dram["bvb"] = nc.dram_tensor("bvb", [128, D], f32, kind="ExternalInput")
        dram["bob"] = nc.dram_tensor("bob", [128, D], f32, kind="ExternalInput")

    with tile.TileContext(nc) as tc:
        with (
            tc.tile_pool(name="consts", bufs=1) as consts,
            tc.tile_pool(name="work", bufs=3) as work,
            tc.tile_pool(name="psA", bufs=2, space="PSUM") as psA,
            tc.tile_pool(name="psB", bufs=2, space="PSUM") as psB,
            tc.tile_pool(name="psC", bufs=4, space="PSUM") as psC,
        ):
            _emit(nc, tc, (consts, work, psA, psB, psC), dram, x_int8, out_int8, has_b)

    nc.compile()
    return nc


def _get_program(x_int8, out_int8, has_b):
    key = (x_int8, out_int8, has_b)
    if key not in _programs:
        _programs[key] = _build_program(x_int8, out_int8, has_b)
    return _programs[key]


# ---------------------------------------------------------------------------
# Custom PJRT runner. Same _bass_exec_p path as bass2jax.run_bass_via_pjrt,
# with two wall-time fixes (the measured metric is transfer-bound over the
# axon tunnel, which serializes all RPC + data bytes in both directions):
#   - the donated output buffers are created ON DEVICE by a separate tiny jit
#     (jnp.zeros + out_shardings) instead of uploading host np.zeros — saves
#     the 4.2MB zero upload (~100ms). They can't be created inside the same
#     jit: neuronx_cc_hook requires every bass_exec operand to be a plain HLO
#     parameter.
#   - everything is issued async (zeros jit, sharded device_put, NEFF call)
#     and only the final np.asarray blocks, so per-RPC round-trip latencies
#     (~60-80ms each when blocked individually) overlap with the data stream.
# ---------------------------------------------------------------------------

_runners = {}


def _make_runner(nc):
    import jax
    import jax.numpy as jnp
    from jax.sharding import Mesh, NamedSharding, PartitionSpec
    import warnings
    with warnings.catch_warnings():
        warnings.simplefilter("ignore")
        from jax.experimental.shard_map import shard_map

    bass2jax.install_neuronx_cc_hook()
    partition_name = nc.partition_id_tensor.name if nc.partition_id_tensor else None
    in_names, out_names, out_avals = [], [], []
    for alloc in nc.m.functions[0].allocations:
        if not isinstance(alloc, mybir.MemoryLocationSet):
            continue
        name = alloc.memorylocations[0].name
        if alloc.kind == "ExternalInput":
            if name != partition_name:
                in_names.append(name)
        elif alloc.kind == "ExternalOutput":
            out_names.append(name)
            out_avals.append(
                jax.core.ShapedArray(
                    tuple(alloc.tensor_shape), mybir.dt.np(alloc.dtype)))
    n_params = len(in_names)
    n_outs = len(out_avals)
    in_names_all = in_names + out_names
    if partition_name is not None:
        in_names_all.append(partition_name)

    def _body(*args):
        operands = list(args)
        if partition_name is not None:
            operands.append(bass2jax.partition_id_tensor())
        outs = bass2jax._bass_exec_p.bind(
            *operands,
            out_avals=tuple(out_avals),
            in_names=tuple(in_names_all),
            out_names=tuple(out_names),
            lowering_input_output_aliases=(),
            sim_require_finite=True,
            sim_require_nnan=True,
            nc=nc,
        )
        return tuple(outs)

    devices = jax.devices()[:NCORES]
    mesh = Mesh(np.asarray(devices), ("core",))
    spec = NamedSharding(mesh, PartitionSpec("core"))
    in_specs = (PartitionSpec("core"),) * (n_params + n_outs)
    out_specs = (PartitionSpec("core"),) * n_outs
    # No donation: our kernel writes every output element, so the NEFF's
    # output operands never need meaningful content. A single device-side
    # zeros tuple is created once and passed (never consumed) every call —
    # zero per-call cost on the terminal's serial RPC queue.
    sharded = jax.jit(
        shard_map(_body, mesh=mesh, in_specs=in_specs, out_specs=out_specs,
                  check_rep=False),
        keep_unused=True)
    gshapes = [(NCORES * a.shape[0], *a.shape[1:]) for a in out_avals]
    zeros_fn = jax.jit(
        lambda: tuple(jnp.zeros(s, a.dtype) for s, a in zip(gshapes, out_avals)),
        out_shardings=(spec,) * n_outs)
    zeros_persist = zeros_fn()
    jax.block_until_ready(zeros_persist)

    def run(concat_in):
        """concat_in: list of global [NCORES*rows, ...] arrays in in_names
        order. Returns list of global output arrays (np, gathered)."""
        g_in = [jax.device_put(a, spec) for a in concat_in]   # async upload
        outs = sharded(*g_in, *zeros_persist)            # async NEFF exec
        return [np.asarray(o) for o in outs]             # blocks

    run.in_names = in_names
    return run


def _get_runner(nc):
    if id(nc) not in _runners:
        _runners[id(nc)] = _make_runner(nc)
    return _runners[id(nc)]


def _build_vones(half: int) -> np.ndarray:
    # vones[t, i] = 1.0 iff padded K/V row (96t + i) holds a real key
    v = np.zeros((NQT, 128), np.float32)
    r = QT * np.arange(NQT)[:, None] + np.arange(128)[None, :]
    lo, hi = (W, PADK) if half == 0 else (0, PADK - W)
    v[:] = ((r >= lo) & (r < hi)).astype(np.float32)
    return v


_vones_cache = {}


def kernel(query, key, value, Wq, bq, Wk, bk, Wv, bv, Wo, bo):
    query = np.asarray(query, np.float32)
    key = np.asarray(key, np.float32)
    value = np.asarray(value, np.float32)
    Wq = np.asarray(Wq, np.float32)
    Wk = np.asarray(Wk, np.float32)
    Wv = np.asarray(Wv, np.float32)
    Wo = np.asarray(Wo, np.float32)
    bq = np.asarray(bq, np.float32)
    bk = np.asarray(bk, np.float32)
    bv = np.asarray(bv, np.float32)
    bo = np.asarray(bo, np.float32)

    has_b = bool(np.any(bq) or np.any(bk) or np.any(bv) or np.any(bo))
    x_int8 = X_INT8
    out_int8 = OUT_INT8
    nc = _get_program(x_int8, out_int8, has_b)

    if x_int8:
        # per-column int8 scales, folded into the weight rows on the host

        def colmax(x):
            return np.maximum(np.abs(x).max(axis=(0, 1)) / 127.0, 1e-30)

        def quant(x, s):
            # s = absmax/127 bounds |x/s| <= 127 (+1 ulp, absorbed by rint),
            # so no clip pass is needed
            t = x * (1.0 / s).astype(np.float32)
            np.rint(t, out=t)
            return t.astype(np.int8)

        sq, sk, sv = _host_pool.map(colmax, (query, key, value))
        qx, kx, vx = _host_pool.map(
            lambda a: quant(*a), ((query, sq), (key, sk), (value, sv)))
        wq_f = Wq * (sq[:, None] * SCALE)
        wk_f = Wk * sk[:, None]
        wv_f = Wv * sv[:, None]
        xdt = np.int8
    else:
        wq_f = Wq * SCALE
        wk_f = Wk
        wv_f = Wv
        qx, kx, vx = query.astype(F16), key.astype(F16), value.astype(F16)
        xdt = F16

    if x_int8 and W12:
        # 12-bit weight quantization: Wq/Wk per-column (d_out, scale applied
        # on the projection psum rows), Wv/Wo per-row (d_in, scale folded
        # into the xvT upcast / ctxT copy respectively)
        def q12(w, axis):
            s = np.maximum(np.abs(w).max(axis=axis), 1e-30) / 2047.0
            q = np.rint(w / (s[None, :] if axis == 0 else s[:, None]))
            return q.astype(np.int32), s.astype(np.float32)
        qq, sq_w = q12(wq_f, 0)
        qk, sk_w = q12(wk_f, 0)
        qv, sv_w = q12(wv_f, 1)
        qo, so_w = q12(Wo, 1)
        u = (np.concatenate([qq, qk, qv, qo], axis=0) + 2048).astype(np.uint16)
        lo = (u & 255).astype(np.uint8)
        hi4 = (u >> 8).astype(np.uint8)
        hi = hi4[:, :D // 2] | (hi4[:, D // 2:] << 4)
        wpacked = np.concatenate([lo, hi], axis=1).view(np.int8)  # [2048, 768]
        # transposed [p, chunk] layout so the device DMA reads contiguous
        # 64-byte runs per partition: wscales_t[p*16 + c] = s[128c + p]
        wscales = np.ascontiguousarray(
            np.concatenate([sq_w, sk_w, sv_w, so_w]).reshape(16, 128).T
        ).reshape(-1).view(np.int8)
        wstack = None
    else:
        wstack = np.ascontiguousarray(
            np.concatenate([wq_f, wk_f, wv_f, Wo], axis=0).astype(F16))

    if not _vones_cache:
        _vones_cache[0] = _build_vones(0)
        _vones_cache[1] = _build_vones(1)

    globals_by_name = {}
    if x_int8:
        gblob = np.empty((NCORES, BLOB), np.int8)
        globals_by_name["blob"] = gblob
    else:
        globals_by_name["xqt"] = np.empty((NCORES * D, SH), F16)
        globals_by_name["xkt"] = np.empty((NCORES * D, PADK), F16)
        globals_by_name["xvt"] = np.empty((NCORES * D, PADK), F16)
        globals_by_name["wchunk"] = np.empty((NCORES * WSH, D), F16)
        globals_by_name["vones"] = np.empty((NCORES * NQT, 128), np.float32)
    def pack_core(core):
        b, half = core // 2, core % 2
        s0 = half * SH
        xq = qx[b, s0:s0 + SH]
        lo, hi = s0 - W, s0 + SH + W
        clo, chi = max(lo, 0), min(hi, S)
        xk = np.zeros((PADK, D), xdt)
        xv = np.zeros((PADK, D), xdt)
        xk[clo - lo:chi - lo] = kx[b, clo:chi]
        xv[clo - lo:chi - lo] = vx[b, clo:chi]

        if x_int8:
            blob = gblob[core]
            blob[OFF_XQ:OFF_XK] = xq.T.reshape(-1)
            blob[OFF_XK:OFF_XV] = xk.T.reshape(-1)
            blob[OFF_XV:OFF_W] = xv.T.reshape(-1)
            if W12:
                blob[OFF_W:OFF_SC] = wpacked[WSH * core:WSH * (core + 1)].reshape(-1)
                blob[OFF_SC:OFF_V] = wscales
            else:
                blob[OFF_W:OFF_V] = wstack[WSH * core:WSH * (core + 1)].view(np.int8).reshape(-1)
            blob[OFF_V:BLOB] = _vones_cache[half].astype(np.int8).reshape(-1)
        else:
            wchunk = wstack[WSH * core:WSH * (core + 1)]
            globals_by_name["xqt"][core * D:(core + 1) * D] = xq.T
            globals_by_name["xkt"][core * D:(core + 1) * D] = xk.T
            globals_by_name["xvt"][core * D:(core + 1) * D] = xv.T
            globals_by_name["wchunk"][core * WSH:(core + 1) * WSH] = wchunk
            globals_by_name["vones"][core * NQT:(core + 1) * NQT] = _vones_cache[half]

    list(_host_pool.map(pack_core, range(NCORES)))
    if has_b:
        def rep(name, arr):
            g = np.empty((NCORES * arr.shape[0], *arr.shape[1:]), arr.dtype)
            g[:] = np.tile(arr, (NCORES,) + (1,) * (arr.ndim - 1))
            globals_by_name[name] = g
        rep("bqc", np.ascontiguousarray((bq * SCALE).reshape(4, 128)))
        rep("bkc", np.ascontiguousarray(bk.reshape(4, 128)))
        rep("bvb", np.broadcast_to(bv, (128, D)).astype(np.float32))
        rep("bob", np.broadcast_to(bo, (128, D)).astype(np.float32))
    if x_int8:
        globals_by_name["blob"] = gblob.reshape(-1)

    import time as _time
    run = _get_runner(nc)
    concat_in = [globals_by_name[nm] for nm in run.in_names]
    outs = run(concat_in)
    if TRACE:
        best = None
        for _ in range(5):
            t0 = _time.perf_counter()
            outs = run(concat_in)
            dtns = (_time.perf_counter() - t0) * 1e9
            best = dtns if best is None else min(best, dtns)
        LAST["wall_ns"] = best
    LAST["exec_time_ns"] = None

    ow = D + 4 if out_int8 else D
    oglob = outs[0].reshape(NCORES, SH, ow)
    out = np.empty((B, S, D), np.float32)

    def unpack_core(core):
        b, half = core // 2, core % 2
        o = oglob[core]
        dst = out[b, half * SH:(half + 1) * SH]
        if out_int8:
            scale = np.ascontiguousarray(o[:, D:D + 4]).view(np.float32)
            np.multiply(o[:, 0:D], scale, out=dst, dtype=np.float32)
        else:
            dst[:] = o

    list(_host_pool.map(unpack_core, range(NCORES)))
    return out


if __name__ == "__main__":
    rng = np.random.default_rng(0)
    sc = 1.0 / np.sqrt(D)
    inputs = {
        "query": rng.standard_normal((B, S, D)).astype(np.float32),
        "key": rng.standard_normal((B, S, D)).astype(np.float32),
        "value": rng.standard_normal((B, S, D)).astype(np.float32),
        "Wq": (rng.standard_normal((D, D)) * sc).astype(np.float32),
        "bq": np.zeros(D, np.float32),
        "Wk": (rng.standard_normal((D, D)) * sc).astype(np.float32),
        "bk": np.zeros(D, np.float32),
        "Wv": (rng.standard_normal((D, D)) * sc).astype(np.float32),
        "bv": np.zeros(D, np.float32),
        "Wo": (rng.standard_normal((D, D)) * sc).astype(np.float32),
        "bo": np.zeros(D, np.float32),
    }
    out = kernel(**inputs)
    print("out", out.shape, out.dtype, out[0, 0, :4])



# revision 35
# speedup vs baseline: 1.0657x; 1.0015x over previous
"""Local (banded) attention kernel for Trainium2, 8 NeuronCores SPMD.

Problem: nn_LocalAttention  (B=4, S=2048, D=512, H=8 heads, DK=64, band W=16)
  out = (softmax(band_mask(QK^T/sqrt(DK))) V) Wo + bo   with Q/K/V = x W* + b*

Sharding: 8 cores = 4 batches x 2 sequence halves. Each core computes its
1024-query slice end-to-end. K/V get a 16-row halo (zero-padded at sequence
ends) so no inter-core attention communication is needed.

The measured metric is the end-to-end wall time of one full execution
(upload + NEFF exec + download) over the axon tunnel, which serializes all
RPC + data bytes in both directions at ~50MB/s peak (~19.6ms/MB floor even
for all-zero payloads, plus a content-entropy-dependent extra of up to
~8ms/MB; transfers do NOT overlap each other or exec, and each *blocked*
RPC costs a ~80ms round trip that async dispatch hides). On-device compute
is a few ms. The kernel therefore minimizes moved bytes and round trips:
  - Custom async PJRT runner (_make_runner): same _bass_exec_p path as
    bass2jax.run_bass_via_pjrt, but (a) the NEFF's output-buffer operands
    are a PERSISTENT device-side zeros tuple created once at runner build
    with no donation (our kernel writes every output element, so their
    content never matters) — saves the 4.2MB host zeros upload (~100ms)
    and all per-call zeros work, and (b) device_put + NEFF call are issued
    async with only the final np.asarray blocking (hides ~3 RPC round
    trips, ~150ms). One device_put of one concatenated global array = one
    streamed transfer (8 separate per-device puts pay ~45ms fixed cost
    EACH and serialize; 2 split puts are ~90ms worse than 1).
  - Q/K/V uploaded int8 with per-column scales folded into the weights on
    the host (X_INT8; per-tensor scales fail the 2e-2 gate, per-column
    passes). X_INT8=False falls back to fp16 (rel ~7e-4).
  - Weights uploaded once as 1/8 shards, 12-bit packed (W12: per row 512
    low bytes + 256 shared hi-nibble bytes for cols j/j+256, plus per-
    column scales; <= f16 abs error), AllGather'd and unpacked on device
    with int16 DVE bit ops; scales are applied per-partition at the
    projection psum-copy (Wq/Wk, per-d_out), the xvT upcast (Wv, per-d_in)
    and the ctxT copy (Wo, per-d_in). 1.69MB vs 2.10MB f16.
  - Band mask is an inline NEFF constant; sequence-edge validity is a tiny
    per-core [NQT,128] "vones" vector that becomes the fused-denominator
    column of V (replaces the 264KB/core mask upload).
  - Output is int8 with a per-row f32 scale packed into the last 4 bytes of
    each row (OUT_INT8; halves the download vs fp16).
  - All inputs ride in ONE int8 blob per core; host packing (quantize,
    transpose, blob assembly) is threaded (ThreadPoolExecutor) since it is
    wall-time before the timed region.
  - jax persistent compilation cache turns recompiles into disk hits.
Measured: 2.655s original -> 0.560s staged baseline -> ~0.406s
(best-of-3; run-to-run noise +-15ms), rel err 1.761e-2 (gate 2e-2).

Per-core device pipeline (fp16 operands, f32 psum):
  - int8 x tiles upcast to fp16 on DVE (values <=127 are exact in fp16).
  - QT = Wq^T @ XqT -> [64,1024] per head; KT likewise [64,1056].
  - V window-major [kpos, 8, 65]; col 64 = vones (validity) -> fused softmax
    denominator that automatically excludes padded keys.
  - Per q-tile (96 queries, 128-key window) and head:
      scoresT = KT_win^T.QT_tile (psum f32); attnT = exp(scoresT) (ACT, f16)
      attnT *= band (gpsimd, inline 0/1 const, broadcast over heads)
      ctx_aug = attnT^T.V_aug (PE); ctx = ctx_aug[:,:64]/den (DVE reciprocal)
      ctxT via PE-transpose -> [512,1024]
  - out = ctxT^T.Wo (+bo) -> [1024,512], per-row absmax/127 int8 quantize,
    scale bitcast into out[:, 512:516] -> DRAM.
"""

import os
import sys

for _p in ("/opt/trn_rl_repo", "/root/.axon_site/_ro/trn_rl_repo"):
    if os.path.isdir(_p) and _p not in sys.path:
        sys.path.insert(0, _p)
        break

import numpy as np
import ml_dtypes

# Persist compiled PJRT executables across calls: run_bass_kernel_spmd builds a
# fresh jit closure per call, so without this every call re-lowers/recompiles
# the identical program (~0.2s) before transferring anything.
try:
    import tempfile

    import jax

    _cache_dir = os.path.join(
        tempfile.gettempdir(), f"jax_comp_cache_{os.getuid()}")
    jax.config.update("jax_compilation_cache_dir", _cache_dir)
    jax.config.update("jax_persistent_cache_min_entry_size_bytes", -1)
    jax.config.update("jax_persistent_cache_min_compile_time_secs", 0.0)
except Exception:
    pass

import concourse.bass as bass
import concourse.tile as tile
from concourse import bacc, bass2jax, mybir

BF16 = ml_dtypes.bfloat16
F16 = np.float16

B, S, D, H, W = 4, 2048, 512, 8, 16
DK = D // H          # 64
NCORES = 8
SH = S // 2          # 1024 rows per core
PADK = SH + 2 * W    # 1056 padded key rows
QT = 96              # q-tile size
NQT = (SH + QT - 1) // QT   # 11 tiles (last = 64)
WIN = QT + 2 * W     # 128-key window per q-tile
SCALE = 1.0 / np.sqrt(DK)
WROWS = 4 * D        # 2048 stacked weight rows
WSH = WROWS // NCORES  # 256 rows per core shard

X_INT8 = True        # upload Q/K/V as int8 (per-column scales folded into W)
OUT_INT8 = True      # download output as int8 + per-row f32 scales
W12 = True           # pack weights as 12-bit planes (vs f16) in the blob

# single-blob input layout (int8-x mode): one ExternalInput array per core.
# Weights ride as 12-bit packed planes: per row, 512 low bytes + 256 shared
# hi-nibble bytes (cols j and j+256 share one byte), plus 2048 f32 scales
# (per-d_out for Wq/Wk, per-d_in for Wv/Wo) duplicated in every core's blob.
SXQ = D * SH          # 524288   xqT int8 [512, 1024]
WPACK = D + D // 2    # 768 packed bytes per weight row
OFF_XQ = 0
OFF_XK = OFF_XQ + SXQ
OFF_XV = OFF_XK + SXQ           # xk/xv ship [512, 1024] own rows only;
OFF_W = OFF_XV + SXQ            # 16-col halos are exchanged on device
if W12:
    OFF_SC = OFF_W + WSH * WPACK    # weight scales f32 [4*D]
    OFF_V = OFF_SC + 4 * D * 4      # vones int8 [NQT, 128]
else:
    OFF_SC = None
    OFF_V = OFF_W + WSH * D * 2     # f16 weight shard [256, 512]
OFF_FL = OFF_V + NQT * 128      # halo flags f32 [128, 2]: col0=p, col1=1-p
BLOB = OFF_FL + 128 * 2 * 4     # 1780096 bytes (W12)

TRACE = False        # set True (from test.py) to collect an NTFF profile
LAST = {}            # stash for exec_time_ns / profile info

from concurrent.futures import ThreadPoolExecutor

_host_pool = ThreadPoolExecutor(max_workers=8)   # numpy packing parallelism

_programs = {}       # (x_int8, out_int8, has_b) -> compiled nc


def _emit(nc, tc, pools, dram, x_int8, out_int8, has_b):
    dt = mybir.dt
    f16, f32, i8 = dt.float16, dt.float32, dt.int8
    consts, work, psA, psB, psC = pools
    out_d = dram["out"]

    def blob_ap(off, pattern):
        b0 = dram["blob"][0:1]
        return bass.AP(tensor=b0.tensor, offset=off, ap=pattern)

    # ---- weights: bounce -> AllGather -> SBUF ----------------------------
    if x_int8 and W12:
        wch_src = blob_ap(OFF_W, [[WPACK, WSH], [1, WPACK]])
    elif x_int8:
        wch_src = blob_ap(OFF_W, [[D * 2, WSH], [1, D * 2]]).bitcast(f16)
    else:
        wch_src = dram["wchunk"][:, :]
    nc.sync.dma_start(out=dram["wch_b"][:, :], in_=wch_src)
    nc.gpsimd.collective_compute(
        "AllGather",
        mybir.AluOpType.bypass,
        replica_groups=[list(range(NCORES))],
        ins=[dram["wch_b"].ap().opt()],
        outs=[dram["wfull"].ap().opt()],
    )
    sc_sb = None
    w_sb = {}
    if x_int8 and W12:
        # per-chunk weight scales [128, 16]; col 4i+k = scales[512i + 128k + p]
        sc_sb = consts.tile([128, 16], f32, tag="wsc")
        nc.sync.dma_start(
            out=sc_sb[:],
            in_=blob_ap(OFF_SC, [[64, 128], [1, 64]]).bitcast(f32),
        )
        i16 = dt.int16
        for i, name in enumerate(("wq", "wk", "wv", "wo")):
            w_sb[name] = []
            for k in range(4):
                r0 = D * i + 128 * k
                lo8 = work.tile([128, D], i8, tag="wlo8")
                hi8 = work.tile([128, D // 2], i8, tag="whi8")
                nc.sync.dma_start(out=lo8[:], in_=dram["wfull"][r0:r0 + 128, 0:D])
                nc.sync.dma_start(out=hi8[:], in_=dram["wfull"][r0:r0 + 128, D:WPACK])
                lo16 = work.tile([128, D], i16, tag="wlo16")
                hi16 = work.tile([128, D // 2], i16, tag="whi16")
                ev16 = work.tile([128, D // 2], i16, tag="wev16")
                od16 = work.tile([128, D // 2], i16, tag="wod16")
                nc.vector.tensor_copy(out=lo16[:], in_=lo8[:])
                nc.vector.tensor_scalar(
                    out=lo16[:], in0=lo16[:], scalar1=255, scalar2=None,
                    op0=mybir.AluOpType.bitwise_and)
                nc.vector.tensor_copy(out=hi16[:], in_=hi8[:])
                nc.vector.tensor_scalar(
                    out=hi16[:], in0=hi16[:], scalar1=255, scalar2=None,
                    op0=mybir.AluOpType.bitwise_and)
                nc.vector.tensor_scalar(
                    out=ev16[:], in0=hi16[:], scalar1=15, scalar2=8,
                    op0=mybir.AluOpType.bitwise_and,
                    op1=mybir.AluOpType.logical_shift_left)
                nc.vector.tensor_scalar(
                    out=od16[:], in0=hi16[:], scalar1=4, scalar2=8,
                    op0=mybir.AluOpType.logical_shift_right,
                    op1=mybir.AluOpType.logical_shift_left)
                v16 = work.tile([128, D], i16, tag="wv16")
                h = D // 2
                nc.vector.tensor_add(out=v16[:, 0:h], in0=lo16[:, 0:h], in1=ev16[:])
                nc.vector.tensor_add(out=v16[:, h:D], in0=lo16[:, h:D], in1=od16[:])
                nc.vector.tensor_scalar_add(out=v16[:], in0=v16[:], scalar1=-2048)
                t = consts.tile([128, D], f16, tag=f"{name}{k}")
                nc.vector.tensor_copy(out=t[:], in_=v16[:])
                w_sb[name].append(t)
    else:
        for i, name in enumerate(("wq", "wk", "wv", "wo")):
            w_sb[name] = []
            for k in range(4):
                t = consts.tile([128, D], f16, tag=f"{name}{k}")
                r0 = D * i + 128 * k
                nc.sync.dma_start(out=t[:], in_=dram["wfull"][r0:r0 + 128, :])
                w_sb[name].append(t)

    # ---- load x (fp16 direct, or int8-from-blob + DVE upcast) ------------
    # ---- halo exchange: xk/xv ship without their 16-col halos; core pairs
    # (2b, 2b+1) AllGather their edge columns and each core rebuilds its
    # halos, zeroing the sequence-boundary side via per-core 0/1 flags ----
    fl_sb = None
    if x_int8:
        fl_sb = consts.tile([128, 2], f32, tag="hflags")
        nc.sync.dma_start(
            out=fl_sb[:], in_=blob_ap(OFF_FL, [[8, 128], [1, 8]]).bitcast(f32))
        # edge_b [512, 64]: cols 0:16/16:32 = xk left/right edge,
        #                   cols 32:48/48:64 = xv left/right edge
        for off, base in ((OFF_XK, 0), (OFF_XV, 32)):
            nc.sync.dma_start(
                out=dram["edge_b"][:, base:base + W],
                in_=blob_ap(off, [[SH, 512], [1, W]]))
            nc.sync.dma_start(
                out=dram["edge_b"][:, base + W:base + 2 * W],
                in_=blob_ap(off + SH - W, [[SH, 512], [1, W]]))
        nc.gpsimd.collective_compute(
            "AllGather",
            mybir.AluOpType.bypass,
            replica_groups=[[2 * b, 2 * b + 1] for b in range(NCORES // 2)],
            ins=[dram["edge_b"].ap().opt()],
            outs=[dram["edge_g"].ap().opt()],
        )

    def load_xt(key, off, sc0=None, halo_base=None):
        tiles = []
        for k in range(4):
            if x_int8:
                ncols = SH if halo_base is None else PADK
                t8 = consts.tile([128, ncols], i8, tag=f"{key}{k}i8")
                c0 = 0 if halo_base is None else W
                nc.sync.dma_start(
                    out=t8[:, c0:c0 + SH],
                    in_=blob_ap(off + 128 * k * SH, [[SH, 128], [1, SH]]),
                )
                if halo_base is not None:
                    # left halo <- even core's right edge (slot 0);
                    # right halo <- odd core's left edge (slot 1)
                    r0 = 128 * k
                    nc.sync.dma_start(
                        out=t8[:, 0:W],
                        in_=dram["edge_g"][r0:r0 + 128,
                                           halo_base + W:halo_base + 2 * W])
                    nc.sync.dma_start(
                        out=t8[:, W + SH:PADK],
                        in_=dram["edge_g"][512 + r0:512 + r0 + 128,
                                           halo_base:halo_base + W])
                t = consts.tile([128, ncols], f16, tag=f"{key}{k}")
                nc.vector.tensor_copy(out=t[:], in_=t8[:])
                if halo_base is not None:
                    # zero the sequence-boundary halo: left keeps only if
                    # this core is odd (flag col0 = p), right only if even
                    nc.vector.tensor_scalar_mul(
                        out=t[:, 0:W], in0=t[:, 0:W], scalar1=fl_sb[:, 0:1])
                    nc.vector.tensor_scalar_mul(
                        out=t[:, W + SH:PADK], in0=t[:, W + SH:PADK],
                        scalar1=fl_sb[:, 1:2])
                if sc0 is not None:
                    # fold Wv's per-d_in 12-bit scale into the upcast
                    nc.vector.tensor_scalar_mul(
                        out=t[:], in0=t[:],
                        scalar1=sc_sb[:, sc0 + k:sc0 + k + 1])
            else:
                ncols = SH if halo_base is None and key == "xqt" else PADK
                t = consts.tile([128, ncols], f16, tag=f"{key}{k}")
                nc.sync.dma_start(out=t[:], in_=dram[key][128 * k:128 * (k + 1), :])
            tiles.append(t)
        return tiles

    xqt_sb = load_xt("xqt", OFF_XQ)
    xkt_sb = load_xt("xkt", OFF_XK, halo_base=0)
    xvt_sb = load_xt("xvt", OFF_XV, sc0=8 if (x_int8 and W12) else None,
                     halo_base=32)

    vones_sb = consts.tile([128, NQT], f32, tag="vones")
    if x_int8:
        # vones int8 [NQT, 128] in the blob; partition-first AP transposes
        v8 = consts.tile([128, NQT], i8, tag="vones8")
        nc.sync.dma_start(out=v8[:], in_=blob_ap(OFF_V, [[1, 128], [128, NQT]]))
        nc.vector.tensor_copy(out=vones_sb[:], in_=v8[:])
    else:
        nc.sync.dma_start(
            out=vones_sb[:], in_=dram["vones"].ap().rearrange("t p -> p t"))

    band_sb = consts.tile([128, QT], f16, tag="band")
    nc.sync.dma_start(out=band_sb[:], in_=dram["band"][:])
    ident_sb = consts.tile([QT, QT], f16, tag="ident")
    nc.sync.dma_start(out=ident_sb[:], in_=dram["ident"][:])

    bq_sb = bk_sb = bv_sb = bo_sb = None
    if has_b:
        bq_sb = consts.tile([128, 4], f32, tag="bq")
        nc.sync.dma_start(out=bq_sb[:], in_=dram["bqc"].ap().rearrange("c p -> p c"))
        bk_sb = consts.tile([128, 4], f32, tag="bk")
        nc.sync.dma_start(out=bk_sb[:], in_=dram["bkc"].ap().rearrange("c p -> p c"))
        bv_sb = consts.tile([128, D], f32, tag="bv")
        nc.sync.dma_start(out=bv_sb[:], in_=dram["bvb"][:])
        bo_sb = consts.tile([128, D], f32, tag="bo")
        nc.sync.dma_start(out=bo_sb[:], in_=dram["bob"][:])

    # ---- Q/K projections -> per-head QT [64, SH], KT [64, PADK] (f16) ----
    # Per-head tiles keep every matmul operand at partition offset 0: the HW
    # crashes on (partition-offset operand + intra-bank psum write offset).
    qt_sb, kt_sb = [], []
    for h in range(H):
        qt_sb.append(consts.tile([64, SH], f16, tag=f"qt{h}", name=f"qt{h}"))
        kt_sb.append(consts.tile([64, PADK], f16, tag=f"kt{h}", name=f"kt{h}"))

    def project_T(xt_sb, w, out_tiles, bias_sb, ncols, sc0):
        # head 2m / 2m+1 live in rows 0:64 / 64:128 of dout-chunk m
        for m in range(4):
            c0 = 0
            while c0 < ncols:
                cw = min(512, ncols - c0)
                ps = psA.tile([128, 512], f32, tag="big")
                for k in range(4):
                    nc.tensor.matmul(
                        ps[:, :cw],
                        lhsT=w[k][:, 128 * m:128 * (m + 1)],
                        rhs=xt_sb[k][:, c0:c0 + cw],
                        start=(k == 0),
                        stop=(k == 3),
                    )
                for half in range(2):
                    r0, r1 = 64 * half, 64 * half + 64
                    dst = out_tiles[2 * m + half][:, c0:c0 + cw]
                    src = ps[r0:r1, :cw]
                    if sc0 is not None:
                        # 12-bit weights: scale rows by per-d_out scale
                        # (optionally fused with bias add)
                        if has_b:
                            nc.vector.tensor_scalar(
                                out=dst, in0=src,
                                scalar1=sc_sb[r0:r1, sc0 + m:sc0 + m + 1],
                                scalar2=bias_sb[r0:r1, m:m + 1],
                                op0=mybir.AluOpType.mult,
                                op1=mybir.AluOpType.add,
                            )
                        else:
                            nc.vector.tensor_scalar_mul(
                                out=dst, in0=src,
                                scalar1=sc_sb[r0:r1, sc0 + m:sc0 + m + 1],
                            )
                    elif has_b:
                        nc.vector.tensor_scalar_add(
                            out=dst, in0=src,
                            scalar1=bias_sb[r0:r1, m:m + 1],
                        )
                    else:
                        nc.vector.tensor_copy(out=dst, in_=src)
                c0 += cw

    wsc0 = 0 if (x_int8 and W12) else None
    project_T(xqt_sb, w_sb["wq"], qt_sb, bq_sb, SH, wsc0)
    project_T(xkt_sb, w_sb["wk"], kt_sb, bk_sb, PADK,
              4 if (x_int8 and W12) else None)

    # ---- V projection, window-major; col 64 = vones (validity) -----------
    v_sb = []
    for t in range(NQT):
        w0 = QT * t
        wr = min(WIN, PADK - w0)
        vt = consts.tile([128, H, DK + 1], f16, tag=f"v{t}")
        v_sb.append(vt)
        ps = psA.tile([128, 512], f32, tag="big")
        for k in range(4):
            nc.tensor.matmul(
                ps[:wr, :],
                lhsT=xvt_sb[k][:, w0:w0 + wr],
                rhs=w_sb["wv"][k][:],
                start=(k == 0),
                stop=(k == 3),
            )
        src = ps[:wr, :].rearrange("p (h x) -> p h x", h=H)
        if has_b:
            bvv = bv_sb[:wr, :].rearrange("p (h x) -> p h x", h=H)
            nc.vector.tensor_add(out=vt[:wr, :, 0:DK], in0=src, in1=bvv)
            # zero out padded-key rows so bias doesn't leak into the band sum
            nc.vector.tensor_scalar_mul(
                out=vt[:wr, :, 0:DK],
                in0=vt[:wr, :, 0:DK],
                scalar1=vones_sb[:wr, t:t + 1],
            )
        else:
            nc.vector.tensor_copy(out=vt[:wr, :, 0:DK], in_=src)
        vb = vones_sb[:wr, t:t + 1]
        vb_bc = bass.AP(
            tensor=vb.tensor, offset=vb.offset,
            ap=[vb.ap[0], [0, H], vb.ap[1]],
        )
        nc.vector.tensor_copy(out=vt[:wr, :, DK:DK + 1], in_=vb_bc)

    # ---- attention -------------------------------------------------------
    ctxT_sb = []
    for c in range(4):
        ctxT_sb.append(consts.tile([128, SH], f16, tag=f"ctxT{c}", name=f"ctxT{c}"))

    head_groups = ((0, 5), (5, 8))
    for t in range(NQT):
        q0 = QT * t
        qw = min(QT, SH - q0)
        w0 = QT * t
        wr = min(WIN, PADK - w0)

        attn_sb = work.tile([128, H, QT], f16, tag="attn")
        for h0, h1 in head_groups:
            nh = h1 - h0
            ps_sc = psB.tile([128, 5, QT], f32, tag="sc")
            for j, h in enumerate(range(h0, h1)):
                nc.tensor.matmul(
                    ps_sc[:wr, j, :qw],
                    lhsT=kt_sb[h][:, w0:w0 + wr],
                    rhs=qt_sb[h][:, q0:q0 + qw],
                    start=True,
                    stop=True,
                )
            nc.scalar.activation(
                out=attn_sb[:wr, h0:h1, :qw],
                in_=ps_sc[:wr, :nh, :qw],
                func=mybir.ActivationFunctionType.Exp,
            )

        # multiplicative band mask, broadcast over heads (gpsimd)
        mbase = band_sb[:wr, :qw]
        mask_bc = bass.AP(
            tensor=mbase.tensor, offset=mbase.offset,
            ap=[mbase.ap[0], [0, H], mbase.ap[1]],
        )
        nc.gpsimd.tensor_mul(
            out=attn_sb[:wr, :, :qw], in0=attn_sb[:wr, :, :qw], in1=mask_bc
        )

        recip_sb = work.tile([QT, H], f32, tag="recip")
        ctx_sb = work.tile([QT, H, DK], f16, tag="ctx")
        for g in range(2):
            ps_ctx = psC.tile([QT, 4, DK + 1], f32, tag="ctx")
            for j, h in enumerate(range(4 * g, 4 * g + 4)):
                nc.tensor.matmul(
                    ps_ctx[:qw, j, :],
                    lhsT=attn_sb[:wr, h, :qw],
                    rhs=v_sb[t][:wr, h, :],
                    start=True,
                    stop=True,
                )
            nc.vector.reciprocal(
                out=recip_sb[:qw, 4 * g:4 * g + 4],
                in_=ps_ctx[:qw, :, DK:DK + 1],
            )
            rbase = recip_sb[:qw, 4 * g:4 * g + 4]
            recip_bc = bass.AP(
                tensor=rbase.tensor, offset=rbase.offset,
                ap=[rbase.ap[0], rbase.ap[1], [0, DK]],
            )
            nc.vector.tensor_mul(
                out=ctx_sb[:qw, 4 * g:4 * g + 4, :],
                in0=ps_ctx[:qw, :, 0:DK],
                in1=recip_bc,
            )

        # transpose ctx [qw, 512] -> ctxT [512, qw]  (4 chunks of 128)
        for c in range(4):
            ps_t = psA.tile([128, QT], f16, tag="big")
            nc.tensor.transpose(
                out=ps_t[:, :qw],
                in_=ctx_sb[:qw, 2 * c:2 * c + 2, :],
                identity=ident_sb[:qw, :qw],
            )
            if x_int8 and W12:
                # fold Wo's per-d_in 12-bit scale into the ctxT copy
                nc.vector.tensor_scalar_mul(
                    out=ctxT_sb[c][:, q0:q0 + qw], in0=ps_t[:, :qw],
                    scalar1=sc_sb[:, 12 + c:12 + c + 1])
            else:
                nc.vector.tensor_copy(
                    out=ctxT_sb[c][:, q0:q0 + qw], in_=ps_t[:, :qw])

    # ---- O-projection ----------------------------------------------------
    for mt in range(8):
        r0 = 128 * mt
        ps = psA.tile([128, 512], f32, tag="big")
        for k in range(4):
            nc.tensor.matmul(
                ps[:],
                lhsT=ctxT_sb[k][:, r0:r0 + 128],
                rhs=w_sb["wo"][k][:],
                start=(k == 0),
                stop=(k == 3),
            )
        src = ps[:]
        if has_b:
            of_sb = work.tile([128, D], f32, tag="osbf")
            nc.vector.tensor_add(out=of_sb[:], in0=ps[:], in1=bo_sb[:])
            src = of_sb[:]
        if out_int8:
            # per-row int8 quantization; scale = absmax/127 rides in the last
            # 4 bytes of each int8 output row (bitcast f32)
            amax_sb = work.tile([128, 1], f32, tag="amax")
            osc_sb = work.tile([128, 1], f32, tag="osc")
            rsc_sb = work.tile([128, 1], f32, tag="rsc")
            o_sb = work.tile([128, D], dt.int8, tag="osb8")
            nc.vector.tensor_reduce(
                out=amax_sb[:], in_=src,
                axis=mybir.AxisListType.X, op=mybir.AluOpType.max,
                apply_absolute_value=True,
            )
            nc.vector.tensor_scalar_max(out=amax_sb[:], in0=amax_sb[:], scalar1=1e-30)
            nc.vector.tensor_scalar_mul(out=osc_sb[:], in0=amax_sb[:], scalar1=1.0 / 127.0)
            nc.vector.reciprocal(out=rsc_sb[:], in_=osc_sb[:])
            nc.vector.tensor_scalar_mul(out=o_sb[:], in0=src, scalar1=rsc_sb[:, 0:1])
            nc.sync.dma_start(out=out_d[r0:r0 + 128, 0:D], in_=o_sb[:])
            nc.sync.dma_start(
                out=out_d[r0:r0 + 128, D:D + 4].bitcast(f32), in_=osc_sb[:]
            )
        else:
            o_sb = work.tile([128, D], f16, tag="osb")
            nc.vector.tensor_copy(out=o_sb[:], in_=src)
            nc.sync.dma_start(out=out_d[r0:r0 + 128, :], in_=o_sb[:])


def _build_band() -> np.ndarray:
    i = np.arange(128)[:, None]   # window row (key)
    j = np.arange(QT)[None, :]    # q column
    band = (i - j >= 0) & (i - j <= 2 * W)
    return band.astype(F16)


def _build_program(x_int8: bool, out_int8: bool, has_b: bool):
    dt = mybir.dt
    f16, f32 = dt.float16, dt.float32
    xdt = dt.int8 if x_int8 else f16
    odt = dt.int8 if out_int8 else f16

    nc = bacc.Bacc("TRN2", target_bir_lowering=False, debug=False, num_devices=NCORES)

    dram = {}
    if x_int8:
        dram["blob"] = nc.dram_tensor("blob", [BLOB], dt.int8, kind="ExternalInput")
    else:
        dram["xqt"] = nc.dram_tensor("xqt", [D, SH], xdt, kind="ExternalInput")
        dram["xkt"] = nc.dram_tensor("xkt", [D, PADK], xdt, kind="ExternalInput")
        dram["xvt"] = nc.dram_tensor("xvt", [D, PADK], xdt, kind="ExternalInput")
        dram["wchunk"] = nc.dram_tensor("wchunk", [WSH, D], f16, kind="ExternalInput")
        dram["vones"] = nc.dram_tensor("vones", [NQT, 128], f32, kind="ExternalInput")
    dram.update({
        "out": nc.dram_tensor(
            "out", [SH, D + 4] if out_int8 else [SH, D], odt, kind="ExternalOutput"),
        "band": nc.inline_tensor(_build_band(), name="band"),
        "ident": nc.inline_tensor(np.eye(QT, dtype=F16), name="ident"),
    })
    if x_int8 and W12:
        dram["wch_b"] = nc.dram_tensor("wch_b", [WSH, WPACK], dt.int8)
        dram["wfull"] = nc.dram_tensor("wfull", [WROWS, WPACK], dt.int8)
    else:
        dram["wch_b"] = nc.dram_tensor("wch_b", [WSH, D], f16)
        dram["wfull"] = nc.dram_tensor("wfull", [WROWS, D], f16)
    if x_int8:
        dram["edge_b"] = nc.dram_tensor("edge_b", [512, 64], dt.int8)
        dram["edge_g"] = nc.dram_tensor("edge_g", [1024, 64], dt.int8)
    if has_b:
        dram["bqc"] = nc.dram_tensor("bqc", [4, 128], f32, kind="ExternalInput")
        dram["bkc"] = nc.dram_tensor("bkc", [4, 128], f32, kind="ExternalInput")
        dram["bvb"] = nc.dram_tensor("bvb", [128, D], f32, kind="ExternalInput")
        dram["bob"] = nc.dram_tensor("bob", [128, D], f32, kind="ExternalInput")

    with tile.TileContext(nc) as tc:
        with (
            tc.tile_pool(name="consts", bufs=1) as consts,
            tc.tile_pool(name="work", bufs=3) as work,
            tc.tile_pool(name="psA", bufs=2, space="PSUM") as psA,
            tc.tile_pool(name="psB", bufs=2, space="PSUM") as psB,
            tc.tile_pool(name="psC", bufs=4, space="PSUM") as psC,
        ):
            _emit(nc, tc, (consts, work, psA, psB, psC), dram, x_int8, out_int8, has_b)

    nc.compile()
    return nc


def _get_program(x_int8, out_int8, has_b):
    key = (x_int8, out_int8, has_b)
    if key not in _programs:
        _programs[key] = _build_program(x_int8, out_int8, has_b)
    return _programs[key]


# ---------------------------------------------------------------------------
# Custom PJRT runner. Same _bass_exec_p path as bass2jax.run_bass_via_pjrt,
# with two wall-time fixes (the measured metric is transfer-bound over the
# axon tunnel, which serializes all RPC + data bytes in both directions):
#   - the donated output buffers are created ON DEVICE by a separate tiny jit
#     (jnp.zeros + out_shardings) instead of uploading host np.zeros — saves
#     the 4.2MB zero upload (~100ms). They can't be created inside the same
#     jit: neuronx_cc_hook requires every bass_exec operand to be a plain HLO
#     parameter.
#   - everything is issued async (zeros jit, sharded device_put, NEFF call)
#     and only the final np.asarray blocks, so per-RPC round-trip latencies
#     (~60-80ms each when blocked individually) overlap with the data stream.
# ---------------------------------------------------------------------------

_runners = {}


def _make_runner(nc):
    import jax
    import jax.numpy as jnp
    from jax.sharding import Mesh, NamedSharding, PartitionSpec
    import warnings
    with warnings.catch_warnings():
        warnings.simplefilter("ignore")
        from jax.experimental.shard_map import shard_map

    bass2jax.install_neuronx_cc_hook()
    partition_name = nc.partition_id_tensor.name if nc.partition_id_tensor else None
    in_names, out_names, out_avals = [], [], []
    for alloc in nc.m.functions[0].allocations:
        if not isinstance(alloc, mybir.MemoryLocationSet):
            continue
        name = alloc.memorylocations[0].name
        if alloc.kind == "ExternalInput":
            if name != partition_name:
                in_names.append(name)
        elif alloc.kind == "ExternalOutput":
            out_names.append(name)
            out_avals.append(
                jax.core.ShapedArray(
                    tuple(alloc.tensor_shape), mybir.dt.np(alloc.dtype)))
    n_params = len(in_names)
    n_outs = len(out_avals)
    in_names_all = in_names + out_names
    if partition_name is not None:
        in_names_all.append(partition_name)

    def _body(*args):
        operands = list(args)
        if partition_name is not None:
            operands.append(bass2jax.partition_id_tensor())
        outs = bass2jax._bass_exec_p.bind(
            *operands,
            out_avals=tuple(out_avals),
            in_names=tuple(in_names_all),
            out_names=tuple(out_names),
            lowering_input_output_aliases=(),
            sim_require_finite=True,
            sim_require_nnan=True,
            nc=nc,
        )
        return tuple(outs)

    devices = jax.devices()[:NCORES]
    mesh = Mesh(np.asarray(devices), ("core",))
    spec = NamedSharding(mesh, PartitionSpec("core"))
    in_specs = (PartitionSpec("core"),) * (n_params + n_outs)
    out_specs = (PartitionSpec("core"),) * n_outs
    # No donation: our kernel writes every output element, so the NEFF's
    # output operands never need meaningful content. A single device-side
    # zeros tuple is created once and passed (never consumed) every call —
    # zero per-call cost on the terminal's serial RPC queue.
    sharded = jax.jit(
        shard_map(_body, mesh=mesh, in_specs=in_specs, out_specs=out_specs,
                  check_rep=False),
        keep_unused=True)
    gshapes = [(NCORES * a.shape[0], *a.shape[1:]) for a in out_avals]
    zeros_fn = jax.jit(
        lambda: tuple(jnp.zeros(s, a.dtype) for s, a in zip(gshapes, out_avals)),
        out_shardings=(spec,) * n_outs)
    zeros_persist = zeros_fn()
    jax.block_until_ready(zeros_persist)

    def run(concat_in):
        """concat_in: list of global [NCORES*rows, ...] arrays in in_names
        order. Returns list of global output arrays (np, gathered)."""
        g_in = [jax.device_put(a, spec) for a in concat_in]   # async upload
        outs = sharded(*g_in, *zeros_persist)            # async NEFF exec
        return [np.asarray(o) for o in outs]             # blocks

    run.in_names = in_names
    return run


def _get_runner(nc):
    if id(nc) not in _runners:
        _runners[id(nc)] = _make_runner(nc)
    return _runners[id(nc)]


def _build_vones(half: int) -> np.ndarray:
    # vones[t, i] = 1.0 iff padded K/V row (96t + i) holds a real key
    v = np.zeros((NQT, 128), np.float32)
    r = QT * np.arange(NQT)[:, None] + np.arange(128)[None, :]
    lo, hi = (W, PADK) if half == 0 else (0, PADK - W)
    v[:] = ((r >= lo) & (r < hi)).astype(np.float32)
    return v


_vones_cache = {}


def kernel(query, key, value, Wq, bq, Wk, bk, Wv, bv, Wo, bo):
    query = np.asarray(query, np.float32)
    key = np.asarray(key, np.float32)
    value = np.asarray(value, np.float32)
    Wq = np.asarray(Wq, np.float32)
    Wk = np.asarray(Wk, np.float32)
    Wv = np.asarray(Wv, np.float32)
    Wo = np.asarray(Wo, np.float32)
    bq = np.asarray(bq, np.float32)
    bk = np.asarray(bk, np.float32)
    bv = np.asarray(bv, np.float32)
    bo = np.asarray(bo, np.float32)

    has_b = bool(np.any(bq) or np.any(bk) or np.any(bv) or np.any(bo))
    x_int8 = X_INT8
    out_int8 = OUT_INT8
    nc = _get_program(x_int8, out_int8, has_b)

    if x_int8:
        # per-column int8 scales, folded into the weight rows on the host

        def colmax(x):
            return np.maximum(np.abs(x).max(axis=(0, 1)) / 127.0, 1e-30)

        def quant(x, s):
            # s = absmax/127 bounds |x/s| <= 127 (+1 ulp, absorbed by rint),
            # so no clip pass is needed
            t = x * (1.0 / s).astype(np.float32)
            np.rint(t, out=t)
            return t.astype(np.int8)

        sq, sk, sv = _host_pool.map(colmax, (query, key, value))
        qx, kx, vx = _host_pool.map(
            lambda a: quant(*a), ((query, sq), (key, sk), (value, sv)))
        wq_f = Wq * (sq[:, None] * SCALE)
        wk_f = Wk * sk[:, None]
        wv_f = Wv * sv[:, None]
        xdt = np.int8
    else:
        wq_f = Wq * SCALE
        wk_f = Wk
        wv_f = Wv
        qx, kx, vx = query.astype(F16), key.astype(F16), value.astype(F16)
        xdt = F16

    if x_int8 and W12:
        # 12-bit weight quantization: Wq/Wk per-column (d_out, scale applied
        # on the projection psum rows), Wv/Wo per-row (d_in, scale folded
        # into the xvT upcast / ctxT copy respectively)
        def q12(w, axis):
            s = np.maximum(np.abs(w).max(axis=axis), 1e-30) / 2047.0
            q = np.rint(w / (s[None, :] if axis == 0 else s[:, None]))
            return q.astype(np.int32), s.astype(np.float32)
        qq, sq_w = q12(wq_f, 0)
        qk, sk_w = q12(wk_f, 0)
        qv, sv_w = q12(wv_f, 1)
        qo, so_w = q12(Wo, 1)
        u = (np.concatenate([qq, qk, qv, qo], axis=0) + 2048).astype(np.uint16)
        lo = (u & 255).astype(np.uint8)
        hi4 = (u >> 8).astype(np.uint8)
        hi = hi4[:, :D // 2] | (hi4[:, D // 2:] << 4)
        wpacked = np.concatenate([lo, hi], axis=1).view(np.int8)  # [2048, 768]
        # transposed [p, chunk] layout so the device DMA reads contiguous
        # 64-byte runs per partition: wscales_t[p*16 + c] = s[128c + p]
        wscales = np.ascontiguousarray(
            np.concatenate([sq_w, sk_w, sv_w, so_w]).reshape(16, 128).T
        ).reshape(-1).view(np.int8)
        wstack = None
    else:
        wstack = np.ascontiguousarray(
            np.concatenate([wq_f, wk_f, wv_f, Wo], axis=0).astype(F16))

    if not _vones_cache:
        _vones_cache[0] = _build_vones(0)
        _vones_cache[1] = _build_vones(1)

    globals_by_name = {}
    if x_int8:
        gblob = np.empty((NCORES, BLOB), np.int8)
        globals_by_name["blob"] = gblob
    else:
        globals_by_name["xqt"] = np.empty((NCORES * D, SH), F16)
        globals_by_name["xkt"] = np.empty((NCORES * D, PADK), F16)
        globals_by_name["xvt"] = np.empty((NCORES * D, PADK), F16)
        globals_by_name["wchunk"] = np.empty((NCORES * WSH, D), F16)
        globals_by_name["vones"] = np.empty((NCORES * NQT, 128), np.float32)
    def pack_core(core):
        b, half = core // 2, core % 2
        s0 = half * SH

        if x_int8:
            # own rows only; halos are exchanged between core pairs on device
            blob = gblob[core]
            blob[OFF_XQ:OFF_XK] = qx[b, s0:s0 + SH].T.reshape(-1)
            blob[OFF_XK:OFF_XV] = kx[b, s0:s0 + SH].T.reshape(-1)
            blob[OFF_XV:OFF_W] = vx[b, s0:s0 + SH].T.reshape(-1)
            if W12:
                blob[OFF_W:OFF_SC] = wpacked[WSH * core:WSH * (core + 1)].reshape(-1)
                blob[OFF_SC:OFF_V] = wscales
            else:
                blob[OFF_W:OFF_V] = wstack[WSH * core:WSH * (core + 1)].view(np.int8).reshape(-1)
            blob[OFF_V:OFF_FL] = _vones_cache[half].astype(np.int8).reshape(-1)
            fl = np.empty((128, 2), np.float32)
            fl[:, 0] = half          # keep left halo only on odd (2nd) half
            fl[:, 1] = 1 - half      # keep right halo only on even half
            blob[OFF_FL:BLOB] = fl.view(np.int8).reshape(-1)
        else:
            xq = qx[b, s0:s0 + SH]
            lo, hi = s0 - W, s0 + SH + W
            clo, chi = max(lo, 0), min(hi, S)
            xk = np.zeros((PADK, D), xdt)
            xv = np.zeros((PADK, D), xdt)
            xk[clo - lo:chi - lo] = kx[b, clo:chi]
            xv[clo - lo:chi - lo] = vx[b, clo:chi]
            wchunk = wstack[WSH * core:WSH * (core + 1)]
            globals_by_name["xqt"][core * D:(core + 1) * D] = xq.T
            globals_by_name["xkt"][core * D:(core + 1) * D] = xk.T
            globals_by_name["xvt"][core * D:(core + 1) * D] = xv.T
            globals_by_name["wchunk"][core * WSH:(core + 1) * WSH] = wchunk
            globals_by_name["vones"][core * NQT:(core + 1) * NQT] = _vones_cache[half]

    list(_host_pool.map(pack_core, range(NCORES)))
    if has_b:
        def rep(name, arr):
            g = np.empty((NCORES * arr.shape[0], *arr.shape[1:]), arr.dtype)
            g[:] = np.tile(arr, (NCORES,) + (1,) * (arr.ndim - 1))
            globals_by_name[name] = g
        rep("bqc", np.ascontiguousarray((bq * SCALE).reshape(4, 128)))
        rep("bkc", np.ascontiguousarray(bk.reshape(4, 128)))
        rep("bvb", np.broadcast_to(bv, (128, D)).astype(np.float32))
        rep("bob", np.broadcast_to(bo, (128, D)).astype(np.float32))
    if x_int8:
        globals_by_name["blob"] = gblob.reshape(-1)

    import time as _time
    run = _get_runner(nc)
    concat_in = [globals_by_name[nm] for nm in run.in_names]
    outs = run(concat_in)
    if TRACE:
        best = None
        for _ in range(5):
            t0 = _time.perf_counter()
            outs = run(concat_in)
            dtns = (_time.perf_counter() - t0) * 1e9
            best = dtns if best is None else min(best, dtns)
        LAST["wall_ns"] = best
    LAST["exec_time_ns"] = None

    ow = D + 4 if out_int8 else D
    oglob = outs[0].reshape(NCORES, SH, ow)
    out = np.empty((B, S, D), np.float32)

    def unpack_core(core):
        b, half = core // 2, core % 2
        o = oglob[core]
        dst = out[b, half * SH:(half + 1) * SH]
        if out_int8:
            scale = np.ascontiguousarray(o[:, D:D + 4]).view(np.float32)
            np.multiply(o[:, 0:D], scale, out=dst, dtype=np.float32)
        else:
            dst[:] = o

    list(_host_pool.map(unpack_core, range(NCORES)))
    return out


if __name__ == "__main__":
    rng = np.random.default_rng(0)
    sc = 1.0 / np.sqrt(D)
    inputs = {
        "query": rng.standard_normal((B, S, D)).astype(np.float32),
        "key": rng.standard_normal((B, S, D)).astype(np.float32),
        "value": rng.standard_normal((B, S, D)).astype(np.float32),
        "Wq": (rng.standard_normal((D, D)) * sc).astype(np.float32),
        "bq": np.zeros(D, np.float32),
        "Wk": (rng.standard_normal((D, D)) * sc).astype(np.float32),
        "bk": np.zeros(D, np.float32),
        "Wv": (rng.standard_normal((D, D)) * sc).astype(np.float32),
        "bv": np.zeros(D, np.float32),
        "Wo": (rng.standard_normal((D, D)) * sc).astype(np.float32),
        "bo": np.zeros(D, np.float32),
    }
    out = kernel(**inputs)
    print("out", out.shape, out.dtype, out[0, 0, :4])



# revision 41
# speedup vs baseline: 1.0989x; 1.0311x over previous
"""Local (banded) attention kernel for Trainium2, 8 NeuronCores SPMD.

Problem: nn_LocalAttention  (B=4, S=2048, D=512, H=8 heads, DK=64, band W=16)
  out = (softmax(band_mask(QK^T/sqrt(DK))) V) Wo + bo   with Q/K/V = x W* + b*

Sharding: 8 cores = 4 batches x 2 sequence halves. Each core computes its
1024-query slice end-to-end. K/V need a 16-row halo; cores upload only
their own 1024 rows and core pairs rebuild the halos on device via a
pair-wise AllGather of edge columns, with per-core 0/1 flags zeroing the
sequence-boundary side (no data-dependent branches needed in SPMD).

The measured metric is the end-to-end wall time of one full execution
(upload + NEFF exec + download) over the axon tunnel, which serializes all
RPC + data bytes in both directions at ~50MB/s peak (~19.6ms/MB floor even
for all-zero payloads, plus a content-entropy-dependent extra of up to
~8ms/MB; transfers do NOT overlap each other or exec, and each *blocked*
RPC costs a ~80ms round trip that async dispatch hides). On-device compute
is a few ms. The kernel therefore minimizes moved bytes and round trips:
  - Custom async PJRT runner (_make_runner): same _bass_exec_p path as
    bass2jax.run_bass_via_pjrt, but (a) the NEFF's output-buffer operands
    are a PERSISTENT device-side zeros tuple created once at runner build
    with no donation (our kernel writes every output element, so their
    content never matters) — saves the 4.2MB host zeros upload (~100ms)
    and all per-call zeros work, and (b) device_put + NEFF call are issued
    async with only the final np.asarray blocking (hides ~3 RPC round
    trips, ~150ms). One device_put of one concatenated global array = one
    streamed transfer (8 separate per-device puts pay ~45ms fixed cost
    EACH and serialize; 2 split puts are ~90ms worse than 1).
  - Q/K/V uploaded int8 with per-column scales folded into the weights on
    the host (X_INT8; per-tensor scales fail the 2e-2 gate, per-column
    passes). X_INT8=False falls back to fp16 (rel ~7e-4).
  - Weights uploaded once as 1/8 shards, 10-bit packed (W12 flag: per row
    512 low bytes + 128 bytes packing the 2-bit highs of cols j, j+128,
    j+256, j+384, plus per-column scales), AllGather'd and unpacked on
    device with int16 DVE bit ops — each 2-bit field extracts in ONE fused
    op ((h & (3<<2j)) << (8-2j)); scales are applied per-partition at the
    projection psum-copy (Wq/Wk, per-d_out), the xvT upcast (Wv, per-d_in)
    and the ctxT copy (Wo, per-d_in). 1.31MB vs 2.10MB f16; numpy-
    simulated error ladder: 12b 1.76e-2, 10b 1.79e-2, 9b 1.89e-2 (sim
    matches hardware to 3 decimals), so 10-bit keeps >10% gate margin.
  - Band mask is an inline NEFF constant; sequence-edge validity is a tiny
    per-core [NQT,128] "vones" vector that becomes the fused-denominator
    column of V (replaces the 264KB/core mask upload).
  - Output is int8 with a per-row f32 scale packed into the last 4 bytes of
    each row (OUT_INT8; halves the download vs fp16).
  - xk/xv upload only their own [512, 1024] slab (no 16-col zero pad, no
    duplicated neighbor halo): a pair-wise AllGather of [512, 64] edge
    columns rebuilds the halos on device (saves 254KB of upload).
  - All inputs ride in ONE int8 blob per core; host packing (quantize,
    transpose, blob assembly) is threaded (ThreadPoolExecutor) since it is
    wall-time before the timed region.
  - jax persistent compilation cache turns recompiles into disk hits.
A null-kernel interleaved A/B (same I/O, trivial compute) runs only ~5ms
faster than the full program: on-device exec is ~5ms, so tiling/overlap
inside the Bass program has nothing left to give; the wire is everything.
Upload (13.98MB) is at the int8-x/10-bit-w representation floor.
Measured: 2.655s original -> 0.560s staged baseline -> ~0.397-0.44s
(min of 5 runs x 3 calls; ambient tunnel noise has multi-minute +-30ms
phases), rel err 1.792e-2 (gate 2e-2), all 4 batches 1.788-1.798e-2.

Per-core device pipeline (fp16 operands, f32 psum):
  - int8 x tiles upcast to fp16 on DVE (values <=127 are exact in fp16).
  - QT = Wq^T @ XqT -> [64,1024] per head; KT likewise [64,1056].
  - V window-major [kpos, 8, 65]; col 64 = vones (validity) -> fused softmax
    denominator that automatically excludes padded keys.
  - Per q-tile (96 queries, 128-key window) and head:
      scoresT = KT_win^T.QT_tile (psum f32); attnT = exp(scoresT) (ACT, f16)
      attnT *= band (gpsimd, inline 0/1 const, broadcast over heads)
      ctx_aug = attnT^T.V_aug (PE); ctx = ctx_aug[:,:64]/den (DVE reciprocal)
      ctxT via PE-transpose -> [512,1024]
  - out = ctxT^T.Wo (+bo) -> [1024,512], per-row absmax/127 int8 quantize,
    scale bitcast into out[:, 512:516] -> DRAM.
"""

import os
import sys

for _p in ("/opt/trn_rl_repo", "/root/.axon_site/_ro/trn_rl_repo"):
    if os.path.isdir(_p) and _p not in sys.path:
        sys.path.insert(0, _p)
        break

import numpy as np
import ml_dtypes

# Persist compiled PJRT executables across calls: run_bass_kernel_spmd builds a
# fresh jit closure per call, so without this every call re-lowers/recompiles
# the identical program (~0.2s) before transferring anything.
try:
    import tempfile

    import jax

    _cache_dir = os.path.join(
        tempfile.gettempdir(), f"jax_comp_cache_{os.getuid()}")
    jax.config.update("jax_compilation_cache_dir", _cache_dir)
    jax.config.update("jax_persistent_cache_min_entry_size_bytes", -1)
    jax.config.update("jax_persistent_cache_min_compile_time_secs", 0.0)
except Exception:
    pass

import concourse.bass as bass
import concourse.tile as tile
from concourse import bacc, bass2jax, mybir

BF16 = ml_dtypes.bfloat16
F16 = np.float16

B, S, D, H, W = 4, 2048, 512, 8, 16
DK = D // H          # 64
NCORES = 8
SH = S // 2          # 1024 rows per core
PADK = SH + 2 * W    # 1056 padded key rows
QT = 96              # q-tile size
NQT = (SH + QT - 1) // QT   # 11 tiles (last = 64)
WIN = QT + 2 * W     # 128-key window per q-tile
SCALE = 1.0 / np.sqrt(DK)
WROWS = 4 * D        # 2048 stacked weight rows
WSH = WROWS // NCORES  # 256 rows per core shard

X_INT8 = True        # upload Q/K/V as int8 (per-column scales folded into W)
OUT_INT8 = True      # download output as int8 + per-row f32 scales
W12 = True           # pack weights as 12-bit planes (vs f16) in the blob

# single-blob input layout (int8-x mode): one ExternalInput array per core.
# Weights ride as 12-bit packed planes: per row, 512 low bytes + 256 shared
# hi-nibble bytes (cols j and j+256 share one byte), plus 2048 f32 scales
# (per-d_out for Wq/Wk, per-d_in for Wv/Wo) duplicated in every core's blob.
SXQ = D * SH          # 524288   xqT int8 [512, 1024]
WPACK = D + D // 4    # 640 packed bytes per weight row (10-bit)
OFF_XQ = 0
OFF_XK = OFF_XQ + SXQ
OFF_XV = OFF_XK + SXQ           # xk/xv ship [512, 1024] own rows only;
OFF_W = OFF_XV + SXQ            # 16-col halos are exchanged on device
if W12:
    OFF_SC = OFF_W + WSH * WPACK    # weight scales f32 [4*D]
    OFF_V = OFF_SC + 4 * D * 4      # vones int8 [NQT, 128]
else:
    OFF_SC = None
    OFF_V = OFF_W + WSH * D * 2     # f16 weight shard [256, 512]
OFF_FL = OFF_V + NQT * 128      # halo flags f32 [128, 2]: col0=p, col1=1-p
BLOB = OFF_FL + 128 * 2 * 4     # 1780096 bytes (W12)

TRACE = False        # set True (from test.py) to collect an NTFF profile
LAST = {}            # stash for exec_time_ns / profile info

from concurrent.futures import ThreadPoolExecutor

_host_pool = ThreadPoolExecutor(max_workers=8)   # numpy packing parallelism

_programs = {}       # (x_int8, out_int8, has_b) -> compiled nc


def _emit(nc, tc, pools, dram, x_int8, out_int8, has_b):
    dt = mybir.dt
    f16, f32, i8 = dt.float16, dt.float32, dt.int8
    consts, work, psA, psB, psC = pools
    out_d = dram["out"]

    def blob_ap(off, pattern):
        b0 = dram["blob"][0:1]
        return bass.AP(tensor=b0.tensor, offset=off, ap=pattern)

    # ---- weights: bounce -> AllGather -> SBUF ----------------------------
    if x_int8 and W12:
        wch_src = blob_ap(OFF_W, [[WPACK, WSH], [1, WPACK]])
    elif x_int8:
        wch_src = blob_ap(OFF_W, [[D * 2, WSH], [1, D * 2]]).bitcast(f16)
    else:
        wch_src = dram["wchunk"][:, :]
    nc.sync.dma_start(out=dram["wch_b"][:, :], in_=wch_src)
    nc.gpsimd.collective_compute(
        "AllGather",
        mybir.AluOpType.bypass,
        replica_groups=[list(range(NCORES))],
        ins=[dram["wch_b"].ap().opt()],
        outs=[dram["wfull"].ap().opt()],
    )
    sc_sb = None
    w_sb = {}
    if x_int8 and W12:
        # per-chunk weight scales [128, 16]; col 4i+k = scales[512i + 128k + p]
        sc_sb = consts.tile([128, 16], f32, tag="wsc")
        nc.sync.dma_start(
            out=sc_sb[:],
            in_=blob_ap(OFF_SC, [[64, 128], [1, 64]]).bitcast(f32),
        )
        i16 = dt.int16
        for i, name in enumerate(("wq", "wk", "wv", "wo")):
            w_sb[name] = []
            for k in range(4):
                r0 = D * i + 128 * k
                lo8 = work.tile([128, D], i8, tag="wlo8")
                hi8 = work.tile([128, D // 4], i8, tag="whi8")
                nc.sync.dma_start(out=lo8[:], in_=dram["wfull"][r0:r0 + 128, 0:D])
                nc.sync.dma_start(out=hi8[:], in_=dram["wfull"][r0:r0 + 128, D:WPACK])
                lo16 = work.tile([128, D], i16, tag="wlo16")
                hi16 = work.tile([128, D // 4], i16, tag="whi16")
                nc.vector.tensor_copy(out=lo16[:], in_=lo8[:])
                nc.vector.tensor_scalar(
                    out=lo16[:], in0=lo16[:], scalar1=255, scalar2=None,
                    op0=mybir.AluOpType.bitwise_and)
                nc.vector.tensor_copy(out=hi16[:], in_=hi8[:])
                nc.vector.tensor_scalar(
                    out=hi16[:], in0=hi16[:], scalar1=255, scalar2=None,
                    op0=mybir.AluOpType.bitwise_and)
                v16 = work.tile([128, D], i16, tag="wv16")
                qtile = work.tile([128, D // 4], i16, tag="wqt16")
                h = D // 4
                # quarter j holds bits 2j..2j+2 of the hi byte: ((h>>2j)&3)<<8
                # == (h & (3<<2j)) << (8-2j), one fused op per quarter
                for j in range(4):
                    nc.vector.tensor_scalar(
                        out=qtile[:], in0=hi16[:],
                        scalar1=3 << (2 * j), scalar2=8 - 2 * j,
                        op0=mybir.AluOpType.bitwise_and,
                        op1=mybir.AluOpType.logical_shift_left)
                    nc.vector.tensor_add(
                        out=v16[:, h * j:h * (j + 1)],
                        in0=lo16[:, h * j:h * (j + 1)], in1=qtile[:])
                nc.vector.tensor_scalar_add(out=v16[:], in0=v16[:], scalar1=-512)
                t = consts.tile([128, D], f16, tag=f"{name}{k}")
                nc.vector.tensor_copy(out=t[:], in_=v16[:])
                w_sb[name].append(t)
    else:
        for i, name in enumerate(("wq", "wk", "wv", "wo")):
            w_sb[name] = []
            for k in range(4):
                t = consts.tile([128, D], f16, tag=f"{name}{k}")
                r0 = D * i + 128 * k
                nc.sync.dma_start(out=t[:], in_=dram["wfull"][r0:r0 + 128, :])
                w_sb[name].append(t)

    # ---- load x (fp16 direct, or int8-from-blob + DVE upcast) ------------
    # ---- halo exchange: xk/xv ship without their 16-col halos; core pairs
    # (2b, 2b+1) AllGather their edge columns and each core rebuilds its
    # halos, zeroing the sequence-boundary side via per-core 0/1 flags ----
    fl_sb = None
    if x_int8:
        fl_sb = consts.tile([128, 2], f32, tag="hflags")
        nc.sync.dma_start(
            out=fl_sb[:], in_=blob_ap(OFF_FL, [[8, 128], [1, 8]]).bitcast(f32))
        # edge_b [512, 64]: cols 0:16/16:32 = xk left/right edge,
        #                   cols 32:48/48:64 = xv left/right edge
        for off, base in ((OFF_XK, 0), (OFF_XV, 32)):
            nc.sync.dma_start(
                out=dram["edge_b"][:, base:base + W],
                in_=blob_ap(off, [[SH, 512], [1, W]]))
            nc.sync.dma_start(
                out=dram["edge_b"][:, base + W:base + 2 * W],
                in_=blob_ap(off + SH - W, [[SH, 512], [1, W]]))
        nc.gpsimd.collective_compute(
            "AllGather",
            mybir.AluOpType.bypass,
            replica_groups=[[2 * b, 2 * b + 1] for b in range(NCORES // 2)],
            ins=[dram["edge_b"].ap().opt()],
            outs=[dram["edge_g"].ap().opt()],
        )

    def load_xt(key, off, sc0=None, halo_base=None):
        tiles = []
        for k in range(4):
            if x_int8:
                ncols = SH if halo_base is None else PADK
                t8 = consts.tile([128, ncols], i8, tag=f"{key}{k}i8")
                c0 = 0 if halo_base is None else W
                nc.sync.dma_start(
                    out=t8[:, c0:c0 + SH],
                    in_=blob_ap(off + 128 * k * SH, [[SH, 128], [1, SH]]),
                )
                if halo_base is not None:
                    # left halo <- even core's right edge (slot 0);
                    # right halo <- odd core's left edge (slot 1)
                    r0 = 128 * k
                    nc.sync.dma_start(
                        out=t8[:, 0:W],
                        in_=dram["edge_g"][r0:r0 + 128,
                                           halo_base + W:halo_base + 2 * W])
                    nc.sync.dma_start(
                        out=t8[:, W + SH:PADK],
                        in_=dram["edge_g"][512 + r0:512 + r0 + 128,
                                           halo_base:halo_base + W])
                t = consts.tile([128, ncols], f16, tag=f"{key}{k}")
                nc.vector.tensor_copy(out=t[:], in_=t8[:])
                if halo_base is not None:
                    # zero the sequence-boundary halo: left keeps only if
                    # this core is odd (flag col0 = p), right only if even
                    nc.vector.tensor_scalar_mul(
                        out=t[:, 0:W], in0=t[:, 0:W], scalar1=fl_sb[:, 0:1])
                    nc.vector.tensor_scalar_mul(
                        out=t[:, W + SH:PADK], in0=t[:, W + SH:PADK],
                        scalar1=fl_sb[:, 1:2])
                if sc0 is not None:
                    # fold Wv's per-d_in 12-bit scale into the upcast
                    nc.vector.tensor_scalar_mul(
                        out=t[:], in0=t[:],
                        scalar1=sc_sb[:, sc0 + k:sc0 + k + 1])
            else:
                ncols = SH if halo_base is None and key == "xqt" else PADK
                t = consts.tile([128, ncols], f16, tag=f"{key}{k}")
                nc.sync.dma_start(out=t[:], in_=dram[key][128 * k:128 * (k + 1), :])
            tiles.append(t)
        return tiles

    xqt_sb = load_xt("xqt", OFF_XQ)
    xkt_sb = load_xt("xkt", OFF_XK, halo_base=0)
    xvt_sb = load_xt("xvt", OFF_XV, sc0=8 if (x_int8 and W12) else None,
                     halo_base=32)

    vones_sb = consts.tile([128, NQT], f32, tag="vones")
    if x_int8:
        # vones int8 [NQT, 128] in the blob; partition-first AP transposes
        v8 = consts.tile([128, NQT], i8, tag="vones8")
        nc.sync.dma_start(out=v8[:], in_=blob_ap(OFF_V, [[1, 128], [128, NQT]]))
        nc.vector.tensor_copy(out=vones_sb[:], in_=v8[:])
    else:
        nc.sync.dma_start(
            out=vones_sb[:], in_=dram["vones"].ap().rearrange("t p -> p t"))

    band_sb = consts.tile([128, QT], f16, tag="band")
    nc.sync.dma_start(out=band_sb[:], in_=dram["band"][:])
    ident_sb = consts.tile([QT, QT], f16, tag="ident")
    nc.sync.dma_start(out=ident_sb[:], in_=dram["ident"][:])

    bq_sb = bk_sb = bv_sb = bo_sb = None
    if has_b:
        bq_sb = consts.tile([128, 4], f32, tag="bq")
        nc.sync.dma_start(out=bq_sb[:], in_=dram["bqc"].ap().rearrange("c p -> p c"))
        bk_sb = consts.tile([128, 4], f32, tag="bk")
        nc.sync.dma_start(out=bk_sb[:], in_=dram["bkc"].ap().rearrange("c p -> p c"))
        bv_sb = consts.tile([128, D], f32, tag="bv")
        nc.sync.dma_start(out=bv_sb[:], in_=dram["bvb"][:])
        bo_sb = consts.tile([128, D], f32, tag="bo")
        nc.sync.dma_start(out=bo_sb[:], in_=dram["bob"][:])

    # ---- Q/K projections -> per-head QT [64, SH], KT [64, PADK] (f16) ----
    # Per-head tiles keep every matmul operand at partition offset 0: the HW
    # crashes on (partition-offset operand + intra-bank psum write offset).
    qt_sb, kt_sb = [], []
    for h in range(H):
        qt_sb.append(consts.tile([64, SH], f16, tag=f"qt{h}", name=f"qt{h}"))
        kt_sb.append(consts.tile([64, PADK], f16, tag=f"kt{h}", name=f"kt{h}"))

    def project_T(xt_sb, w, out_tiles, bias_sb, ncols, sc0):
        # head 2m / 2m+1 live in rows 0:64 / 64:128 of dout-chunk m
        for m in range(4):
            c0 = 0
            while c0 < ncols:
                cw = min(512, ncols - c0)
                ps = psA.tile([128, 512], f32, tag="big")
                for k in range(4):
                    nc.tensor.matmul(
                        ps[:, :cw],
                        lhsT=w[k][:, 128 * m:128 * (m + 1)],
                        rhs=xt_sb[k][:, c0:c0 + cw],
                        start=(k == 0),
                        stop=(k == 3),
                    )
                for half in range(2):
                    r0, r1 = 64 * half, 64 * half + 64
                    dst = out_tiles[2 * m + half][:, c0:c0 + cw]
                    src = ps[r0:r1, :cw]
                    if sc0 is not None:
                        # 12-bit weights: scale rows by per-d_out scale
                        # (optionally fused with bias add)
                        if has_b:
                            nc.vector.tensor_scalar(
                                out=dst, in0=src,
                                scalar1=sc_sb[r0:r1, sc0 + m:sc0 + m + 1],
                                scalar2=bias_sb[r0:r1, m:m + 1],
                                op0=mybir.AluOpType.mult,
                                op1=mybir.AluOpType.add,
                            )
                        else:
                            nc.vector.tensor_scalar_mul(
                                out=dst, in0=src,
                                scalar1=sc_sb[r0:r1, sc0 + m:sc0 + m + 1],
                            )
                    elif has_b:
                        nc.vector.tensor_scalar_add(
                            out=dst, in0=src,
                            scalar1=bias_sb[r0:r1, m:m + 1],
                        )
                    else:
                        nc.vector.tensor_copy(out=dst, in_=src)
                c0 += cw

    wsc0 = 0 if (x_int8 and W12) else None
    project_T(xqt_sb, w_sb["wq"], qt_sb, bq_sb, SH, wsc0)
    project_T(xkt_sb, w_sb["wk"], kt_sb, bk_sb, PADK,
              4 if (x_int8 and W12) else None)

    # ---- V projection, window-major; col 64 = vones (validity) -----------
    v_sb = []
    for t in range(NQT):
        w0 = QT * t
        wr = min(WIN, PADK - w0)
        vt = consts.tile([128, H, DK + 1], f16, tag=f"v{t}")
        v_sb.append(vt)
        ps = psA.tile([128, 512], f32, tag="big")
        for k in range(4):
            nc.tensor.matmul(
                ps[:wr, :],
                lhsT=xvt_sb[k][:, w0:w0 + wr],
                rhs=w_sb["wv"][k][:],
                start=(k == 0),
                stop=(k == 3),
            )
        src = ps[:wr, :].rearrange("p (h x) -> p h x", h=H)
        if has_b:
            bvv = bv_sb[:wr, :].rearrange("p (h x) -> p h x", h=H)
            nc.vector.tensor_add(out=vt[:wr, :, 0:DK], in0=src, in1=bvv)
            # zero out padded-key rows so bias doesn't leak into the band sum
            nc.vector.tensor_scalar_mul(
                out=vt[:wr, :, 0:DK],
                in0=vt[:wr, :, 0:DK],
                scalar1=vones_sb[:wr, t:t + 1],
            )
        else:
            nc.vector.tensor_copy(out=vt[:wr, :, 0:DK], in_=src)
        vb = vones_sb[:wr, t:t + 1]
        vb_bc = bass.AP(
            tensor=vb.tensor, offset=vb.offset,
            ap=[vb.ap[0], [0, H], vb.ap[1]],
        )
        nc.vector.tensor_copy(out=vt[:wr, :, DK:DK + 1], in_=vb_bc)

    # ---- attention -------------------------------------------------------
    ctxT_sb = []
    for c in range(4):
        ctxT_sb.append(consts.tile([128, SH], f16, tag=f"ctxT{c}", name=f"ctxT{c}"))

    head_groups = ((0, 5), (5, 8))
    for t in range(NQT):
        q0 = QT * t
        qw = min(QT, SH - q0)
        w0 = QT * t
        wr = min(WIN, PADK - w0)

        attn_sb = work.tile([128, H, QT], f16, tag="attn")
        for h0, h1 in head_groups:
            nh = h1 - h0
            ps_sc = psB.tile([128, 5, QT], f32, tag="sc")
            for j, h in enumerate(range(h0, h1)):
                nc.tensor.matmul(
                    ps_sc[:wr, j, :qw],
                    lhsT=kt_sb[h][:, w0:w0 + wr],
                    rhs=qt_sb[h][:, q0:q0 + qw],
                    start=True,
                    stop=True,
                )
            nc.scalar.activation(
                out=attn_sb[:wr, h0:h1, :qw],
                in_=ps_sc[:wr, :nh, :qw],
                func=mybir.ActivationFunctionType.Exp,
            )

        # multiplicative band mask, broadcast over heads (gpsimd)
        mbase = band_sb[:wr, :qw]
        mask_bc = bass.AP(
            tensor=mbase.tensor, offset=mbase.offset,
            ap=[mbase.ap[0], [0, H], mbase.ap[1]],
        )
        nc.gpsimd.tensor_mul(
            out=attn_sb[:wr, :, :qw], in0=attn_sb[:wr, :, :qw], in1=mask_bc
        )

        recip_sb = work.tile([QT, H], f32, tag="recip")
        ctx_sb = work.tile([QT, H, DK], f16, tag="ctx")
        for g in range(2):
            ps_ctx = psC.tile([QT, 4, DK + 1], f32, tag="ctx")
            for j, h in enumerate(range(4 * g, 4 * g + 4)):
                nc.tensor.matmul(
                    ps_ctx[:qw, j, :],
                    lhsT=attn_sb[:wr, h, :qw],
                    rhs=v_sb[t][:wr, h, :],
                    start=True,
                    stop=True,
                )
            nc.vector.reciprocal(
                out=recip_sb[:qw, 4 * g:4 * g + 4],
                in_=ps_ctx[:qw, :, DK:DK + 1],
            )
            rbase = recip_sb[:qw, 4 * g:4 * g + 4]
            recip_bc = bass.AP(
                tensor=rbase.tensor, offset=rbase.offset,
                ap=[rbase.ap[0], rbase.ap[1], [0, DK]],
            )
            nc.vector.tensor_mul(
                out=ctx_sb[:qw, 4 * g:4 * g + 4, :],
                in0=ps_ctx[:qw, :, 0:DK],
                in1=recip_bc,
            )

        # transpose ctx [qw, 512] -> ctxT [512, qw]  (4 chunks of 128)
        for c in range(4):
            ps_t = psA.tile([128, QT], f16, tag="big")
            nc.tensor.transpose(
                out=ps_t[:, :qw],
                in_=ctx_sb[:qw, 2 * c:2 * c + 2, :],
                identity=ident_sb[:qw, :qw],
            )
            if x_int8 and W12:
                # fold Wo's per-d_in 12-bit scale into the ctxT copy
                nc.vector.tensor_scalar_mul(
                    out=ctxT_sb[c][:, q0:q0 + qw], in0=ps_t[:, :qw],
                    scalar1=sc_sb[:, 12 + c:12 + c + 1])
            else:
                nc.vector.tensor_copy(
                    out=ctxT_sb[c][:, q0:q0 + qw], in_=ps_t[:, :qw])

    # ---- O-projection ----------------------------------------------------
    for mt in range(8):
        r0 = 128 * mt
        ps = psA.tile([128, 512], f32, tag="big")
        for k in range(4):
            nc.tensor.matmul(
                ps[:],
                lhsT=ctxT_sb[k][:, r0:r0 + 128],
                rhs=w_sb["wo"][k][:],
                start=(k == 0),
                stop=(k == 3),
            )
        src = ps[:]
        if has_b:
            of_sb = work.tile([128, D], f32, tag="osbf")
            nc.vector.tensor_add(out=of_sb[:], in0=ps[:], in1=bo_sb[:])
            src = of_sb[:]
        if out_int8:
            # per-row int8 quantization; scale = absmax/127 rides in the last
            # 4 bytes of each int8 output row (bitcast f32)
            amax_sb = work.tile([128, 1], f32, tag="amax")
            osc_sb = work.tile([128, 1], f32, tag="osc")
            rsc_sb = work.tile([128, 1], f32, tag="rsc")
            o_sb = work.tile([128, D], dt.int8, tag="osb8")
            nc.vector.tensor_reduce(
                out=amax_sb[:], in_=src,
                axis=mybir.AxisListType.X, op=mybir.AluOpType.max,
                apply_absolute_value=True,
            )
            nc.vector.tensor_scalar_max(out=amax_sb[:], in0=amax_sb[:], scalar1=1e-30)
            nc.vector.tensor_scalar_mul(out=osc_sb[:], in0=amax_sb[:], scalar1=1.0 / 127.0)
            nc.vector.reciprocal(out=rsc_sb[:], in_=osc_sb[:])
            nc.vector.tensor_scalar_mul(out=o_sb[:], in0=src, scalar1=rsc_sb[:, 0:1])
            nc.sync.dma_start(out=out_d[r0:r0 + 128, 0:D], in_=o_sb[:])
            nc.sync.dma_start(
                out=out_d[r0:r0 + 128, D:D + 4].bitcast(f32), in_=osc_sb[:]
            )
        else:
            o_sb = work.tile([128, D], f16, tag="osb")
            nc.vector.tensor_copy(out=o_sb[:], in_=src)
            nc.sync.dma_start(out=out_d[r0:r0 + 128, :], in_=o_sb[:])


def _build_band() -> np.ndarray:
    i = np.arange(128)[:, None]   # window row (key)
    j = np.arange(QT)[None, :]    # q column
    band = (i - j >= 0) & (i - j <= 2 * W)
    return band.astype(F16)


def _build_program(x_int8: bool, out_int8: bool, has_b: bool):
    dt = mybir.dt
    f16, f32 = dt.float16, dt.float32
    xdt = dt.int8 if x_int8 else f16
    odt = dt.int8 if out_int8 else f16

    nc = bacc.Bacc("TRN2", target_bir_lowering=False, debug=False, num_devices=NCORES)

    dram = {}
    if x_int8:
        dram["blob"] = nc.dram_tensor("blob", [BLOB], dt.int8, kind="ExternalInput")
    else:
        dram["xqt"] = nc.dram_tensor("xqt", [D, SH], xdt, kind="ExternalInput")
        dram["xkt"] = nc.dram_tensor("xkt", [D, PADK], xdt, kind="ExternalInput")
        dram["xvt"] = nc.dram_tensor("xvt", [D, PADK], xdt, kind="ExternalInput")
        dram["wchunk"] = nc.dram_tensor("wchunk", [WSH, D], f16, kind="ExternalInput")
        dram["vones"] = nc.dram_tensor("vones", [NQT, 128], f32, kind="ExternalInput")
    dram.update({
        "out": nc.dram_tensor(
            "out", [SH, D + 4] if out_int8 else [SH, D], odt, kind="ExternalOutput"),
        "band": nc.inline_tensor(_build_band(), name="band"),
        "ident": nc.inline_tensor(np.eye(QT, dtype=F16), name="ident"),
    })
    if x_int8 and W12:
        dram["wch_b"] = nc.dram_tensor("wch_b", [WSH, WPACK], dt.int8)
        dram["wfull"] = nc.dram_tensor("wfull", [WROWS, WPACK], dt.int8)
    else:
        dram["wch_b"] = nc.dram_tensor("wch_b", [WSH, D], f16)
        dram["wfull"] = nc.dram_tensor("wfull", [WROWS, D], f16)
    if x_int8:
        dram["edge_b"] = nc.dram_tensor("edge_b", [512, 64], dt.int8)
        dram["edge_g"] = nc.dram_tensor("edge_g", [1024, 64], dt.int8)
    if has_b:
        dram["bqc"] = nc.dram_tensor("bqc", [4, 128], f32, kind="ExternalInput")
        dram["bkc"] = nc.dram_tensor("bkc", [4, 128], f32, kind="ExternalInput")
        dram["bvb"] = nc.dram_tensor("bvb", [128, D], f32, kind="ExternalInput")
        dram["bob"] = nc.dram_tensor("bob", [128, D], f32, kind="ExternalInput")

    with tile.TileContext(nc) as tc:
        with (
            tc.tile_pool(name="consts", bufs=1) as consts,
            tc.tile_pool(name="work", bufs=3) as work,
            tc.tile_pool(name="psA", bufs=2, space="PSUM") as psA,
            tc.tile_pool(name="psB", bufs=2, space="PSUM") as psB,
            tc.tile_pool(name="psC", bufs=4, space="PSUM") as psC,
        ):
            _emit(nc, tc, (consts, work, psA, psB, psC), dram, x_int8, out_int8, has_b)

    nc.compile()
    return nc


def _get_program(x_int8, out_int8, has_b):
    key = (x_int8, out_int8, has_b)
    if key not in _programs:
        _programs[key] = _build_program(x_int8, out_int8, has_b)
    return _programs[key]


# ---------------------------------------------------------------------------
# Custom PJRT runner. Same _bass_exec_p path as bass2jax.run_bass_via_pjrt,
# with two wall-time fixes (the measured metric is transfer-bound over the
# axon tunnel, which serializes all RPC + data bytes in both directions):
#   - the donated output buffers are created ON DEVICE by a separate tiny jit
#     (jnp.zeros + out_shardings) instead of uploading host np.zeros — saves
#     the 4.2MB zero upload (~100ms). They can't be created inside the same
#     jit: neuronx_cc_hook requires every bass_exec operand to be a plain HLO
#     parameter.
#   - everything is issued async (zeros jit, sharded device_put, NEFF call)
#     and only the final np.asarray blocks, so per-RPC round-trip latencies
#     (~60-80ms each when blocked individually) overlap with the data stream.
# ---------------------------------------------------------------------------

_runners = {}


def _make_runner(nc):
    import jax
    import jax.numpy as jnp
    from jax.sharding import Mesh, NamedSharding, PartitionSpec
    import warnings
    with warnings.catch_warnings():
        warnings.simplefilter("ignore")
        from jax.experimental.shard_map import shard_map

    bass2jax.install_neuronx_cc_hook()
    partition_name = nc.partition_id_tensor.name if nc.partition_id_tensor else None
    in_names, out_names, out_avals = [], [], []
    for alloc in nc.m.functions[0].allocations:
        if not isinstance(alloc, mybir.MemoryLocationSet):
            continue
        name = alloc.memorylocations[0].name
        if alloc.kind == "ExternalInput":
            if name != partition_name:
                in_names.append(name)
        elif alloc.kind == "ExternalOutput":
            out_names.append(name)
            out_avals.append(
                jax.core.ShapedArray(
                    tuple(alloc.tensor_shape), mybir.dt.np(alloc.dtype)))
    n_params = len(in_names)
    n_outs = len(out_avals)
    in_names_all = in_names + out_names
    if partition_name is not None:
        in_names_all.append(partition_name)

    def _body(*args):
        operands = list(args)
        if partition_name is not None:
            operands.append(bass2jax.partition_id_tensor())
        outs = bass2jax._bass_exec_p.bind(
            *operands,
            out_avals=tuple(out_avals),
            in_names=tuple(in_names_all),
            out_names=tuple(out_names),
            lowering_input_output_aliases=(),
            sim_require_finite=True,
            sim_require_nnan=True,
            nc=nc,
        )
        return tuple(outs)

    devices = jax.devices()[:NCORES]
    mesh = Mesh(np.asarray(devices), ("core",))
    spec = NamedSharding(mesh, PartitionSpec("core"))
    in_specs = (PartitionSpec("core"),) * (n_params + n_outs)
    out_specs = (PartitionSpec("core"),) * n_outs
    # No donation: our kernel writes every output element, so the NEFF's
    # output operands never need meaningful content. A single device-side
    # zeros tuple is created once and passed (never consumed) every call —
    # zero per-call cost on the terminal's serial RPC queue.
    sharded = jax.jit(
        shard_map(_body, mesh=mesh, in_specs=in_specs, out_specs=out_specs,
                  check_rep=False),
        keep_unused=True)
    gshapes = [(NCORES * a.shape[0], *a.shape[1:]) for a in out_avals]
    zeros_fn = jax.jit(
        lambda: tuple(jnp.zeros(s, a.dtype) for s, a in zip(gshapes, out_avals)),
        out_shardings=(spec,) * n_outs)
    zeros_persist = zeros_fn()
    jax.block_until_ready(zeros_persist)

    def run(concat_in):
        """concat_in: list of global [NCORES*rows, ...] arrays in in_names
        order. Returns list of global output arrays (np, gathered)."""
        g_in = [jax.device_put(a, spec) for a in concat_in]   # async upload
        outs = sharded(*g_in, *zeros_persist)            # async NEFF exec
        return [np.asarray(o) for o in outs]             # blocks

    run.in_names = in_names
    return run


def _get_runner(nc):
    if id(nc) not in _runners:
        _runners[id(nc)] = _make_runner(nc)
    return _runners[id(nc)]


def _build_vones(half: int) -> np.ndarray:
    # vones[t, i] = 1.0 iff padded K/V row (96t + i) holds a real key
    v = np.zeros((NQT, 128), np.float32)
    r = QT * np.arange(NQT)[:, None] + np.arange(128)[None, :]
    lo, hi = (W, PADK) if half == 0 else (0, PADK - W)
    v[:] = ((r >= lo) & (r < hi)).astype(np.float32)
    return v


_vones_cache = {}


def kernel(query, key, value, Wq, bq, Wk, bk, Wv, bv, Wo, bo):
    query = np.asarray(query, np.float32)
    key = np.asarray(key, np.float32)
    value = np.asarray(value, np.float32)
    Wq = np.asarray(Wq, np.float32)
    Wk = np.asarray(Wk, np.float32)
    Wv = np.asarray(Wv, np.float32)
    Wo = np.asarray(Wo, np.float32)
    bq = np.asarray(bq, np.float32)
    bk = np.asarray(bk, np.float32)
    bv = np.asarray(bv, np.float32)
    bo = np.asarray(bo, np.float32)

    has_b = bool(np.any(bq) or np.any(bk) or np.any(bv) or np.any(bo))
    x_int8 = X_INT8
    out_int8 = OUT_INT8
    nc = _get_program(x_int8, out_int8, has_b)

    if x_int8:
        # per-column int8 scales, folded into the weight rows on the host

        def colmax(x):
            return np.maximum(np.abs(x).max(axis=(0, 1)) / 127.0, 1e-30)

        def quant(x, s):
            # s = absmax/127 bounds |x/s| <= 127 (+1 ulp, absorbed by rint),
            # so no clip pass is needed
            t = x * (1.0 / s).astype(np.float32)
            np.rint(t, out=t)
            return t.astype(np.int8)

        sq, sk, sv = _host_pool.map(colmax, (query, key, value))
        qx, kx, vx = _host_pool.map(
            lambda a: quant(*a), ((query, sq), (key, sk), (value, sv)))
        wq_f = Wq * (sq[:, None] * SCALE)
        wk_f = Wk * sk[:, None]
        wv_f = Wv * sv[:, None]
        xdt = np.int8
    else:
        wq_f = Wq * SCALE
        wk_f = Wk
        wv_f = Wv
        qx, kx, vx = query.astype(F16), key.astype(F16), value.astype(F16)
        xdt = F16

    if x_int8 and W12:
        # 12-bit weight quantization: Wq/Wk per-column (d_out, scale applied
        # on the projection psum rows), Wv/Wo per-row (d_in, scale folded
        # into the xvT upcast / ctxT copy respectively)
        def q12(w, axis):
            s = np.maximum(np.abs(w).max(axis=axis), 1e-30) / 511.0
            q = np.rint(w / (s[None, :] if axis == 0 else s[:, None]))
            return q.astype(np.int32), s.astype(np.float32)
        qq, sq_w = q12(wq_f, 0)
        qk, sk_w = q12(wk_f, 0)
        qv, sv_w = q12(wv_f, 1)
        qo, so_w = q12(Wo, 1)
        u = (np.concatenate([qq, qk, qv, qo], axis=0) + 512).astype(np.uint16)
        lo = (u & 255).astype(np.uint8)
        hi2 = (u >> 8).astype(np.uint8)
        h4 = D // 4
        hi = (hi2[:, :h4] | (hi2[:, h4:2 * h4] << 2)
              | (hi2[:, 2 * h4:3 * h4] << 4) | (hi2[:, 3 * h4:] << 6))
        wpacked = np.concatenate([lo, hi], axis=1).view(np.int8)  # [2048, 640]
        # transposed [p, chunk] layout so the device DMA reads contiguous
        # 64-byte runs per partition: wscales_t[p*16 + c] = s[128c + p]
        wscales = np.ascontiguousarray(
            np.concatenate([sq_w, sk_w, sv_w, so_w]).reshape(16, 128).T
        ).reshape(-1).view(np.int8)
        wstack = None
    else:
        wstack = np.ascontiguousarray(
            np.concatenate([wq_f, wk_f, wv_f, Wo], axis=0).astype(F16))

    if not _vones_cache:
        _vones_cache[0] = _build_vones(0)
        _vones_cache[1] = _build_vones(1)

    globals_by_name = {}
    if x_int8:
        gblob = np.empty((NCORES, BLOB), np.int8)
        globals_by_name["blob"] = gblob
    else:
        globals_by_name["xqt"] = np.empty((NCORES * D, SH), F16)
        globals_by_name["xkt"] = np.empty((NCORES * D, PADK), F16)
        globals_by_name["xvt"] = np.empty((NCORES * D, PADK), F16)
        globals_by_name["wchunk"] = np.empty((NCORES * WSH, D), F16)
        globals_by_name["vones"] = np.empty((NCORES * NQT, 128), np.float32)
    def pack_core(core):
        b, half = core // 2, core % 2
        s0 = half * SH

        if x_int8:
            # own rows only; halos are exchanged between core pairs on device
            blob = gblob[core]
            blob[OFF_XQ:OFF_XK] = qx[b, s0:s0 + SH].T.reshape(-1)
            blob[OFF_XK:OFF_XV] = kx[b, s0:s0 + SH].T.reshape(-1)
            blob[OFF_XV:OFF_W] = vx[b, s0:s0 + SH].T.reshape(-1)
            if W12:
                blob[OFF_W:OFF_SC] = wpacked[WSH * core:WSH * (core + 1)].reshape(-1)
                blob[OFF_SC:OFF_V] = wscales
            else:
                blob[OFF_W:OFF_V] = wstack[WSH * core:WSH * (core + 1)].view(np.int8).reshape(-1)
            blob[OFF_V:OFF_FL] = _vones_cache[half].astype(np.int8).reshape(-1)
            fl = np.empty((128, 2), np.float32)
            fl[:, 0] = half          # keep left halo only on odd (2nd) half
            fl[:, 1] = 1 - half      # keep right halo only on even half
            blob[OFF_FL:BLOB] = fl.view(np.int8).reshape(-1)
        else:
            xq = qx[b, s0:s0 + SH]
            lo, hi = s0 - W, s0 + SH + W
            clo, chi = max(lo, 0), min(hi, S)
            xk = np.zeros((PADK, D), xdt)
            xv = np.zeros((PADK, D), xdt)
            xk[clo - lo:chi - lo] = kx[b, clo:chi]
            xv[clo - lo:chi - lo] = vx[b, clo:chi]
            wchunk = wstack[WSH * core:WSH * (core + 1)]
            globals_by_name["xqt"][core * D:(core + 1) * D] = xq.T
            globals_by_name["xkt"][core * D:(core + 1) * D] = xk.T
            globals_by_name["xvt"][core * D:(core + 1) * D] = xv.T
            globals_by_name["wchunk"][core * WSH:(core + 1) * WSH] = wchunk
            globals_by_name["vones"][core * NQT:(core + 1) * NQT] = _vones_cache[half]

    list(_host_pool.map(pack_core, range(NCORES)))
    if has_b:
        def rep(name, arr):
            g = np.empty((NCORES * arr.shape[0], *arr.shape[1:]), arr.dtype)
            g[:] = np.tile(arr, (NCORES,) + (1,) * (arr.ndim - 1))
            globals_by_name[name] = g
        rep("bqc", np.ascontiguousarray((bq * SCALE).reshape(4, 128)))
        rep("bkc", np.ascontiguousarray(bk.reshape(4, 128)))
        rep("bvb", np.broadcast_to(bv, (128, D)).astype(np.float32))
        rep("bob", np.broadcast_to(bo, (128, D)).astype(np.float32))
    if x_int8:
        globals_by_name["blob"] = gblob.reshape(-1)

    import time as _time
    run = _get_runner(nc)
    concat_in = [globals_by_name[nm] for nm in run.in_names]
    outs = run(concat_in)
    if TRACE:
        best = None
        for _ in range(5):
            t0 = _time.perf_counter()
            outs = run(concat_in)
            dtns = (_time.perf_counter() - t0) * 1e9
            best = dtns if best is None else min(best, dtns)
        LAST["wall_ns"] = best
    LAST["exec_time_ns"] = None

    ow = D + 4 if out_int8 else D
    oglob = outs[0].reshape(NCORES, SH, ow)
    out = np.empty((B, S, D), np.float32)

    def unpack_core(core):
        b, half = core // 2, core % 2
        o = oglob[core]
        dst = out[b, half * SH:(half + 1) * SH]
        if out_int8:
            scale = np.ascontiguousarray(o[:, D:D + 4]).view(np.float32)
            np.multiply(o[:, 0:D], scale, out=dst, dtype=np.float32)
        else:
            dst[:] = o

    list(_host_pool.map(unpack_core, range(NCORES)))
    return out


if __name__ == "__main__":
    rng = np.random.default_rng(0)
    sc = 1.0 / np.sqrt(D)
    inputs = {
        "query": rng.standard_normal((B, S, D)).astype(np.float32),
        "key": rng.standard_normal((B, S, D)).astype(np.float32),
        "value": rng.standard_normal((B, S, D)).astype(np.float32),
        "Wq": (rng.standard_normal((D, D)) * sc).astype(np.float32),
        "bq": np.zeros(D, np.float32),
        "Wk": (rng.standard_normal((D, D)) * sc).astype(np.float32),
        "bk": np.zeros(D, np.float32),
        "Wv": (rng.standard_normal((D, D)) * sc).astype(np.float32),
        "bv": np.zeros(D, np.float32),
        "Wo": (rng.standard_normal((D, D)) * sc).astype(np.float32),
        "bo": np.zeros(D, np.float32),
    }
    out = kernel(**inputs)
    print("out", out.shape, out.dtype, out[0, 0, :4])

